# revision 1
# baseline (speedup 1.0000x reference)
"""DGCNN part-segmentation forward pass on 8 Trainium2 NeuronCores.

Sharding: data-parallel over the batch (B=4) x 2-way split of the N=4096
points within each batch element.  Core c handles batch element c//2,
point rows [(c%2)*2048, (c%2+1)*2048).  The two cores of a pair exchange
their half of each EdgeConv output with a pairwise AllGather (and a
pairwise AllReduce-max for the global pooling feature).

Device algorithm per EdgeConv layer (transform-then-gather):
  dist  : one fused matmul  s[i,j] = 2<x_i,x_j> - |x_j|^2  (row-rank equal
          to the reference's negative squared distance)
  top-20: per 128-row tile: 16x chunk-max8 (DVE Max) -> 128 candidates,
          3 peeling rounds (max8 + match_replace) -> top-24 values,
          3x max_index over the full row -> global indices (first 20 used)
  gather: GPSIMD ap_gather of the u = U x transform (per neighbor slot k)
  edge   : psum = I*u_gathered + (V/2)*(2 x_i)  (two matmuls), then
           LeakyReLU(. + c) on the scalar engine (Prelu, alpha=0.2)
  conv2  : 64x64 matmul + LeakyReLU epilogue (layers 1, 2)
  k-max  : running tensor_max over the 20 neighbor slots (DVE)
"""

import sys

sys.path.insert(0, "/opt/trn_rl_repo")

import numpy as np

B = 4
N = 4096
H = 2048  # points per core (half of a batch element)
KNN = 20
EPS = 1e-5
NEG = -3.0e38

_CACHE = {}


# --------------------------------------------------------------------------
# host-side weight preparation
# --------------------------------------------------------------------------

def _fold_bn(w, b, g, be):
    s = (g / np.sqrt(np.float32(1.0) + np.float32(EPS))).astype(np.float32)
    return (w * s[:, None]).astype(np.float32), (s * b + be).astype(np.float32)


def _prep_weights(inp):
    f = np.float32
    W, C = {}, {}
    for i in range(1, 9):
        W[i], C[i] = _fold_bn(
            inp["w%d" % i], inp["b%d" % i], inp["g%d" % i], inp["be%d" % i]
        )
    d = {}
    # edge conv layers: split into U (neighbor part) and V (center part)
    for lid, wi in ((1, 1), (2, 3), (3, 5)):
        w = W[wi]
        cin = w.shape[1] // 2
        U = w[:, :cin]
        V = w[:, cin:] - w[:, :cin]
        d["u%dT" % lid] = np.ascontiguousarray(U.T)
        d["v%dTh" % lid] = np.ascontiguousarray((V / f(2.0)).T)
        d["c%d" % lid] = C[wi].reshape(64, 1)
    d["w2T"] = np.ascontiguousarray(W[2].T)
    d["cc2"] = C[2].reshape(64, 1)
    d["w4T"] = np.ascontiguousarray(W[4].T)
    d["cc4"] = C[4].reshape(64, 1)
    # conv6 (192 -> 1024)
    w6T = np.ascontiguousarray(W[6].T)  # (192, 1024)
    d["w6aT"] = np.ascontiguousarray(w6T[:128])
    d["w6bT"] = np.ascontiguousarray(w6T[128:])
    d["c6v"] = np.ascontiguousarray(C[6].reshape(8, 128).T)  # (128, 8)
    # conv7 (1216 -> 512): xg part (1024) and local part (192)
    w7 = W[7]
    w7gT = np.ascontiguousarray(w7[:, :1024].T)  # (1024, 512)
    d["w7gT"] = np.ascontiguousarray(w7gT.reshape(8, 128, 512).transpose(1, 0, 2))
    w7lT = np.ascontiguousarray(w7[:, 1024:].T)  # (192, 512)
    d["w7laT"] = np.ascontiguousarray(w7lT[:128])
    d["w7lbT"] = np.ascontiguousarray(w7lT[128:])
    d["c7v"] = np.ascontiguousarray(C[7].reshape(4, 128).T)  # (128, 4)
    # conv8 (512 -> 256)
    w8T = np.ascontiguousarray(W[8].T)  # (512, 256)
    d["w8T"] = np.ascontiguousarray(w8T.reshape(4, 128, 256).transpose(1, 0, 2))
    d["c8v"] = np.ascontiguousarray(C[8].reshape(2, 128).T)  # (128, 2)
    # conv9 (256 -> 13), plain linear
    w9T = np.ascontiguousarray(inp["w9"].T.astype(f))  # (256, 13)
    d["w9T"] = np.ascontiguousarray(w9T.reshape(2, 128, 13).transpose(1, 0, 2))
    d["b9v"] = inp["b9"].astype(f).reshape(13, 1)
    # constants
    d["id64"] = np.eye(64, dtype=f)
    d["ones3"] = np.ones((3, 1), dtype=f)
    d["ones64"] = np.ones((64, 1), dtype=f)
    return d


_WEIGHT_SPECS = [
    ("u1T", (3, 64)), ("v1Th", (3, 64)), ("c1", (64, 1)),
    ("w2T", (64, 64)), ("cc2", (64, 1)),
    ("u2T", (64, 64)), ("v2Th", (64, 64)), ("c2", (64, 1)),
    ("w4T", (64, 64)), ("cc4", (64, 1)),
    ("u3T", (64, 64)), ("v3Th", (64, 64)), ("c3", (64, 1)),
    ("w6aT", (128, 1024)), ("w6bT", (64, 1024)), ("c6v", (128, 8)),
    ("w7gT", (128, 8, 512)), ("w7laT", (128, 512)), ("w7lbT", (64, 512)),
    ("c7v", (128, 4)),
    ("w8T", (128, 4, 256)), ("c8v", (128, 2)),
    ("w9T", (128, 2, 13)), ("b9v", (13, 1)),
    ("id64", (64, 64)), ("ones3", (3, 1)), ("ones64", (64, 1)),
]


# --------------------------------------------------------------------------
# device kernel builder
# --------------------------------------------------------------------------

def build_kernel():
    import concourse.bacc as bacc
    import concourse.mybir as mybir
    import concourse.tile as tile

    f32 = mybir.dt.float32
    f32r = mybir.dt.float32r
    i16 = mybir.dt.int16
    AF = mybir.ActivationFunctionType

    def R(ap):
        # float32r needs fp32r-rounded producers end-to-end; disabled.
        return ap
    PAIRS = [[0, 1], [2, 3], [4, 5], [6, 7]]

    nc = bacc.Bacc("TRN2", target_bir_lowering=False, num_devices=8)

    u16 = mybir.dt.uint16
    pts_full = nc.dram_tensor("pts_full", [3, N], f32, kind="ExternalInput")
    choff_d = nc.dram_tensor("choff", [128, 128], u16, kind="ExternalInput")
    pts_mine = nc.dram_tensor("pts_mine", [3, H], f32, kind="ExternalInput")
    wdram = {}
    for name, shape in _WEIGHT_SPECS:
        wdram[name] = nc.dram_tensor(name, list(shape), f32, kind="ExternalInput")
    out_d = nc.dram_tensor("out", [13, H], f32, kind="ExternalOutput")

    with tile.TileContext(nc) as tc:
        with (
            tc.tile_pool(name="wp", bufs=1) as wp,
            tc.tile_pool(name="per", bufs=1) as per,
            tc.tile_pool(name="psd", bufs=4, space="PSUM") as psd,
            tc.tile_pool(name="pse", bufs=2, space="PSUM") as pse,
            tc.tile_pool(name="dram", bufs=1, space="DRAM") as drp,
        ):
            # ---- load weights ----
            wsb = {}
            for name, shape in _WEIGHT_SPECS:
                t = wp.tile(list(shape), f32, tag=name, name="w_" + name)
                nc.sync.dma_start(t, wdram[name][:])
                wsb[name] = t

            # ---- persistent tiles ----
            rhsF = per.tile([65, N], f32, tag="rhsF", name="rhsF")
            lhsTm = per.tile([65, H], f32, tag="lhsTm", name="lhsTm")
            u_t = per.tile([64, N], f32, tag="u", name="u_t")
            xcat_a = per.tile([128, H], f32, tag="xcata", name="xcat_a")
            xcat_b = per.tile([64, H], f32, tag="xcatb", name="xcat_b")
            widx = per.tile([64, KNN * 128], i16, tag="widx", name="widx")
            acc3 = per.tile([64, H], f32, tag="acc3", name="acc3")
            x2acc = per.tile([64, H], f32, tag="x2acc", name="x2acc")
            xg_sb = per.tile([128, 8], f32, tag="xg", name="xg_sb")
            b7_sb = per.tile([128, 4], f32, tag="b7", name="b7_sb")

            idx_dram = drp.tile([H, KNN], i16, tag="idxd", name="idx_dram")
            choff_sb = per.tile([128, 128], u16, tag="choff", name="choff_sb")
            nc.sync.dma_start(choff_sb, choff_d[:])
            zero128 = per.tile([128, 128], f32, tag="z128", name="zero128")
            nc.vector.memset(zero128, 0.0)

            x1h = xcat_a[0:64]
            x2h = xcat_a[64:128]
            x3h = xcat_b

            with (
                tc.tile_pool(name="dsb", bufs=3) as dsbp,
                tc.tile_pool(name="tk", bufs=4) as tkp,
                tc.tile_pool(name="gp", bufs=2) as gp,
            ):
                def prep_sq_and_u(cin, ones_sb, uT_sb):
                    """rhsF[64] = -sum_c rhsF[c]^2 ; u = uT.T @ rhsF[0:cin]."""
                    xsq = dsbp.tile([64, N], f32, tag="dsb", name="xsq")[0:cin]
                    nc.scalar.activation(xsq, rhsF[0:cin], AF.Square)
                    sqrow = dsbp.tile([1, N], f32, tag="dsb", name="sqrow")
                    for j in range(8):
                        sl = slice(j * 512, (j + 1) * 512)
                        pq = psd.tile([1, 512], f32, tag="d", name="pq")
                        nc.tensor.matmul(pq, R(ones_sb), R(xsq[:, sl]))
                        nc.scalar.mul(sqrow[:, sl], pq, -1.0)
                        pu = psd.tile([64, 512], f32, tag="d", name="pu")
                        nc.tensor.matmul(pu, R(uT_sb), R(rhsF[0:cin, sl]))
                        nc.scalar.copy(u_t[:, sl], pu)
                    nc.sync.dma_start(rhsF[64:65], sqrow)

                def topk_phase(cin, grp):
                    """distances + top-20 indices for 1024 rows of group grp."""
                    for t in range(grp * 8, grp * 8 + 8):
                        dsb = dsbp.tile([128, N], f32, tag="dsb", name="dsb")
                        for j in range(8):
                            sl = slice(j * 512, (j + 1) * 512)
                            pd = psd.tile([128, 512], f32, tag="d", name="pd")
                            nc.tensor.matmul(
                                pd,
                                R(lhsTm[:, t * 128 : (t + 1) * 128]),
                                R(rhsF[:, sl]),
                            )
                            nc.scalar.copy(dsb[:, sl], pd)
                        cand = tkp.tile([128, 128], f32, tag="cand", name="cand")
                        cidx = tkp.tile([128, 128], mybir.dt.uint16, tag="cidx",
                                        name="cidx")
                        for c in range(16):
                            nc.vector.max(
                                out=cand[:, c * 8 : (c + 1) * 8],
                                in_=dsb[:, c * 256 : (c + 1) * 256],
                            )
                        for c in range(16):
                            nc.vector.max_index(
                                cidx[:, c * 8 : (c + 1) * 8],
                                cand[:, c * 8 : (c + 1) * 8],
                                dsb[:, c * 256 : (c + 1) * 256],
                            )
                        # chunk-local -> global indices
                        nc.vector.tensor_add(cidx, cidx, choff_sb)
                        candw = tkp.tile([128, 128], f32, tag="candw", name="candw")
                        nc.scalar.copy(candw, cand)
                        t8 = tkp.tile([128, 24], f32, tag="t8", name="t8")
                        nc.vector.max(out=t8[:, 0:8], in_=candw)
                        nc.vector.match_replace(
                            out=candw, in_to_replace=t8[:, 0:8], in_values=candw,
                            imm_value=NEG,
                        )
                        nc.vector.max(out=t8[:, 8:16], in_=candw)
                        nc.vector.match_replace(
                            out=candw, in_to_replace=t8[:, 8:16], in_values=candw,
                            imm_value=NEG,
                        )
                        nc.vector.max(out=t8[:, 16:24], in_=candw)
                        # rank slots: mask of top-20 -> prefix-sum compaction
                        mask = tkp.tile([128, 128], f32, tag="mask", name="mask")
                        nc.vector.tensor_scalar(
                            mask, cand, t8[:, 19:20], None,
                            op0=mybir.AluOpType.is_ge,
                        )
                        cums = tkp.tile([128, 128], f32, tag="cums", name="cums")
                        nc.vector.tensor_tensor_scan(
                            cums, mask, zero128, 0.0,
                            op0=mybir.AluOpType.add, op1=mybir.AluOpType.add,
                        )
                        nc.vector.tensor_mul(cums, cums, mask)
                        nc.vector.tensor_scalar_add(cums, cums, -1.0)
                        slot = tkp.tile([128, 128], i16, tag="slot", name="slot")
                        nc.vector.tensor_copy(slot, cums)
                        sel = tkp.tile([128, 24], mybir.dt.uint16, tag="sel",
                                       name="sel")
                        nc.gpsimd.local_scatter(
                            out_ap=sel,
                            data_ap=cidx,
                            idxs_ap=slot,
                            channels=128,
                            num_elems=24,
                            num_idxs=128,
                        )
                        nc.sync.dma_start(
                            idx_dram[t * 128 : (t + 1) * 128, :],
                            sel[:, 0:KNN].bitcast(i16),
                        )
                    # wrapped-mod-16 reformat for ap_gather (group grp):
                    # widx[p, G*grp + k*64 + r] = idx_dram[1024*grp + 16*r + p, k]
                    G = KNN * 64
                    src = idx_dram[grp * 1024 : (grp + 1) * 1024, :].rearrange(
                        "(r p) k -> p k r", p=16
                    )
                    for rep in range(4):
                        dst = widx[rep * 16 : (rep + 1) * 16,
                                   grp * G : (grp + 1) * G].rearrange(
                            "p (k r) -> p k r", r=64
                        )
                        nc.sync.dma_start(dst, src)

                def edge_phase(lid, cin, vTh_sb, c_ap, w2T_sb, c2_ap, x_out,
                               grp):
                    G = KNN * 64
                    gsl = slice(grp * 1024, (grp + 1) * 1024)
                    for k in range(KNN):
                        if lid == 3 and k == 0:
                            g = acc3[:, gsl]
                        else:
                            g = gp.tile([64, 1024], f32, tag="g", name="g", bufs=3)
                        nc.gpsimd.ap_gather(
                            out_ap=g,
                            in_ap=u_t,
                            idxs_ap=widx[:, grp * G + k * 64 : grp * G + (k + 1) * 64],
                            channels=64,
                            num_elems=N,
                            d=1,
                            num_idxs=1024,
                        )
                        if lid == 3:
                            if k > 0:
                                nc.vector.tensor_max(acc3[:, gsl], acc3[:, gsl], g)
                            continue
                        y = gp.tile([64, 1024], f32, tag="y", name="y")
                        for q in range(2):
                            sl = slice(q * 512, (q + 1) * 512)
                            msl = slice(grp * 1024 + q * 512,
                                        grp * 1024 + (q + 1) * 512)
                            pe_ = pse.tile([64, 512], f32, tag="e", name="pe")
                            nc.tensor.matmul(pe_, R(wsb["id64"]), R(g[:, sl]),
                                             start=True, stop=False)
                            nc.tensor.matmul(pe_, R(vTh_sb),
                                             R(lhsTm[0:cin, msl]),
                                             start=False, stop=True)
                            nc.scalar.activation(y[:, sl], pe_, AF.Prelu,
                                                 bias=c_ap, alpha=0.2)
                        z = (x_out[:, gsl] if k == 0 else
                             gp.tile([64, 1024], f32, tag="z", name="z"))
                        for q in range(2):
                            sl = slice(q * 512, (q + 1) * 512)
                            pc = pse.tile([64, 512], f32, tag="c2", name="pc")
                            nc.tensor.matmul(pc, R(w2T_sb), R(y[:, sl]))
                            nc.scalar.activation(z[:, sl], pc, AF.Prelu,
                                                 bias=c2_ap, alpha=0.2)
                        if k > 0:
                            nc.vector.tensor_max(x_out[:, gsl], x_out[:, gsl], z)
                    if lid == 3:
                        # x3 = Lrelu(max_k(u_j) + V x_i + c)  (monotone)
                        for q in range(2):
                            sl = slice(grp * 1024 + q * 512,
                                       grp * 1024 + (q + 1) * 512)
                            pe_ = pse.tile([64, 512], f32, tag="e", name="pe")
                            nc.tensor.matmul(pe_, R(wsb["id64"]),
                                             R(acc3[:, sl]),
                                             start=True, stop=False)
                            nc.tensor.matmul(pe_, R(vTh_sb),
                                             R(lhsTm[0:cin, sl]),
                                             start=False, stop=True)
                            nc.scalar.activation(x_out[:, sl], pe_, AF.Prelu,
                                                 bias=c_ap, alpha=0.2)

                def allgather_x(x_half):
                    """x_half (64, H) -> rhsF[0:64] = full (64, N), pair AG."""
                    ccin = drp.tile([64, H], f32, tag="ccin", name="ccin")
                    nc.sync.dma_start(ccin, x_half)
                    ccout = drp.tile([128, H], f32, tag="ccout", name="ccout")
                    nc.gpsimd.collective_compute(
                        "AllGather",
                        mybir.AluOpType.bypass,
                        replica_groups=PAIRS,
                        ins=[ccin],
                        outs=[ccout],
                    )
                    nc.sync.dma_start(
                        rhsF[0:64].rearrange("c (h e) -> c h e", h=2),
                        ccout.rearrange("(h c) e -> c h e", c=64),
                    )

                # ================= layer 1 =================
                nc.vector.memset(rhsF[0:64], 0.0)
                nc.sync.dma_start(rhsF[0:3], pts_full[:])
                tmp3 = gp.tile([3, H], f32, tag="tmp3", name="tmp3", bufs=1)
                nc.sync.dma_start(tmp3, pts_mine[:])
                nc.vector.memset(lhsTm[0:64], 0.0)
                nc.scalar.mul(lhsTm[0:3], tmp3, 2.0)
                nc.vector.memset(lhsTm[64:65], 1.0)
                prep_sq_and_u(3, wsb["ones3"], wsb["u1T"])
                for grp in range(2):
                    topk_phase(3, grp)
                    edge_phase(1, 3, wsb["v1Th"], wsb["c1"], wsb["w2T"],
                               wsb["cc2"], x1h, grp)

                # ================= layer 2 =================
                allgather_x(x1h)
                nc.scalar.mul(lhsTm[0:64], x1h, 2.0)
                nc.vector.memset(lhsTm[64:65], 1.0)
                prep_sq_and_u(64, wsb["ones64"], wsb["u2T"])
                for grp in range(2):
                    topk_phase(64, grp)
                    edge_phase(2, 64, wsb["v2Th"], wsb["c2"], wsb["w4T"],
                               wsb["cc4"], x2acc, grp)
                nc.sync.dma_start(x2h, x2acc)

                # ================= layer 3 =================
                allgather_x(x2acc)
                nc.scalar.mul(lhsTm[0:64], x2acc, 2.0)
                nc.vector.memset(lhsTm[64:65], 1.0)
                prep_sq_and_u(64, wsb["ones64"], wsb["u3T"])
                for grp in range(2):
                    topk_phase(64, grp)
                    edge_phase(3, 64, wsb["v3Th"], wsb["c3"], None, None, x3h,
                               grp)

                # ================= conv6 + global max pool =================
                for ob in range(8):
                    obs = slice(ob * 128, (ob + 1) * 128)
                    xgt = tkp.tile([128, 4], f32, tag="xgt", name="xgt")
                    for q in range(4):
                        sl = slice(q * 512, (q + 1) * 512)
                        pf = psd.tile([128, 512], f32, tag="d", name="pf6")
                        nc.tensor.matmul(pf, R(wsb["w6aT"][:, obs]),
                                         R(xcat_a[:, sl]),
                                         start=True, stop=False)
                        nc.tensor.matmul(pf, R(wsb["w6bT"][:, obs]),
                                         R(xcat_b[:, sl]),
                                         start=False, stop=True)
                        h6 = gp.tile([128, 512], f32, tag="g", name="h6", bufs=3)
                        nc.scalar.activation(h6, pf, AF.Prelu,
                                             bias=wsb["c6v"][:, ob : ob + 1],
                                             alpha=0.2)
                        nc.vector.reduce_max(xgt[:, q : q + 1], h6,
                                             axis=mybir.AxisListType.X)
                    nc.vector.reduce_max(xg_sb[:, ob : ob + 1], xgt,
                                         axis=mybir.AxisListType.X)

            # layer scratch pools released here; final stage below.
            ccg_in = drp.tile([128, 8], f32, tag="ccgi", name="ccg_in")
            nc.sync.dma_start(ccg_in, xg_sb)
            ccg_out = drp.tile([128, 8], f32, tag="ccgo", name="ccg_out")
            nc.gpsimd.collective_compute(
                "AllReduce",
                mybir.AluOpType.max,
                replica_groups=PAIRS,
                ins=[ccg_in],
                outs=[ccg_out],
            )
            nc.sync.dma_start(xg_sb, ccg_out)

            # conv7 effective bias: c7 + W7g @ xg
            for ob in range(4):
                pb = psd.tile([128, 1], f32, tag="d", name="pb7")
                for kb in range(8):
                    nc.tensor.matmul(
                        pb,
                        wsb["w7gT"][:, kb, ob * 128 : (ob + 1) * 128],
                        xg_sb[:, kb : kb + 1],
                        start=(kb == 0),
                        stop=(kb == 7),
                    )
                nc.scalar.activation(b7_sb[:, ob : ob + 1], pb, AF.Identity,
                                     bias=wsb["c7v"][:, ob : ob + 1])

            with tc.tile_pool(name="fin", bufs=1) as fin:
                h7 = fin.tile([128, 4 * H], f32, tag="h7", name="h7")
                for ob in range(4):
                    obs = slice(ob * 128, (ob + 1) * 128)
                    for q in range(4):
                        sl = slice(q * 512, (q + 1) * 512)
                        pf = psd.tile([128, 512], f32, tag="d", name="pf7")
                        nc.tensor.matmul(pf, R(wsb["w7laT"][:, obs]),
                                         R(xcat_a[:, sl]),
                                         start=True, stop=False)
                        nc.tensor.matmul(pf, R(wsb["w7lbT"][:, obs]),
                                         R(xcat_b[:, sl]),
                                         start=False, stop=True)
                        nc.scalar.activation(
                            h7[:, ob * H + q * 512 : ob * H + (q + 1) * 512], pf,
                            AF.Prelu, bias=b7_sb[:, ob : ob + 1], alpha=0.2,
                        )
                h8 = fin.tile([128, 2 * H], f32, tag="h8", name="h8")
                for ob in range(2):
                    for q in range(4):
                        pf = psd.tile([128, 512], f32, tag="d", name="pf8")
                        for kb in range(4):
                            nc.tensor.matmul(
                                pf,
                                R(wsb["w8T"][:, kb, ob * 128 : (ob + 1) * 128]),
                                R(h7[:, kb * H + q * 512 : kb * H + (q + 1) * 512]),
                                start=(kb == 0),
                                stop=(kb == 3),
                            )
                        nc.scalar.activation(
                            h8[:, ob * H + q * 512 : ob * H + (q + 1) * 512], pf,
                            AF.Prelu, bias=wsb["c8v"][:, ob : ob + 1], alpha=0.2,
                        )
                o_sb = fin.tile([13, H], f32, tag="osb", name="o_sb")
                for q in range(4):
                    sl = slice(q * 512, (q + 1) * 512)
                    pf = psd.tile([13, 512], f32, tag="d", name="pf9")
                    for kb in range(2):
                        nc.tensor.matmul(
                            pf,
                            R(wsb["w9T"][:, kb, :]),
                            R(h8[:, kb * H + q * 512 : kb * H + (q + 1) * 512]),
                            start=(kb == 0),
                            stop=(kb == 1),
                        )
                    nc.scalar.activation(o_sb[:, sl], pf, AF.Identity,
                                         bias=wsb["b9v"])
                nc.sync.dma_start(out_d[:], o_sb)

    nc.compile()
    return nc


def make_in_maps(inputs):
    """Per-core input dicts from the full problem inputs."""
    wd = _prep_weights(inputs)
    pts = np.asarray(inputs["points"], dtype=np.float32)
    in_maps = []
    for c in range(8):
        b, h = c // 2, c % 2
        m = {name: np.ascontiguousarray(wd[name]) for name, _ in _WEIGHT_SPECS}
        m["choff"] = np.ascontiguousarray(
            np.tile(np.repeat(np.arange(16, dtype=np.uint16) * 256, 8), (128, 1)))
        m["pts_full"] = np.ascontiguousarray(pts[b])
        m["pts_mine"] = np.ascontiguousarray(pts[b][:, h * H : (h + 1) * H])
        in_maps.append(m)
    return in_maps


def kernel(**inputs):
    from concourse.bass_utils import run_bass_kernel_spmd

    if "nc" not in _CACHE:
        _CACHE["nc"] = build_kernel()
    nc = _CACHE["nc"]
    in_maps = make_in_maps(inputs)
    res = run_bass_kernel_spmd(nc, in_maps, core_ids=list(range(8)))
    out = np.zeros((B, 13, N), dtype=np.float32)
    for c in range(8):
        b, h = c // 2, c % 2
        out[b][:, h * H : (h + 1) * H] = res.results[c]["out"]
    return out



# revision 40
# speedup vs baseline: 1.3897x; 1.3897x over previous
"""DGCNN part-segmentation forward pass on 8 Trainium2 NeuronCores.

Sharding: data-parallel over the batch (B=4) x 2-way split of the N=4096
points within each batch element.  Core c handles batch element c//2,
point rows [(c%2)*2048, (c%2+1)*2048).  The two cores of a pair exchange
their half of each EdgeConv output with pairwise AllGathers (one per
1024-point group, overlapped with the other group's compute) and two
pairwise AllReduce-max halves for the global pooling feature.

The emission order is software-pipelined: every engine queue is
in-order, so distance/top-k work for one point group is interleaved
instruction-by-instruction with edge-conv work for the other group, and
the first half of the next layer's distance tiles is interleaved with
the current layer's second edge phase.

Device algorithm per EdgeConv layer:
  dist  : fp32 matmul  s[i,j] = 2<x_i,x_j> - |x_j|^2  (row-rank equal to
          the reference's negative squared distance; fp32 inputs so the
          neighbor ranking is exact for the fp32r-rounded features)
  top-20: per 128-row tile: 8x top-8 per 512-column chunk (DVE Max) ->
          64 candidates; 3 peeling rounds (max8 + match_replace) ->
          top-24 values; threshold mask + prefix-sum compaction (gpsimd)
          + gpsimd local_scatter -> global indices of the top-20
  gather: batched GPSIMD ap_gather (4 neighbor slots = 4096 indices per
          call) of the fp32r u = U x transform
  edge   : psum = I*u_gathered + V*x_i  (fp32r matmuls, 4x PE rate),
          LeakyReLU (Prelu) on the scalar engine, conv2 fp32r matmul;
          the k-max runs on the raw conv2 psums (LeakyReLU is monotone)
          and one Prelu finalizes each group.
"""

import sys

sys.path.insert(0, "/opt/trn_rl_repo")

import numpy as np

B = 4
N = 4096
H = 2048  # points per core (half of a batch element)
KNN = 20
EPS = 1e-5
NEG = -3.0e38
G = KNN * 64  # widx columns per point group

_CACHE = {}


# --------------------------------------------------------------------------
# host-side weight preparation
# --------------------------------------------------------------------------

def _r11(x):
    """Round fp32 -> fp32r (11 explicit mantissa bits, RNE)."""
    b = np.ascontiguousarray(x, dtype=np.float32).view(np.uint32)
    low = b & np.uint32(0xFFF)
    base = b & np.uint32(0xFFFFF000)
    half = np.uint32(0x800)
    rup = (low > half) | ((low == half) & (((b >> 12) & np.uint32(1)) == 1))
    out = base + np.where(rup, np.uint32(0x1000), np.uint32(0))
    return out.view(np.float32)


def _fold_bn(w, b, g, be):
    s = (g / np.sqrt(np.float32(1.0) + np.float32(EPS))).astype(np.float32)
    return (w * s[:, None]).astype(np.float32), (s * b + be).astype(np.float32)


def _prep_weights(inp):
    f = np.float32
    W, C = {}, {}
    for i in range(1, 9):
        W[i], C[i] = _fold_bn(
            inp["w%d" % i], inp["b%d" % i], inp["g%d" % i], inp["be%d" % i]
        )
    d = {}
    # edge conv layers: split into U (neighbor part) and V (center part).
    # Layer 1's x2r tile holds raw points, layers 2-3 hold 2*x, so the V
    # transpose is halved only for layers 2-3.
    for lid, wi, vscale in ((1, 1, 1.0), (2, 3, 0.5), (3, 5, 0.5)):
        w = W[wi]
        cin = w.shape[1] // 2
        U = w[:, :cin]
        V = w[:, cin:] - w[:, :cin]
        d["u%dT" % lid] = np.ascontiguousarray(U.T)
        d["v%dTh" % lid] = _r11(np.ascontiguousarray((V * f(vscale)).T))
        d["c%d" % lid] = C[wi].reshape(64, 1)
    d["w2T"] = _r11(np.ascontiguousarray(W[2].T))
    d["cc2"] = C[2].reshape(64, 1)
    d["w4T"] = _r11(np.ascontiguousarray(W[4].T))
    d["cc4"] = C[4].reshape(64, 1)
    # conv6 (192 -> 1024)
    w6T = np.ascontiguousarray(W[6].T)  # (192, 1024)
    d["w6aT"] = _r11(np.ascontiguousarray(w6T[:128]))
    d["w6bT"] = _r11(np.ascontiguousarray(w6T[128:]))
    d["c6v"] = np.ascontiguousarray(C[6].reshape(8, 128).T)  # (128, 8)
    # conv7 (1216 -> 512): xg part (1024) and local part (192)
    w7 = W[7]
    w7gT = np.ascontiguousarray(w7[:, :1024].T)  # (1024, 512)
    d["w7gT"] = np.ascontiguousarray(w7gT.reshape(8, 128, 512).transpose(1, 0, 2))
    w7lT = np.ascontiguousarray(w7[:, 1024:].T)  # (192, 512)
    d["w7laT"] = _r11(np.ascontiguousarray(w7lT[:128]))
    d["w7lbT"] = _r11(np.ascontiguousarray(w7lT[128:]))
    d["c7v"] = np.ascontiguousarray(C[7].reshape(4, 128).T)  # (128, 4)
    # conv8 (512 -> 256)
    w8T = np.ascontiguousarray(W[8].T)  # (512, 256)
    d["w8T"] = _r11(np.ascontiguousarray(w8T.reshape(4, 128, 256).transpose(1, 0, 2)))
    d["c8v"] = np.ascontiguousarray(C[8].reshape(2, 128).T)  # (128, 2)
    # conv9 (256 -> 13), plain linear
    w9T = np.ascontiguousarray(inp["w9"].T.astype(f))  # (256, 13)
    d["w9T"] = _r11(np.ascontiguousarray(w9T.reshape(2, 128, 13).transpose(1, 0, 2)))
    d["b9v"] = inp["b9"].astype(f).reshape(13, 1)
    # constants
    d["id64"] = np.eye(64, dtype=f)
    d["ones3"] = np.ones((3, 1), dtype=f)
    d["ones64"] = np.ones((64, 1), dtype=f)
    return d


# name -> (shape, is_f32r)
_WEIGHT_SPECS = [
    ("u1T", (3, 64), 0), ("v1Th", (3, 64), 1), ("c1", (64, 1), 0),
    ("w2T", (64, 64), 1), ("cc2", (64, 1), 0),
    ("u2T", (64, 64), 0), ("v2Th", (64, 64), 1), ("c2", (64, 1), 0),
    ("w4T", (64, 64), 1), ("cc4", (64, 1), 0),
    ("u3T", (64, 64), 0), ("v3Th", (64, 64), 1), ("c3", (64, 1), 0),
    ("w6aT", (128, 1024), 1), ("w6bT", (64, 1024), 1), ("c6v", (128, 8), 0),
    ("w7gT", (128, 8, 512), 0), ("w7laT", (128, 512), 1),
    ("w7lbT", (64, 512), 1), ("c7v", (128, 4), 0),
    ("w8T", (128, 4, 256), 1), ("c8v", (128, 2), 0),
    ("w9T", (128, 2, 13), 1), ("b9v", (13, 1), 0),
    ("id64", (64, 64), 1), ("ones3", (3, 1), 0), ("ones64", (64, 1), 0),
]


def _mix(a_steps, b_steps, lead=0):
    """Emit `lead` a-steps, then alternate a/b 1:1 until b is exhausted, then
    the remaining a-steps.  Front-loads the (shorter) b stream so its tail
    dependencies (e.g. the AllGather launch) fire as early as possible."""
    ia = 0
    for _ in range(min(lead, len(a_steps))):
        a_steps[ia]()
        ia += 1
    for ib in range(len(b_steps)):
        if ia < len(a_steps):
            a_steps[ia]()
            ia += 1
        b_steps[ib]()
    while ia < len(a_steps):
        a_steps[ia]()
        ia += 1


# --------------------------------------------------------------------------
# device kernel builder
# --------------------------------------------------------------------------

def build_kernel():
    import concourse.bacc as bacc
    import concourse.mybir as mybir
    import concourse.tile as tile

    import os

    f32 = mybir.dt.float32
    f32r = (mybir.dt.float32 if os.environ.get("BASSK_NO_F32R")
            else mybir.dt.float32r)
    i16 = mybir.dt.int16
    u16 = mybir.dt.uint16
    AF = mybir.ActivationFunctionType
    ALU = mybir.AluOpType
    PAIRS = [[0, 1], [2, 3], [4, 5], [6, 7]]

    nc = bacc.Bacc("TRN2", target_bir_lowering=False, num_devices=8)

    pts_full = nc.dram_tensor("pts_full", [3, N], f32, kind="ExternalInput")
    choff_d = nc.dram_tensor("choff", [128, 64], u16, kind="ExternalInput")
    pts_mine = nc.dram_tensor("pts_mine", [3, H], f32, kind="ExternalInput")
    out_d = nc.dram_tensor("out", [13, H], f32, kind="ExternalOutput")

    with tile.TileContext(nc) as tc:
        with (
            tc.tile_pool(name="wp", bufs=1) as wp,
            tc.tile_pool(name="per", bufs=1) as per,
            tc.tile_pool(name="psd", bufs=2, space="PSUM") as psd,
            tc.tile_pool(name="pse", bufs=2, space="PSUM") as pse,
            tc.tile_pool(name="dram", bufs=1, space="DRAM") as drp,
        ):
            # ---- persistent tiles ----
            rhsF = per.tile([65, N], f32, tag="rhsF", name="rhsF")
            lhsTm = per.tile([65, H], f32, tag="lhsTm", name="lhsTm")
            u_t = per.tile([64, N], f32r, tag="u", name="u_t")
            x2r = per.tile([64, H], f32r, tag="x2r", name="x2r")
            xcat_a = per.tile([128, H], f32r, tag="xcata", name="xcat_a")
            xcat_b = per.tile([64, H], f32r, tag="xcatb", name="xcat_b")
            widx = per.tile([64, KNN * 128], i16, tag="widx", name="widx")
            acc3 = per.tile([64, H], f32r, tag="acc3", name="acc3")
            xg_sb = per.tile([128, 8], f32, tag="xg", name="xg_sb")
            b7_sb = per.tile([128, 4], f32, tag="b7", name="b7_sb")
            choff_sb = per.tile([128, 64], u16, tag="choff", name="choff_sb")
            zero128 = per.tile([128, 64], f32, tag="z128", name="zero128")

            idx_dram = drp.tile([H, KNN], i16, tag="idxd", name="idx_dram")

            x1h = xcat_a[0:64]
            x2h = xcat_a[64:128]
            x3h = xcat_b

            # critical inputs first; rows 3:64 of rhsF are zeroed separately
            # so the pts DMA does not wait on the big memset
            nc.vector.memset(rhsF[0:32], 0.0)
            nc.sync.dma_start(rhsF[0:3], pts_full[:])
            nc.sync.dma_start(x2r[0:3], pts_mine[:].bitcast(f32r))
            nc.sync.dma_start(choff_sb, choff_d[:])
            nc.vector.memset(zero128, 0.0)
            nc.vector.memset(rhsF[32:64], 0.0)
            nc.vector.memset(lhsTm[0:32], 0.0)
            nc.vector.memset(lhsTm[32:64], 0.0)
            nc.vector.memset(lhsTm[64:65], 1.0)
            nc.scalar.mul(lhsTm[0:3], x2r[0:3].bitcast(f32), 2.0)
            # all weights arrive in one packed DMA (single HWDGE dispatch);
            # per-weight tiles are views into the packed tile
            wcols = sum(int(np.prod(s[1:])) if len(s) > 1 else 1
                        for _, s, _ in _WEIGHT_SPECS)
            wpack_d = nc.dram_tensor("wpack", [128, wcols], f32r,
                                     kind="ExternalInput")
            wpt = wp.tile([128, wcols], f32r, tag="wpack", name="wpack_sb")
            nc.sync.dma_start(wpt, wpack_d[:])
            wsb = {}
            col = 0
            for name, shape, isr in _WEIGHT_SPECS:
                w = int(np.prod(shape[1:])) if len(shape) > 1 else 1
                v = wpt[0 : shape[0], col : col + w]
                if len(shape) == 3:
                    v = v.rearrange("p (a b) -> p a b", a=shape[1])
                wsb[name] = v if isr else v.bitcast(f32)
                col += w

            LAYER = {
                1: dict(cin=3, uT="u1T", ones="ones3", vTh="v1Th", c="c1",
                        w2="w2T", cc="cc2"),
                2: dict(cin=64, uT="u2T", ones="ones64", vTh="v2Th", c="c2",
                        w2="w4T", cc="cc4"),
                3: dict(cin=64, uT="u3T", ones="ones64", vTh="v3Th", c="c3",
                        w2=None, cc=None),
            }
            XOUT = {1: x1h, 2: x2h, 3: x3h}

            with (
                tc.tile_pool(name="dsb", bufs=4) as dsbp,
                tc.tile_pool(name="pp", bufs=2) as ppl,
                tc.tile_pool(name="tk", bufs=2) as tkp,
                tc.tile_pool(name="gp", bufs=2) as gp,
                tc.tile_pool(name="yp", bufs=3) as ypl,
                tc.tile_pool(name="za", bufs=1) as zap,
            ):
                # per-tile cand/cidx live from the first quarter scan (during
                # the previous layer's tail) until the peel -- keep many bufs
                cands = {}

                def prep_block(lid, lo):
                    """u_t[:, lo:lo+1024] = uT.T @ rhsF[0:cin] for layer lid.
                    For layer 1 also computes rhsF[64] = -sum_c rhsF[c]^2 (for
                    layers 2-3 the sq row arrives inside the AllGather)."""
                    P = LAYER[lid]
                    cin = P["cin"]
                    if lid == 1:
                        xsq = ppl.tile([64, 1024], f32, tag="xsq",
                                       name="xsq")[0:cin]
                        nc.scalar.activation(xsq, rhsF[0:cin, lo : lo + 1024],
                                             AF.Square)
                        sqrow = ppl.tile([1, 1024], f32, tag="sqr",
                                         name="sqrow")
                        for j in range(2):
                            jl = slice(j * 512, (j + 1) * 512)
                            pq = psd.tile([128, 1024], f32, tag="d", name="pq")
                            nc.tensor.matmul(pq[0:1, 0:512], wsb[P["ones"]],
                                             xsq[:, jl])
                            nc.scalar.mul(sqrow[:, jl], pq[0:1, 0:512], -1.0)
                        nc.sync.dma_start(rhsF[64:65, lo : lo + 1024], sqrow)
                    for j in range(2):
                        sl = slice(lo + j * 512, lo + (j + 1) * 512)
                        pu = psd.tile([128, 1024], f32, tag="d", name="pu")
                        nc.tensor.matmul(pu[0:64, 0:512], wsb[P["uT"]],
                                         rhsF[0:cin, sl])
                        nc.scalar.copy(u_t[:, sl], pu[0:64, 0:512])

                def dist_quarter(t, blk):
                    """distance psum + copy + top8 scan for tile t, column
                    block blk (1024 cols)."""
                    if t not in cands:
                        cands[t] = (
                            tkp.tile([128, 64], f32, tag="cand", name="cand",
                                     bufs=12),
                            tkp.tile([128, 64], u16, tag="cidx", name="cidx",
                                     bufs=12),
                        )
                    cand, cidx = cands[t]
                    lo = blk * 1024
                    pd = psd.tile([128, 1024], f32, tag="d", name="pd")
                    for q in range(2):
                        nc.tensor.matmul(
                            pd[:, q * 512 : (q + 1) * 512],
                            lhsTm[:, t * 128 : (t + 1) * 128],
                            rhsF[:, lo + q * 512 : lo + (q + 1) * 512],
                        )
                    dsb = dsbp.tile([128, 1024], f32, tag="dsb", name="dsb")
                    nc.scalar.copy(dsb, pd)
                    for q in range(2):
                        cc = blk * 2 + q
                        nc.vector.max(
                            out=cand[:, cc * 8 : (cc + 1) * 8],
                            in_=dsb[:, q * 512 : (q + 1) * 512],
                        )
                        nc.vector.max_index(
                            cidx[:, cc * 8 : (cc + 1) * 8],
                            cand[:, cc * 8 : (cc + 1) * 8],
                            dsb[:, q * 512 : (q + 1) * 512],
                        )

                def peel_compact(t):
                    """top-20 selection for tile t from its 64 candidates."""
                    cand, cidx = cands.pop(t)
                    nc.vector.tensor_add(cidx, cidx, choff_sb)
                    candw = tkp.tile([128, 64], f32, tag="candw", name="candw")
                    nc.vector.tensor_copy(candw, cand)
                    t8 = tkp.tile([128, 24], f32, tag="t8", name="t8")
                    nc.vector.max(out=t8[:, 0:8], in_=candw)
                    nc.vector.match_replace(out=candw, in_to_replace=t8[:, 0:8],
                                            in_values=candw, imm_value=NEG)
                    nc.vector.max(out=t8[:, 8:16], in_=candw)
                    nc.vector.match_replace(out=candw,
                                            in_to_replace=t8[:, 8:16],
                                            in_values=candw, imm_value=NEG)
                    nc.vector.max(out=t8[:, 16:24], in_=candw)
                    mask = tkp.tile([128, 64], f32, tag="mask", name="mask")
                    nc.vector.tensor_scalar(mask, cand, t8[:, 19:20], None,
                                            op0=ALU.is_ge)
                    cums = tkp.tile([128, 64], f32, tag="cums", name="cums")
                    nc.vector.tensor_tensor_scan(cums, mask, zero128, 0.0,
                                                 op0=ALU.add, op1=ALU.add)
                    # slot = cums*mask - 1  (-1 marks non-selected: ignored
                    # by local_scatter)
                    nc.vector.tensor_mul(cums, cums, mask)
                    nc.vector.tensor_scalar_add(cums, cums, -1.0)
                    slot = tkp.tile([128, 64], i16, tag="slot", name="slot")
                    nc.vector.tensor_copy(slot, cums)
                    sel = tkp.tile([128, 24], u16, tag="sel", name="sel")
                    nc.gpsimd.local_scatter(out_ap=sel, data_ap=cidx,
                                            idxs_ap=slot, channels=128,
                                            num_elems=24, num_idxs=64)
                    nc.sync.dma_start(
                        idx_dram[t * 128 : (t + 1) * 128, :],
                        sel[:, 0:KNN].bitcast(i16),
                    )

                # gather calls: (k0, nk, widx column base within the group).
                # The first call covers a single neighbor slot so the edge
                # pipeline starts as soon as possible after the peels.
                CALLS = [(0, 1, 0), (1, 4, 64), (5, 4, 320), (9, 4, 576),
                         (13, 4, 832), (17, 3, 1088)]

                def reformat(grp):
                    """widx[p, grp*G + base + r*nk + kk]
                         = idx_dram[1024*grp + 16*r + p, k0 + kk],
                    replicated over the four 16-partition groups; one DMA per
                    (gather call, replica) so the first call's indices land
                    first and later transfers overlap the gathers."""
                    for k0, nk, base in CALLS:
                        src = idx_dram[grp * 1024 : (grp + 1) * 1024,
                                       k0 : k0 + nk].rearrange(
                            "(r p) k -> p r k", p=16
                        )
                        for rep in range(4):
                            dst = widx[rep * 16 : (rep + 1) * 16,
                                       grp * G + base
                                       : grp * G + base + 64 * nk].rearrange(
                                "p (r k) -> p r k", k=nk
                            )
                            nc.sync.dma_start(dst, src)

                def gather_call(grp, call):
                    k0, nk, base = CALLS[call]
                    g = gp.tile([64, 4096], f32r, tag="g", name="g")
                    nc.gpsimd.ap_gather(
                        out_ap=g[:, 0 : 1024 * nk],
                        in_ap=u_t,
                        idxs_ap=widx[:, grp * G + base
                                     : grp * G + base + 64 * nk],
                        channels=64,
                        num_elems=N,
                        d=1,
                        num_idxs=1024 * nk,
                    )
                    # columns are (r, kk)-interleaved: col = nk*16*r + 16*kk + p
                    return g[:, 0 : 1024 * nk].rearrange(
                        "c (r f p) -> c f r p", f=nk, p=16
                    )

                def edge_k(lid, grp, gv, kk, k, zacc):
                    """one neighbor slot: y = Prelu(I u_j + V x_i + c);
                    z psum = W2 y; zacc = max(zacc, z).  For layer 3 only the
                    running max of u_j is needed: group 0 accumulates on the
                    Pool engine (the DVE is busy with dist scans), group 1
                    splits into two parallel chains (DVE evens, Pool odds)."""
                    P = LAYER[lid]
                    cin = P["cin"]
                    gk = gv[:, kk]  # (64, 64, 16): r-major, p-minor
                    if lid == 3:
                        gsl = slice(grp * 1024, (grp + 1) * 1024)
                        a3 = acc3[:, gsl].rearrange("c (r p) -> c r p", p=16)
                        if k == 0:
                            nc.vector.tensor_copy(a3, gk)
                        else:
                            nc.vector.tensor_max(a3, a3, gk)
                        return
                    for q in range(2):
                        rsl = slice(q * 32, (q + 1) * 32)
                        csl = slice(q * 512, (q + 1) * 512)
                        msl = slice(grp * 1024 + q * 512,
                                    grp * 1024 + (q + 1) * 512)
                        yp = pse.tile([64, 512], f32, tag="e", name="yp")
                        nc.tensor.matmul(yp, wsb["id64"], gk[:, rsl],
                                         start=True, stop=False)
                        nc.tensor.matmul(yp, wsb[P["vTh"]], x2r[0:cin, msl],
                                         start=False, stop=True)
                        y = ypl.tile([64, 512], f32r, tag="y", name="y")
                        nc.scalar.activation(y, yp, AF.Prelu, bias=wsb[P["c"]],
                                             alpha=0.2)
                        zp = pse.tile([64, 512], f32, tag="c2", name="zp")
                        nc.tensor.matmul(zp, wsb[P["w2"]], y)
                        if k == 0:
                            nc.vector.tensor_copy(zacc[:, csl], zp)
                        else:
                            nc.vector.tensor_max(zacc[:, csl], zacc[:, csl],
                                                 zp)

                def edge_fin(lid, grp, zacc):
                    P = LAYER[lid]
                    gsl = slice(grp * 1024, (grp + 1) * 1024)
                    x_out = XOUT[lid]
                    if lid == 3:
                        for q in range(2):
                            sl = slice(grp * 1024 + q * 512,
                                       grp * 1024 + (q + 1) * 512)
                            pe_ = pse.tile([64, 512], f32, tag="e", name="pe3")
                            nc.tensor.matmul(pe_, wsb["id64"], acc3[:, sl],
                                             start=True, stop=False)
                            nc.tensor.matmul(pe_, wsb[P["vTh"]],
                                             x2r[0 : P["cin"], sl],
                                             start=False, stop=True)
                            nc.scalar.activation(x_out[:, sl], pe_, AF.Prelu,
                                                 bias=wsb[P["c"]], alpha=0.2)
                    else:
                        nc.scalar.activation(x_out[:, gsl], zacc, AF.Prelu,
                                             bias=wsb[P["cc"]], alpha=0.2)

                ccs = {}

                def edge_steps(lid, grp):
                    """list of emission steps for the edge phase of (lid, grp).
                    For layers 1-2 the final step also launches the pairwise
                    AllGather of the fresh x_out group and refreshes the
                    lhsTm/x2r columns for the next layer."""
                    steps = []
                    state = {}
                    zacc = None
                    if lid != 3:
                        zacc = zap.tile([64, 1024], f32, tag="za", name="zacc")

                    def mk_gather(call):
                        def f():
                            state[call] = gather_call(grp, call)
                        return f

                    def mk_k(call, kk, k):
                        def f():
                            edge_k(lid, grp, state[call], kk, k, zacc)
                        return f

                    def fin():
                        edge_fin(lid, grp, zacc)
                        if lid < 3:
                            ccs[grp] = exchange_start(lid, grp)
                            lhs_prep(lid, grp)

                    for call, (k0, nk, base) in enumerate(CALLS):
                        steps.append(mk_gather(call))
                        for kk in range(nk):
                            steps.append(mk_k(call, kk, k0 + kk))
                    steps.append(fin)
                    return steps

                def exchange_start(lid, grp):
                    """export x_out group (plus its negated sq row, so the
                    peer needs no post-AllGather distance prep) and launch the
                    pairwise AllGather."""
                    gsl = slice(grp * 1024, (grp + 1) * 1024)
                    x_half = XOUT[lid]
                    xsq = ppl.tile([64, 1024], f32, tag="xsq", name="xsqe")
                    nc.scalar.activation(xsq, x_half[:, gsl].bitcast(f32),
                                         AF.Square)
                    sqrow = ppl.tile([1, 1024], f32, tag="sqr", name="sqrowe")
                    for j in range(2):
                        jl = slice(j * 512, (j + 1) * 512)
                        pq = psd.tile([128, 1024], f32, tag="d", name="pqe")
                        nc.tensor.matmul(pq[0:1, 0:512], wsb["ones64"],
                                         xsq[:, jl])
                        nc.scalar.mul(sqrow[:, jl], pq[0:1, 0:512], -1.0)
                    ccin = drp.tile([65, 1024], f32, tag="cci%d" % grp,
                                    name="ccin%d" % grp)
                    nc.scalar.dma_start(ccin[0:64], x_half[:, gsl].bitcast(f32))
                    nc.scalar.dma_start(ccin[64:65], sqrow)
                    ccout = drp.tile([130, 1024], f32, tag="cco%d" % grp,
                                     name="ccout%d" % grp)
                    nc.gpsimd.collective_compute(
                        "AllGather", ALU.bypass, replica_groups=PAIRS,
                        ins=[ccin], outs=[ccout],
                    )
                    return ccout

                def exchange_finish(ccout, grp):
                    lo0 = grp * 1024
                    lo1 = 2048 + grp * 1024
                    nc.sync.dma_start(rhsF[0:64, lo0 : lo0 + 1024], ccout[0:64])
                    nc.sync.dma_start(rhsF[64:65, lo0 : lo0 + 1024],
                                      ccout[64:65])
                    nc.sync.dma_start(rhsF[0:64, lo1 : lo1 + 1024],
                                      ccout[65:129])
                    nc.sync.dma_start(rhsF[64:65, lo1 : lo1 + 1024],
                                      ccout[129:130])

                def lhs_prep(lid, grp):
                    """lhsTm/x2r columns for group grp from layer lid's out."""
                    gsl = slice(grp * 1024, (grp + 1) * 1024)
                    x_out = XOUT[lid]
                    nc.scalar.mul(lhsTm[0:64, gsl], x_out[:, gsl], 2.0)
                    nc.scalar.mul(x2r[0:64, gsl], x_out[:, gsl], 2.0)

                # ================= layer 1 lead-in =================
                # interleave per-block prep with that block's dist quarters so
                # the DVE starts scanning as soon as the first block is ready
                for blk in range(4):
                    prep_block(1, blk * 1024)
                    for t in range(8):
                        dist_quarter(t, blk)
                for t in range(8):
                    peel_compact(t)
                reformat(0)

                carry = []  # dist-quarter steps for the next layer's tiles 0..8

                def phase_B(lid):
                    """grp0 edge (launched early so its AllGather fires as
                    soon as possible) followed by grp1 tiles' dist/topk.  A
                    few dist quarters lead to cover the gather latency."""
                    a = []
                    for t in range(8, 16):
                        for blk in range(4):
                            a.append(lambda t=t, blk=blk: dist_quarter(t, blk))
                        a.append(lambda t=t: peel_compact(t))
                    for f in a[:4]:
                        f()
                    for f in edge_steps(lid, 0):
                        f()
                    for f in a[4:]:
                        f()
                    reformat(1)

                def phase_C(lid):
                    """grp1 edge followed by the next layer's lead work (or
                    conv6's first half for layer 3)."""
                    nxt = lid + 1 if lid < 3 else None
                    if nxt is not None:
                        exchange_finish(ccs[0], 0)
                    es = edge_steps(lid, 1)
                    if nxt is None:
                        _mix([lambda ob=ob: conv6_chunk(ob, 0)
                              for ob in range(8)], es)
                        return
                    for f in es:
                        f()
                    prep_block(nxt, 0)
                    prep_block(nxt, 2048)
                    for t in range(8):
                        dist_quarter(t, 0)
                    for t in range(3):
                        dist_quarter(t, 2)
                    for t in range(3, 8):
                        carry.append(lambda t=t: dist_quarter(t, 2))

                def phase_A(lid):
                    """finish the leftover and second-half dist quarters of
                    tiles 0..8 for layer lid, then peel."""
                    exchange_finish(ccs[1], 1)
                    for f in carry:
                        f()
                    carry.clear()
                    prep_block(lid, 1024)
                    prep_block(lid, 3072)
                    for t in range(8):
                        dist_quarter(t, 1)
                        dist_quarter(t, 3)
                    for t in range(8):
                        peel_compact(t)
                    reformat(0)

                # conv6 pieces
                def conv6_chunk(ob, jp):
                    obs = slice(ob * 128, (ob + 1) * 128)
                    pf = psd.tile([128, 1024], f32, tag="d", name="pf6")
                    for q in range(2):
                        sl = slice(jp * 1024 + q * 512,
                                   jp * 1024 + (q + 1) * 512)
                        po = slice(q * 512, (q + 1) * 512)
                        nc.tensor.matmul(pf[:, po], wsb["w6aT"][:, obs],
                                         xcat_a[:, sl], start=True, stop=False)
                        nc.tensor.matmul(pf[:, po], wsb["w6bT"][:, obs],
                                         xcat_b[:, sl], start=False, stop=True)
                    h6 = gp.tile([128, 1024], f32, tag="h6", name="h6", bufs=2)
                    nc.scalar.activation(h6, pf, AF.Prelu,
                                         bias=wsb["c6v"][:, ob : ob + 1],
                                         alpha=0.2)
                    xgt = xgts[ob]
                    nc.vector.reduce_max(xgt[:, jp : jp + 1], h6,
                                         axis=mybir.AxisListType.X)

                xgts = [tkp.tile([128, 2], f32, tag="xgt%d" % ob,
                                 name="xgt%d" % ob) for ob in range(8)]

                # ================= the three layers =================
                phase_B(1)
                phase_C(1)
                phase_A(2)
                phase_B(2)
                phase_C(2)
                phase_A(3)
                phase_B(3)
                phase_C(3)

                # ============ conv6 second half + global max pool ============
                for ob in range(8):
                    conv6_chunk(ob, 1)
                    nc.vector.reduce_max(xg_sb[:, ob : ob + 1], xgts[ob],
                                         axis=mybir.AxisListType.X)
                ccg_i = drp.tile([128, 8], f32, tag="cgi", name="ccg_in")
                nc.scalar.dma_start(ccg_i, xg_sb)
                ccg_o = drp.tile([128, 8], f32, tag="cgo", name="ccg_out")
                nc.gpsimd.collective_compute(
                    "AllReduce", ALU.max, replica_groups=PAIRS,
                    ins=[ccg_i], outs=[ccg_o],
                )

            # layer scratch pools released here; final stage below.
            with tc.tile_pool(name="fin", bufs=1) as fin:
                # conv7 local part without bias (overlaps the AllReduce)
                h7 = fin.tile([128, 4 * H], f32r, tag="h7", name="h7")
                for ob in range(4):
                    obs = slice(ob * 128, (ob + 1) * 128)
                    for jp in range(2):
                        pf = psd.tile([128, 1024], f32, tag="d", name="pf7")
                        for q in range(2):
                            sl = slice(jp * 1024 + q * 512,
                                       jp * 1024 + (q + 1) * 512)
                            po = slice(q * 512, (q + 1) * 512)
                            nc.tensor.matmul(pf[:, po], wsb["w7laT"][:, obs],
                                             xcat_a[:, sl],
                                             start=True, stop=False)
                            nc.tensor.matmul(pf[:, po], wsb["w7lbT"][:, obs],
                                             xcat_b[:, sl],
                                             start=False, stop=True)
                        nc.scalar.copy(
                            h7[:, ob * H + jp * 1024 : ob * H + (jp + 1) * 1024],
                            pf,
                        )

                nc.sync.dma_start(xg_sb, ccg_o)

                # conv7 effective bias: c7 + W7g @ xg
                for ob in range(4):
                    pb = psd.tile([128, 1024], f32, tag="d", name="pb7")
                    for kb in range(8):
                        nc.tensor.matmul(
                            pb[:, 0:1],
                            wsb["w7gT"][:, kb, ob * 128 : (ob + 1) * 128],
                            xg_sb[:, kb : kb + 1],
                            start=(kb == 0),
                            stop=(kb == 7),
                        )
                    nc.scalar.activation(b7_sb[:, ob : ob + 1], pb[:, 0:1],
                                         AF.Identity,
                                         bias=wsb["c7v"][:, ob : ob + 1])

                h7a = fin.tile([128, 4 * H], f32r, tag="h7a", name="h7a")
                for ob in range(4):
                    for jp in range(4):
                        sl = slice(ob * H + jp * 512, ob * H + (jp + 1) * 512)
                        nc.scalar.activation(
                            h7a[:, sl], h7[:, sl], AF.Prelu,
                            bias=b7_sb[:, ob : ob + 1], alpha=0.2,
                        )
                h8 = fin.tile([128, 2 * H], f32r, tag="h8", name="h8")
                for ob in range(2):
                    for jp in range(2):
                        pf = psd.tile([128, 1024], f32, tag="d", name="pf8")
                        for q in range(2):
                            po = slice(q * 512, (q + 1) * 512)
                            co = jp * 1024 + q * 512
                            for kb in range(4):
                                nc.tensor.matmul(
                                    pf[:, po],
                                    wsb["w8T"][:, kb, ob * 128 : (ob + 1) * 128],
                                    h7a[:, kb * H + co : kb * H + co + 512],
                                    start=(kb == 0),
                                    stop=(kb == 3),
                                )
                        nc.scalar.activation(
                            h8[:, ob * H + jp * 1024 : ob * H + (jp + 1) * 1024],
                            pf, AF.Prelu, bias=wsb["c8v"][:, ob : ob + 1],
                            alpha=0.2,
                        )
                o_sb = fin.tile([13, H], f32, tag="osb", name="o_sb")
                for jp in range(2):
                    pf = psd.tile([128, 1024], f32, tag="d", name="pf9")
                    for q in range(2):
                        po = slice(q * 512, (q + 1) * 512)
                        co = jp * 1024 + q * 512
                        for kb in range(2):
                            nc.tensor.matmul(
                                pf[0:13, po],
                                wsb["w9T"][:, kb, :],
                                h8[:, kb * H + co : kb * H + co + 512],
                                start=(kb == 0),
                                stop=(kb == 1),
                            )
                    nc.scalar.activation(
                        o_sb[:, jp * 1024 : (jp + 1) * 1024], pf[0:13],
                        AF.Identity, bias=wsb["b9v"],
                    )
                nc.sync.dma_start(out_d[:], o_sb)

    nc.compile()
    return nc


def make_in_maps(inputs):
    """Per-core input dicts from the full problem inputs."""
    wd = _prep_weights(inputs)
    wcols = sum(int(np.prod(s[1:])) if len(s) > 1 else 1
                for _, s, _ in _WEIGHT_SPECS)
    wpack = np.zeros((128, wcols), dtype=np.float32)
    col = 0
    for name, shape, isr in _WEIGHT_SPECS:
        w = int(np.prod(shape[1:])) if len(shape) > 1 else 1
        wpack[0 : shape[0], col : col + w] = wd[name].reshape(shape[0], w)
        col += w
    pts = np.asarray(inputs["points"], dtype=np.float32)
    in_maps = []
    for c in range(8):
        b, h = c // 2, c % 2
        m = {"wpack": wpack}
        m["choff"] = np.ascontiguousarray(
            np.tile(np.repeat(np.arange(8, dtype=np.uint16) * 512, 8), (128, 1)))
        m["pts_full"] = np.ascontiguousarray(pts[b])
        m["pts_mine"] = np.ascontiguousarray(pts[b][:, h * H : (h + 1) * H])
        in_maps.append(m)
    return in_maps


def kernel(**inputs):
    from concourse.bass_utils import run_bass_kernel_spmd

    if "nc" not in _CACHE:
        _CACHE["nc"] = build_kernel()
    nc = _CACHE["nc"]
    in_maps = make_in_maps(inputs)
    res = run_bass_kernel_spmd(nc, in_maps, core_ids=list(range(8)))
    out = np.zeros((B, 13, N), dtype=np.float32)
    for c in range(8):
        b, h = c // 2, c % 2
        out[b][:, h * H : (h + 1) * H] = res.results[c]["out"]
    return out


# revision 44
# speedup vs baseline: 1.7266x; 1.2425x over previous
"""DGCNN part-segmentation forward pass on 8 Trainium2 NeuronCores.

Sharding: data-parallel over the batch (B=4) x 2-way split of the N=4096
points within each batch element.  Core c handles batch element c//2,
point rows [(c%2)*2048, (c%2+1)*2048).  The two cores of a pair exchange
their half of each EdgeConv output with pairwise AllGathers (one per
1024-point group, launched as soon as that group's output is ready) and
a pairwise AllReduce-max for the global pooling feature.

Precision: the feature path (EdgeConv transforms, convs 6-9) runs in
bfloat16 with fp32 PSUM accumulation.  Layer 1's kNN runs on raw fp32
points (3-D points have many near-ties; bf16 inputs measurably flip
neighbors), via a small separate fp32 lhs/rhs pair.  Layers 2-3 rank
neighbors of the bf16 features exactly: the distance matmul contracts
66 bf16 rows = 64 feature channels + a hi/lo bf16 pair carrying the
fp32 -|x_j|^2 row at ~16-bit precision.  The sq rows travel inside the
AllGather so no distance prep remains on the post-exchange critical
path.

The emission order is software-pipelined: engine queues are in-order,
so each phase leads with a few distance tiles, then runs the edge-conv
phase solid (its AllGather fires early), then the bulk of the
distance/top-k scans; the next layer's first distance quarters overlap
the current layer's second edge phase.

Per EdgeConv layer:
  top-20: per 128-row tile: 8x top-8 per 512-column chunk (DVE Max) ->
          64 candidates; 3 peeling rounds (max8 + match_replace) ->
          top-24 values; threshold mask + prefix-sum compaction +
          gpsimd local_scatter -> global indices of the top-20
  gather: batched GPSIMD ap_gather of the bf16 u = U x transform; the
          first call covers one neighbor slot so the edge pipeline
          starts right after the peels, later calls cover 4 slots
  edge  : psum = I*u_gathered + V*x_i (bf16 matmuls), LeakyReLU (Prelu)
          on the scalar engine, conv2 bf16 matmul; the k-max runs on
          the raw conv2 psums (LeakyReLU is monotone) and one Prelu
          finalizes each group.
"""

import sys

sys.path.insert(0, "/opt/trn_rl_repo")

import numpy as np

B = 4
N = 4096
H = 2048  # points per core (half of a batch element)
KNN = 20
EPS = 1e-5
NEG = -3.0e38
G = KNN * 64  # widx columns per point group

_CACHE = {}


# --------------------------------------------------------------------------
# host-side weight preparation
# --------------------------------------------------------------------------

def _bf16(x):
    import ml_dtypes

    return np.ascontiguousarray(x, dtype=np.float32).astype(ml_dtypes.bfloat16)


def _fold_bn(w, b, g, be):
    s = (g / np.sqrt(np.float32(1.0) + np.float32(EPS))).astype(np.float32)
    return (w * s[:, None]).astype(np.float32), (s * b + be).astype(np.float32)


def _prep_weights(inp):
    f = np.float32
    W, C = {}, {}
    for i in range(1, 9):
        W[i], C[i] = _fold_bn(
            inp["w%d" % i], inp["b%d" % i], inp["g%d" % i], inp["be%d" % i]
        )
    d = {}
    # edge conv layers: split into U (neighbor part) and V (center part).
    # Layer 1's x2r tile holds raw points, layers 2-3 hold 2*x, so the V
    # transpose is halved only for layers 2-3.
    for lid, wi, vscale in ((1, 1, 1.0), (2, 3, 0.5), (3, 5, 0.5)):
        w = W[wi]
        cin = w.shape[1] // 2
        U = w[:, :cin]
        V = w[:, cin:] - w[:, :cin]
        d["u%dT" % lid] = np.ascontiguousarray(U.T)
        d["v%dTh" % lid] = np.ascontiguousarray((V * f(vscale)).T)
        d["c%d" % lid] = C[wi].reshape(64, 1)
    d["w2T"] = np.ascontiguousarray(W[2].T)
    d["cc2"] = C[2].reshape(64, 1)
    d["w4T"] = np.ascontiguousarray(W[4].T)
    d["cc4"] = C[4].reshape(64, 1)
    # conv6 (192 -> 1024)
    w6T = np.ascontiguousarray(W[6].T)  # (192, 1024)
    d["w6aT"] = np.ascontiguousarray(w6T[:128])
    d["w6bT"] = np.ascontiguousarray(w6T[128:])
    d["c6v"] = np.ascontiguousarray(C[6].reshape(8, 128).T)  # (128, 8)
    # conv7 (1216 -> 512): xg part (1024) and local part (192)
    w7 = W[7]
    w7gT = np.ascontiguousarray(w7[:, :1024].T)  # (1024, 512)
    d["w7gT"] = np.ascontiguousarray(w7gT.reshape(8, 128, 512).transpose(1, 0, 2))
    w7lT = np.ascontiguousarray(w7[:, 1024:].T)  # (192, 512)
    d["w7laT"] = np.ascontiguousarray(w7lT[:128])
    d["w7lbT"] = np.ascontiguousarray(w7lT[128:])
    d["c7v"] = np.ascontiguousarray(C[7].reshape(4, 128).T)  # (128, 4)
    # conv8 (512 -> 256)
    w8T = np.ascontiguousarray(W[8].T)  # (512, 256)
    d["w8T"] = np.ascontiguousarray(w8T.reshape(4, 128, 256).transpose(1, 0, 2))
    d["c8v"] = np.ascontiguousarray(C[8].reshape(2, 128).T)  # (128, 2)
    # conv9 (256 -> 13), plain linear
    w9T = np.ascontiguousarray(inp["w9"].T.astype(f))  # (256, 13)
    d["w9T"] = np.ascontiguousarray(w9T.reshape(2, 128, 13).transpose(1, 0, 2))
    d["b9v"] = inp["b9"].astype(f).reshape(13, 1)
    # constants
    d["id64"] = np.eye(64, dtype=f)
    d["ones3"] = np.ones((3, 1), dtype=f)
    d["ones64"] = np.ones((64, 1), dtype=f)
    return d


# name -> (shape, is_bf16)
_WEIGHT_SPECS = [
    ("u1T", (3, 64), 0), ("v1Th", (3, 64), 1), ("c1", (64, 1), 0),
    ("w2T", (64, 64), 1), ("cc2", (64, 1), 0),
    ("u2T", (64, 64), 1), ("v2Th", (64, 64), 1), ("c2", (64, 1), 0),
    ("w4T", (64, 64), 1), ("cc4", (64, 1), 0),
    ("u3T", (64, 64), 1), ("v3Th", (64, 64), 1), ("c3", (64, 1), 0),
    ("w6aT", (128, 1024), 1), ("w6bT", (64, 1024), 1), ("c6v", (128, 8), 0),
    ("w7gT", (128, 8, 512), 0), ("w7laT", (128, 512), 1),
    ("w7lbT", (64, 512), 1), ("c7v", (128, 4), 0),
    ("w8T", (128, 4, 256), 1), ("c8v", (128, 2), 0),
    ("w9T", (128, 2, 13), 1), ("b9v", (13, 1), 0),
    ("id64", (64, 64), 0), ("ones3", (3, 1), 0), ("ones64", (64, 1), 0),
]


def _wcols(bf):
    return sum(int(np.prod(s[1:])) if len(s) > 1 else 1
               for _, s, isb in _WEIGHT_SPECS if isb == bf)


# --------------------------------------------------------------------------
# device kernel builder
# --------------------------------------------------------------------------

def build_kernel():
    import concourse.bacc as bacc
    import concourse.mybir as mybir
    import concourse.tile as tile

    f32 = mybir.dt.float32
    bf = mybir.dt.bfloat16
    i16 = mybir.dt.int16
    u16 = mybir.dt.uint16
    AF = mybir.ActivationFunctionType
    ALU = mybir.AluOpType
    PAIRS = [[0, 1], [2, 3], [4, 5], [6, 7]]

    nc = bacc.Bacc("TRN2", target_bir_lowering=False, num_devices=8)

    pts_full = nc.dram_tensor("pts_full", [3, N], f32, kind="ExternalInput")
    pts_mine = nc.dram_tensor("pts_mine", [3, H], f32, kind="ExternalInput")
    pts_mine_bf = nc.dram_tensor("pts_mine_bf", [3, H], bf,
                                 kind="ExternalInput")
    choff_d = nc.dram_tensor("choff", [128, 64], u16, kind="ExternalInput")
    wpack32_d = nc.dram_tensor("wpack32", [128, _wcols(0)], f32,
                               kind="ExternalInput")
    wpack16_d = nc.dram_tensor("wpack16", [128, _wcols(1)], bf,
                               kind="ExternalInput")
    out_d = nc.dram_tensor("out", [13, H], f32, kind="ExternalOutput")

    with tile.TileContext(nc) as tc:
        with (
            tc.tile_pool(name="wp", bufs=1) as wp,
            tc.tile_pool(name="per", bufs=1) as per,
            tc.tile_pool(name="psd", bufs=2, space="PSUM") as psd,
            tc.tile_pool(name="pse", bufs=2, space="PSUM") as pse,
            tc.tile_pool(name="dram", bufs=1, space="DRAM") as drp,
        ):
            # ---- persistent tiles ----
            rhsF = per.tile([66, N], bf, tag="rhsF", name="rhsF")
            rhsF1 = per.tile([4, N], f32, tag="rhsF1", name="rhsF1")
            lhsTm = per.tile([66, H], bf, tag="lhsTm", name="lhsTm")
            lhsTm1 = per.tile([4, H], f32, tag="lhsTm1", name="lhsTm1")
            u_t = per.tile([64, N], f32, tag="u", name="u_t")
            x2r = per.tile([64, H], bf, tag="x2r", name="x2r")
            xcat_a = per.tile([128, H], bf, tag="xcata", name="xcat_a")
            xcat_b = per.tile([64, H], bf, tag="xcatb", name="xcat_b")
            widx = per.tile([64, KNN * 128], i16, tag="widx", name="widx")
            acc3 = per.tile([64, H], f32, tag="acc3", name="acc3")
            xg_sb = per.tile([128, 8], f32, tag="xg", name="xg_sb")
            b7_sb = per.tile([128, 4], f32, tag="b7", name="b7_sb")
            choff_sb = per.tile([128, 64], u16, tag="choff", name="choff_sb")
            zero128 = per.tile([128, 64], f32, tag="z128", name="zero128")

            idx_dram = drp.tile([H, KNN], i16, tag="idxd", name="idx_dram")

            x1h = xcat_a[0:64]
            x2h = xcat_a[64:128]
            x3h = xcat_b

            # critical inputs first; the row-3 ones of lhsTm1 come from a
            # 32-aligned memset that the pts DMA then partially overwrites
            nc.vector.memset(lhsTm1[0:4], 1.0)
            nc.sync.dma_start(lhsTm1[0:3], pts_mine[:])
            nc.sync.dma_start(x2r[0:3], pts_mine_bf[:])
            nc.sync.dma_start(choff_sb, choff_d[:])
            nc.vector.memset(zero128, 0.0)
            nc.vector.memset(lhsTm[64:66], 1.0)
            # all weights arrive in two packed DMAs; per-weight tiles are
            # views into the packed tiles
            wpt32 = wp.tile([128, _wcols(0)], f32, tag="wp32", name="wp32")
            nc.sync.dma_start(wpt32, wpack32_d[:])
            wpt16 = wp.tile([128, _wcols(1)], bf, tag="wp16", name="wp16")
            nc.sync.dma_start(wpt16, wpack16_d[:])
            wsb = {}
            col = {0: 0, 1: 0}
            for name, shape, isb in _WEIGHT_SPECS:
                w = int(np.prod(shape[1:])) if len(shape) > 1 else 1
                src = wpt16 if isb else wpt32
                v = src[0 : shape[0], col[isb] : col[isb] + w]
                if len(shape) == 3:
                    v = v.rearrange("p (a b) -> p a b", a=shape[1])
                wsb[name] = v
                col[isb] += w

            LAYER = {
                1: dict(cin=3, uT="u1T", vTh="v1Th", c="c1", w2="w2T",
                        cc="cc2"),
                2: dict(cin=64, uT="u2T", vTh="v2Th", c="c2", w2="w4T",
                        cc="cc4"),
                3: dict(cin=64, uT="u3T", vTh="v3Th", c="c3", w2=None,
                        cc=None),
            }
            XOUT = {1: x1h, 2: x2h, 3: x3h}

            with (
                tc.tile_pool(name="dsb", bufs=4) as dsbp,
                tc.tile_pool(name="pp", bufs=2) as ppl,
                tc.tile_pool(name="tk", bufs=2) as tkp,
                tc.tile_pool(name="gp", bufs=2) as gp,
                tc.tile_pool(name="yp", bufs=3) as ypl,
                tc.tile_pool(name="za", bufs=1) as zap,
            ):
                cands = {}

                def prep_block1(lo):
                    """layer-1 lead: rhsF1 = [2*pts; -|pts|^2] and
                    u_t = u1T.T @ pts for columns [lo, lo+1024)."""
                    stage = ppl.tile([3, 1024], f32, tag="pts", name="stage")
                    nc.sync.dma_start(stage, pts_full[:, lo : lo + 1024])
                    nc.scalar.mul(rhsF1[0:3, lo : lo + 1024], stage, 2.0)
                    xsq = ppl.tile([64, 1024], f32, tag="xsq", name="xsq1")
                    nc.scalar.activation(xsq[0:3], stage, AF.Square)
                    sqrow = ppl.tile([1, 1024], f32, tag="sqr", name="sqrow1")
                    for j in range(2):
                        jl = slice(j * 512, (j + 1) * 512)
                        pq = psd.tile([128, 1024], f32, tag="d", name="pq")
                        nc.tensor.matmul(pq[0:1, 0:512], wsb["ones3"],
                                         xsq[0:3, jl])
                        nc.scalar.mul(sqrow[:, jl], pq[0:1, 0:512], -1.0)
                        pu = psd.tile([128, 1024], f32, tag="d", name="pu")
                        nc.tensor.matmul(pu[0:64, 0:512], wsb["u1T"],
                                         stage[:, jl])
                        nc.scalar.copy(u_t[:, lo + j * 512 : lo + (j + 1) * 512],
                                       pu[0:64, 0:512])
                    nc.sync.dma_start(rhsF1[3:4, lo : lo + 1024], sqrow)

                def prep_block(lid, lo):
                    """u_t[:, lo:lo+1024] = uT.T @ rhsF[0:cin] (bf16)."""
                    P = LAYER[lid]
                    cin = P["cin"]
                    for j in range(2):
                        sl = slice(lo + j * 512, lo + (j + 1) * 512)
                        pu = psd.tile([128, 1024], f32, tag="d", name="pu")
                        nc.tensor.matmul(pu[0:64, 0:512], wsb[P["uT"]],
                                         rhsF[0:cin, sl])
                        nc.scalar.copy(u_t[:, sl], pu[0:64, 0:512])

                def dist_quarter(lid, t, blk):
                    """distance psum + copy + top8 scan for tile t, column
                    block blk (1024 cols), ranking for layer lid."""
                    if t not in cands:
                        cands[t] = (
                            tkp.tile([128, 64], f32, tag="cand", name="cand",
                                     bufs=12),
                            tkp.tile([128, 64], u16, tag="cidx", name="cidx",
                                     bufs=12),
                        )
                    cand, cidx = cands[t]
                    lo = blk * 1024
                    pd = psd.tile([128, 1024], f32, tag="d", name="pd")
                    for q in range(2):
                        cs = slice(lo + q * 512, lo + (q + 1) * 512)
                        if lid == 1:
                            nc.tensor.matmul(
                                pd[:, q * 512 : (q + 1) * 512],
                                lhsTm1[:, t * 128 : (t + 1) * 128],
                                rhsF1[:, cs],
                            )
                        else:
                            nc.tensor.matmul(
                                pd[:, q * 512 : (q + 1) * 512],
                                lhsTm[:, t * 128 : (t + 1) * 128],
                                rhsF[:, cs],
                            )
                    dsb = dsbp.tile([128, 1024], f32, tag="dsb", name="dsb")
                    nc.scalar.copy(dsb, pd)
                    for q in range(2):
                        cc = blk * 2 + q
                        nc.vector.max(
                            out=cand[:, cc * 8 : (cc + 1) * 8],
                            in_=dsb[:, q * 512 : (q + 1) * 512],
                        )
                        nc.vector.max_index(
                            cidx[:, cc * 8 : (cc + 1) * 8],
                            cand[:, cc * 8 : (cc + 1) * 8],
                            dsb[:, q * 512 : (q + 1) * 512],
                        )

                def peel_compact(t):
                    """top-20 selection for tile t from its 64 candidates."""
                    cand, cidx = cands.pop(t)
                    nc.vector.tensor_add(cidx, cidx, choff_sb)
                    candw = tkp.tile([128, 64], f32, tag="candw", name="candw")
                    nc.vector.tensor_copy(candw, cand)
                    t8 = tkp.tile([128, 24], f32, tag="t8", name="t8")
                    nc.vector.max(out=t8[:, 0:8], in_=candw)
                    nc.vector.match_replace(out=candw, in_to_replace=t8[:, 0:8],
                                            in_values=candw, imm_value=NEG)
                    nc.vector.max(out=t8[:, 8:16], in_=candw)
                    nc.vector.match_replace(out=candw,
                                            in_to_replace=t8[:, 8:16],
                                            in_values=candw, imm_value=NEG)
                    nc.vector.max(out=t8[:, 16:24], in_=candw)
                    mask = tkp.tile([128, 64], f32, tag="mask", name="mask")
                    nc.vector.tensor_scalar(mask, cand, t8[:, 19:20], None,
                                            op0=ALU.is_ge)
                    cums = tkp.tile([128, 64], f32, tag="cums", name="cums")
                    nc.vector.tensor_tensor_scan(cums, mask, zero128, 0.0,
                                                 op0=ALU.add, op1=ALU.add)
                    # slot = cums*mask - 1  (-1 marks non-selected: ignored
                    # by local_scatter)
                    nc.vector.tensor_mul(cums, cums, mask)
                    nc.vector.tensor_scalar_add(cums, cums, -1.0)
                    slot = tkp.tile([128, 64], i16, tag="slot", name="slot")
                    nc.vector.tensor_copy(slot, cums)
                    sel = tkp.tile([128, 24], u16, tag="sel", name="sel")
                    nc.gpsimd.local_scatter(out_ap=sel, data_ap=cidx,
                                            idxs_ap=slot, channels=128,
                                            num_elems=24, num_idxs=64)
                    nc.sync.dma_start(
                        idx_dram[t * 128 : (t + 1) * 128, :],
                        sel[:, 0:KNN].bitcast(i16),
                    )

                # gather calls: (k0, nk, widx column base within the group).
                # The first call covers a single neighbor slot so the edge
                # pipeline starts as soon as possible after the peels.
                CALLS = [(0, 1, 0), (1, 4, 64), (5, 4, 320), (9, 4, 576),
                         (13, 4, 832), (17, 3, 1088)]

                def reformat(grp):
                    """widx[p, grp*G + base + r*nk + kk]
                         = idx_dram[1024*grp + 16*r + p, k0 + kk],
                    replicated over the four 16-partition groups; one DMA per
                    (gather call, replica) so the first call's indices land
                    first and later transfers overlap the gathers."""
                    for k0, nk, base in CALLS:
                        src = idx_dram[grp * 1024 : (grp + 1) * 1024,
                                       k0 : k0 + nk].rearrange(
                            "(r p) k -> p r k", p=16
                        )
                        for rep in range(4):
                            dst = widx[rep * 16 : (rep + 1) * 16,
                                       grp * G + base
                                       : grp * G + base + 64 * nk].rearrange(
                                "p (r k) -> p r k", k=nk
                            )
                            nc.sync.dma_start(dst, src)

                def gather_call(grp, call):
                    k0, nk, base = CALLS[call]
                    g = gp.tile([64, 4096], f32, tag="g", name="g")
                    nc.gpsimd.ap_gather(
                        out_ap=g[:, 0 : 1024 * nk],
                        in_ap=u_t,
                        idxs_ap=widx[:, grp * G + base
                                     : grp * G + base + 64 * nk],
                        channels=64,
                        num_elems=N,
                        d=1,
                        num_idxs=1024 * nk,
                    )
                    # columns are (r, kk)-interleaved: col = nk*16*r + 16*kk + p
                    return g[:, 0 : 1024 * nk].rearrange(
                        "c (r f p) -> c f r p", f=nk, p=16
                    )

                def edge_k(lid, grp, gv, kk, k, zacc):
                    """one neighbor slot: y = Prelu(I u_j + V x_i + c);
                    z psum = W2 y; zacc = max(zacc, z).  Layer 3 needs only
                    the running max of the gathered u."""
                    P = LAYER[lid]
                    cin = P["cin"]
                    gk = gv[:, kk]  # (64, 64, 16): r-major, p-minor
                    if lid == 3:
                        gsl = slice(grp * 1024, (grp + 1) * 1024)
                        a3 = acc3[:, gsl].rearrange("c (r p) -> c r p", p=16)
                        if k == 0:
                            nc.vector.tensor_copy(a3, gk)
                        else:
                            nc.vector.tensor_max(a3, a3, gk)
                        return
                    for q in range(2):
                        rsl = slice(q * 32, (q + 1) * 32)
                        csl = slice(q * 512, (q + 1) * 512)
                        msl = slice(grp * 1024 + q * 512,
                                    grp * 1024 + (q + 1) * 512)
                        yp = pse.tile([64, 512], f32, tag="e", name="yp")
                        nc.tensor.matmul(yp, wsb["id64"], gk[:, rsl],
                                         start=True, stop=False)
                        nc.tensor.matmul(yp, wsb[P["vTh"]], x2r[0:cin, msl],
                                         start=False, stop=True)
                        y = ypl.tile([64, 512], bf, tag="y", name="y")
                        nc.scalar.activation(y, yp, AF.Prelu, bias=wsb[P["c"]],
                                             alpha=0.2)
                        zp = pse.tile([64, 512], f32, tag="c2", name="zp")
                        nc.tensor.matmul(zp, wsb[P["w2"]], y)
                        if k == 0:
                            nc.vector.tensor_copy(zacc[:, csl], zp)
                        else:
                            nc.vector.tensor_max(zacc[:, csl], zacc[:, csl],
                                                 zp)

                def edge_fin(lid, grp, zacc):
                    P = LAYER[lid]
                    gsl = slice(grp * 1024, (grp + 1) * 1024)
                    x_out = XOUT[lid]
                    if lid == 3:
                        for q in range(2):
                            sl = slice(grp * 1024 + q * 512,
                                       grp * 1024 + (q + 1) * 512)
                            pe_ = pse.tile([64, 512], f32, tag="e", name="pe3")
                            nc.tensor.matmul(pe_, wsb["id64"], acc3[:, sl],
                                             start=True, stop=False)
                            nc.tensor.matmul(pe_, wsb[P["vTh"]],
                                             x2r[0 : P["cin"], sl],
                                             start=False, stop=True)
                            nc.scalar.activation(x_out[:, sl], pe_, AF.Prelu,
                                                 bias=wsb[P["c"]], alpha=0.2)
                    else:
                        nc.scalar.activation(x_out[:, gsl], zacc, AF.Prelu,
                                             bias=wsb[P["cc"]], alpha=0.2)

                ccs = {}

                def exchange_start(lid, grp):
                    """export x_out group plus the hi/lo bf16 pair carrying
                    its fp32 -|x|^2 row, and launch the pairwise AllGather."""
                    gsl = slice(grp * 1024, (grp + 1) * 1024)
                    x_half = XOUT[lid]
                    xsq = ppl.tile([64, 1024], f32, tag="xsq", name="xsqe")
                    nc.scalar.activation(xsq, x_half[:, gsl], AF.Square)
                    sqrow = ppl.tile([1, 1024], f32, tag="sqr", name="sqrowe")
                    for j in range(2):
                        jl = slice(j * 512, (j + 1) * 512)
                        pq = psd.tile([128, 1024], f32, tag="d", name="pqe")
                        nc.tensor.matmul(pq[0:1, 0:512], wsb["ones64"],
                                         xsq[:, jl])
                        nc.scalar.mul(sqrow[:, jl], pq[0:1, 0:512], -1.0)
                    hi = ppl.tile([1, 1024], bf, tag="hi", name="hi")
                    nc.scalar.copy(hi, sqrow)
                    lo = ppl.tile([1, 1024], bf, tag="lo", name="lo")
                    nc.vector.tensor_tensor(lo, sqrow, hi, op=ALU.subtract)
                    ccin = drp.tile([66, 1024], bf, tag="cci%d" % grp,
                                    name="ccin%d" % grp)
                    nc.scalar.dma_start(ccin[0:64], x_half[:, gsl])
                    nc.scalar.dma_start(ccin[64:65], hi)
                    nc.scalar.dma_start(ccin[65:66], lo)
                    ccout = drp.tile([132, 1024], bf, tag="cco%d" % grp,
                                     name="ccout%d" % grp)
                    nc.gpsimd.collective_compute(
                        "AllGather", ALU.bypass, replica_groups=PAIRS,
                        ins=[ccin], outs=[ccout],
                    )
                    return ccout

                def exchange_finish(ccout, grp):
                    lo0 = grp * 1024
                    lo1 = 2048 + grp * 1024
                    nc.sync.dma_start(rhsF[0:64, lo0 : lo0 + 1024], ccout[0:64])
                    nc.sync.dma_start(rhsF[64:66, lo0 : lo0 + 1024],
                                      ccout[64:66])
                    nc.sync.dma_start(rhsF[0:64, lo1 : lo1 + 1024],
                                      ccout[66:130])
                    nc.sync.dma_start(rhsF[64:66, lo1 : lo1 + 1024],
                                      ccout[130:132])

                def lhs_prep(lid, grp):
                    """lhsTm/x2r columns for group grp from layer lid's out."""
                    gsl = slice(grp * 1024, (grp + 1) * 1024)
                    x_out = XOUT[lid]
                    nc.scalar.mul(lhsTm[0:64, gsl], x_out[:, gsl], 2.0)
                    nc.scalar.mul(x2r[0:64, gsl], x_out[:, gsl], 2.0)

                def edge_steps(lid, grp):
                    """emission steps for the edge phase of (lid, grp); the
                    final step also launches the AllGather for layers 1-2."""
                    steps = []
                    state = {}
                    zacc = None
                    if lid != 3:
                        zacc = zap.tile([64, 1024], f32, tag="za", name="zacc")

                    def mk_gather(call):
                        def f():
                            state[call] = gather_call(grp, call)
                        return f

                    def mk_k(call, kk, k):
                        def f():
                            edge_k(lid, grp, state[call], kk, k, zacc)
                        return f

                    def fin():
                        edge_fin(lid, grp, zacc)
                        if lid < 3:
                            ccs[grp] = exchange_start(lid, grp)
                            lhs_prep(lid, grp)

                    for call, (k0, nk, base) in enumerate(CALLS):
                        steps.append(mk_gather(call))
                        for kk in range(nk):
                            steps.append(mk_k(call, kk, k0 + kk))
                    steps.append(fin)
                    return steps

                carry = []

                def phase_B(lid):
                    """grp0 edge (leading, so its AllGather fires early)
                    followed by grp1 tiles' dist/topk."""
                    a = []
                    for t in range(8, 16):
                        for blk in range(4):
                            a.append(lambda t=t, blk=blk:
                                     dist_quarter(lid, t, blk))
                        a.append(lambda t=t: peel_compact(t))
                    for f in a[:4]:
                        f()
                    for f in edge_steps(lid, 0):
                        f()
                    for f in a[4:]:
                        f()
                    reformat(1)

                def phase_C(lid):
                    """grp1 edge followed by the next layer's lead work (or
                    conv6's first half for layer 3)."""
                    nxt = lid + 1 if lid < 3 else None
                    if nxt is not None:
                        exchange_finish(ccs[0], 0)
                    es = edge_steps(lid, 1)
                    if nxt is None:
                        _mix([lambda ob=ob: conv6_chunk(ob, 0)
                              for ob in range(8)], es)
                        return
                    for f in es:
                        f()
                    prep_block(nxt, 0)
                    prep_block(nxt, 2048)
                    for t in range(8):
                        dist_quarter(nxt, t, 0)
                    for t in range(3):
                        dist_quarter(nxt, t, 2)
                    for t in range(3, 8):
                        carry.append(lambda t=t: dist_quarter(nxt, t, 2))

                def phase_A(lid):
                    """finish the leftover and second-half dist quarters of
                    tiles 0..8 for layer lid, then peel."""
                    exchange_finish(ccs[1], 1)
                    for f in carry:
                        f()
                    carry.clear()
                    prep_block(lid, 1024)
                    prep_block(lid, 3072)
                    for t in range(8):
                        dist_quarter(lid, t, 1)
                        dist_quarter(lid, t, 3)
                    for t in range(8):
                        peel_compact(t)
                    reformat(0)

                def conv6_chunk(ob, jp):
                    obs = slice(ob * 128, (ob + 1) * 128)
                    pf = psd.tile([128, 1024], f32, tag="d", name="pf6")
                    for q in range(2):
                        sl = slice(jp * 1024 + q * 512,
                                   jp * 1024 + (q + 1) * 512)
                        po = slice(q * 512, (q + 1) * 512)
                        nc.tensor.matmul(pf[:, po], wsb["w6aT"][:, obs],
                                         xcat_a[:, sl], start=True, stop=False)
                        nc.tensor.matmul(pf[:, po], wsb["w6bT"][:, obs],
                                         xcat_b[:, sl], start=False, stop=True)
                    h6 = gp.tile([128, 1024], f32, tag="h6", name="h6", bufs=2)
                    nc.scalar.activation(h6, pf, AF.Prelu,
                                         bias=wsb["c6v"][:, ob : ob + 1],
                                         alpha=0.2)
                    xgt = xgts[ob]
                    nc.vector.reduce_max(xgt[:, jp : jp + 1], h6,
                                         axis=mybir.AxisListType.X)

                xgts = [tkp.tile([128, 2], f32, tag="xgt%d" % ob,
                                 name="xgt%d" % ob) for ob in range(8)]

                # ================= layer 1 lead-in =================
                for blk in range(4):
                    prep_block1(blk * 1024)
                    for t in range(8):
                        dist_quarter(1, t, blk)
                for t in range(8):
                    peel_compact(t)
                reformat(0)

                # ================= the three layers =================
                phase_B(1)
                phase_C(1)
                phase_A(2)
                phase_B(2)
                phase_C(2)
                phase_A(3)
                phase_B(3)
                phase_C(3)

                # ============ conv6 second half + global max pool ============
                for ob in range(8):
                    conv6_chunk(ob, 1)
                    nc.vector.reduce_max(xg_sb[:, ob : ob + 1], xgts[ob],
                                         axis=mybir.AxisListType.X)
                ccg_i = drp.tile([128, 8], f32, tag="cgi", name="ccg_in")
                nc.scalar.dma_start(ccg_i, xg_sb)
                ccg_o = drp.tile([128, 8], f32, tag="cgo", name="ccg_out")
                nc.gpsimd.collective_compute(
                    "AllReduce", ALU.max, replica_groups=PAIRS,
                    ins=[ccg_i], outs=[ccg_o],
                )

            # layer scratch pools released here; final stage below.
            with tc.tile_pool(name="fin", bufs=1) as fin:
                # conv7 local part without bias (overlaps the AllReduce)
                h7 = fin.tile([128, 4 * H], bf, tag="h7", name="h7")
                for ob in range(4):
                    obs = slice(ob * 128, (ob + 1) * 128)
                    for jp in range(2):
                        pf = psd.tile([128, 1024], f32, tag="d", name="pf7")
                        for q in range(2):
                            sl = slice(jp * 1024 + q * 512,
                                       jp * 1024 + (q + 1) * 512)
                            po = slice(q * 512, (q + 1) * 512)
                            nc.tensor.matmul(pf[:, po], wsb["w7laT"][:, obs],
                                             xcat_a[:, sl],
                                             start=True, stop=False)
                            nc.tensor.matmul(pf[:, po], wsb["w7lbT"][:, obs],
                                             xcat_b[:, sl],
                                             start=False, stop=True)
                        nc.scalar.copy(
                            h7[:, ob * H + jp * 1024 : ob * H + (jp + 1) * 1024],
                            pf,
                        )

                nc.sync.dma_start(xg_sb, ccg_o)

                # conv7 effective bias: c7 + W7g @ xg
                for ob in range(4):
                    pb = psd.tile([128, 1024], f32, tag="d", name="pb7")
                    for kb in range(8):
                        nc.tensor.matmul(
                            pb[:, 0:1],
                            wsb["w7gT"][:, kb, ob * 128 : (ob + 1) * 128],
                            xg_sb[:, kb : kb + 1],
                            start=(kb == 0),
                            stop=(kb == 7),
                        )
                    nc.scalar.activation(b7_sb[:, ob : ob + 1], pb[:, 0:1],
                                         AF.Identity,
                                         bias=wsb["c7v"][:, ob : ob + 1])

                h7a = fin.tile([128, 4 * H], bf, tag="h7a", name="h7a")
                for ob in range(4):
                    for jp in range(4):
                        sl = slice(ob * H + jp * 512, ob * H + (jp + 1) * 512)
                        nc.scalar.activation(
                            h7a[:, sl], h7[:, sl], AF.Prelu,
                            bias=b7_sb[:, ob : ob + 1], alpha=0.2,
                        )
                h8 = fin.tile([128, 2 * H], bf, tag="h8", name="h8")
                for ob in range(2):
                    for jp in range(2):
                        pf = psd.tile([128, 1024], f32, tag="d", name="pf8")
                        for q in range(2):
                            po = slice(q * 512, (q + 1) * 512)
                            co = jp * 1024 + q * 512
                            for kb in range(4):
                                nc.tensor.matmul(
                                    pf[:, po],
                                    wsb["w8T"][:, kb, ob * 128 : (ob + 1) * 128],
                                    h7a[:, kb * H + co : kb * H + co + 512],
                                    start=(kb == 0),
                                    stop=(kb == 3),
                                )
                        nc.scalar.activation(
                            h8[:, ob * H + jp * 1024 : ob * H + (jp + 1) * 1024],
                            pf, AF.Prelu, bias=wsb["c8v"][:, ob : ob + 1],
                            alpha=0.2,
                        )
                o_sb = fin.tile([13, H], f32, tag="osb", name="o_sb")
                for jp in range(2):
                    pf = psd.tile([128, 1024], f32, tag="d", name="pf9")
                    for q in range(2):
                        po = slice(q * 512, (q + 1) * 512)
                        co = jp * 1024 + q * 512
                        for kb in range(2):
                            nc.tensor.matmul(
                                pf[0:13, po],
                                wsb["w9T"][:, kb, :],
                                h8[:, kb * H + co : kb * H + co + 512],
                                start=(kb == 0),
                                stop=(kb == 1),
                            )
                    nc.scalar.activation(
                        o_sb[:, jp * 1024 : (jp + 1) * 1024], pf[0:13],
                        AF.Identity, bias=wsb["b9v"],
                    )
                nc.sync.dma_start(out_d[:], o_sb)

    nc.compile()
    return nc


def _mix(a_steps, b_steps, lead=0):
    """Emit `lead` a-steps, then alternate a/b 1:1 until b is exhausted, then
    the remaining a-steps."""
    ia = 0
    for _ in range(min(lead, len(a_steps))):
        a_steps[ia]()
        ia += 1
    for ib in range(len(b_steps)):
        if ia < len(a_steps):
            a_steps[ia]()
            ia += 1
        b_steps[ib]()
    while ia < len(a_steps):
        a_steps[ia]()
        ia += 1


def make_in_maps(inputs):
    """Per-core input dicts from the full problem inputs."""
    wd = _prep_weights(inputs)
    wpack32 = np.zeros((128, _wcols(0)), dtype=np.float32)
    import ml_dtypes

    wpack16 = np.zeros((128, _wcols(1)), dtype=ml_dtypes.bfloat16)
    col = {0: 0, 1: 0}
    for name, shape, isb in _WEIGHT_SPECS:
        w = int(np.prod(shape[1:])) if len(shape) > 1 else 1
        dst = wpack16 if isb else wpack32
        v = wd[name].reshape(shape[0], w)
        dst[0 : shape[0], col[isb] : col[isb] + w] = (
            _bf16(v) if isb else v.astype(np.float32)
        )
        col[isb] += w
    pts = np.asarray(inputs["points"], dtype=np.float32)
    in_maps = []
    for c in range(8):
        b, h = c // 2, c % 2
        m = {"wpack32": wpack32, "wpack16": wpack16}
        m["choff"] = np.ascontiguousarray(
            np.tile(np.repeat(np.arange(8, dtype=np.uint16) * 512, 8), (128, 1)))
        m["pts_full"] = np.ascontiguousarray(pts[b])
        pm = np.ascontiguousarray(pts[b][:, h * H : (h + 1) * H])
        m["pts_mine"] = pm
        m["pts_mine_bf"] = _bf16(pm)
        in_maps.append(m)
    return in_maps


def kernel(**inputs):
    from concourse.bass_utils import run_bass_kernel_spmd

    if "nc" not in _CACHE:
        _CACHE["nc"] = build_kernel()
    nc = _CACHE["nc"]
    in_maps = make_in_maps(inputs)
    res = run_bass_kernel_spmd(nc, in_maps, core_ids=list(range(8)))
    out = np.zeros((B, 13, N), dtype=np.float32)
    for c in range(8):
        b, h = c // 2, c % 2
        out[b][:, h * H : (h + 1) * H] = res.results[c]["out"]
    return out


# revision 48
# speedup vs baseline: 1.7275x; 1.0005x over previous
"""DGCNN part-segmentation forward pass on 8 Trainium2 NeuronCores.

Sharding: data-parallel over the batch (B=4) x 2-way split of the N=4096
points within each batch element.  Core c handles batch element c//2,
point rows [(c%2)*2048, (c%2+1)*2048).  The two cores of a pair exchange
their half of each EdgeConv output with pairwise AllGathers (one per
1024-point group, launched as soon as that group's output is ready) and
a pairwise AllReduce-max for the global pooling feature.

Precision: the feature path (EdgeConv transforms, convs 6-9) runs in
bfloat16 with fp32 PSUM accumulation.  Layer 1's kNN runs on raw fp32
points (3-D points have many near-ties; bf16 inputs measurably flip
neighbors), via a small separate fp32 lhs/rhs pair.  Layers 2-3 rank
neighbors of the bf16 features exactly: the distance matmul contracts
66 bf16 rows = 64 feature channels + a hi/lo bf16 pair carrying the
fp32 -|x_j|^2 row at ~16-bit precision.  The sq rows travel inside the
AllGather so no distance prep remains on the post-exchange critical
path.

The emission order is software-pipelined: engine queues are in-order,
so each phase leads with a few distance tiles, then runs the edge-conv
phase solid (its AllGather fires early), then the bulk of the
distance/top-k scans; the next layer's first distance quarters overlap
the current layer's second edge phase.

Per EdgeConv layer:
  top-20: per 128-row tile: 8x top-8 per 512-column chunk (DVE Max) ->
          64 candidates; 3 peeling rounds (max8 + match_replace) ->
          top-24 values; threshold mask + prefix-sum compaction +
          gpsimd local_scatter -> global indices of the top-20
  gather: batched GPSIMD ap_gather of the bf16 u = U x transform; the
          first call covers one neighbor slot so the edge pipeline
          starts right after the peels, later calls cover 4 slots
  edge  : psum = I*u_gathered + V*x_i (bf16 matmuls), LeakyReLU (Prelu)
          on the scalar engine, conv2 bf16 matmul; the k-max runs on
          the raw conv2 psums (LeakyReLU is monotone) and one Prelu
          finalizes each group.
"""

import sys

sys.path.insert(0, "/opt/trn_rl_repo")

import numpy as np

B = 4
N = 4096
H = 2048  # points per core (half of a batch element)
KNN = 20
EPS = 1e-5
NEG = -3.0e38
G = KNN * 64  # widx columns per point group

_CACHE = {}


# --------------------------------------------------------------------------
# host-side weight preparation
# --------------------------------------------------------------------------

def _bf16(x):
    import ml_dtypes

    return np.ascontiguousarray(x, dtype=np.float32).astype(ml_dtypes.bfloat16)


def _fold_bn(w, b, g, be):
    s = (g / np.sqrt(np.float32(1.0) + np.float32(EPS))).astype(np.float32)
    return (w * s[:, None]).astype(np.float32), (s * b + be).astype(np.float32)


def _prep_weights(inp):
    f = np.float32
    W, C = {}, {}
    for i in range(1, 9):
        W[i], C[i] = _fold_bn(
            inp["w%d" % i], inp["b%d" % i], inp["g%d" % i], inp["be%d" % i]
        )
    d = {}
    # edge conv layers: split into U (neighbor part) and V (center part).
    # Layer 1's x2r tile holds raw points, layers 2-3 hold 2*x, so the V
    # transpose is halved only for layers 2-3.
    for lid, wi, vscale in ((1, 1, 1.0), (2, 3, 0.5), (3, 5, 0.5)):
        w = W[wi]
        cin = w.shape[1] // 2
        U = w[:, :cin]
        V = w[:, cin:] - w[:, :cin]
        d["u%dT" % lid] = np.ascontiguousarray(U.T)
        d["v%dTh" % lid] = np.ascontiguousarray((V * f(vscale)).T)
        d["c%d" % lid] = C[wi].reshape(64, 1)
    d["w2T"] = np.ascontiguousarray(W[2].T)
    d["cc2"] = C[2].reshape(64, 1)
    d["w4T"] = np.ascontiguousarray(W[4].T)
    d["cc4"] = C[4].reshape(64, 1)
    # conv6 (192 -> 1024)
    w6T = np.ascontiguousarray(W[6].T)  # (192, 1024)
    d["w6aT"] = np.ascontiguousarray(w6T[:128])
    d["w6bT"] = np.ascontiguousarray(w6T[128:])
    d["c6v"] = np.ascontiguousarray(C[6].reshape(8, 128).T)  # (128, 8)
    # conv7 (1216 -> 512): xg part (1024) and local part (192)
    w7 = W[7]
    w7gT = np.ascontiguousarray(w7[:, :1024].T)  # (1024, 512)
    d["w7gT"] = np.ascontiguousarray(w7gT.reshape(8, 128, 512).transpose(1, 0, 2))
    w7lT = np.ascontiguousarray(w7[:, 1024:].T)  # (192, 512)
    d["w7laT"] = np.ascontiguousarray(w7lT[:128])
    d["w7lbT"] = np.ascontiguousarray(w7lT[128:])
    d["c7v"] = np.ascontiguousarray(C[7].reshape(4, 128).T)  # (128, 4)
    # conv8 (512 -> 256)
    w8T = np.ascontiguousarray(W[8].T)  # (512, 256)
    d["w8T"] = np.ascontiguousarray(w8T.reshape(4, 128, 256).transpose(1, 0, 2))
    d["c8v"] = np.ascontiguousarray(C[8].reshape(2, 128).T)  # (128, 2)
    # conv9 (256 -> 13), plain linear
    w9T = np.ascontiguousarray(inp["w9"].T.astype(f))  # (256, 13)
    d["w9T"] = np.ascontiguousarray(w9T.reshape(2, 128, 13).transpose(1, 0, 2))
    d["b9v"] = inp["b9"].astype(f).reshape(13, 1)
    # constants
    d["id64"] = np.eye(64, dtype=f)
    d["ones3"] = np.ones((3, 1), dtype=f)
    d["ones64"] = np.ones((64, 1), dtype=f)
    return d


# name -> (shape, is_bf16)
_WEIGHT_SPECS = [
    ("u1T", (3, 64), 0), ("v1Th", (3, 64), 1), ("c1", (64, 1), 0),
    ("w2T", (64, 64), 1), ("cc2", (64, 1), 0),
    ("u2T", (64, 64), 1), ("v2Th", (64, 64), 1), ("c2", (64, 1), 0),
    ("w4T", (64, 64), 1), ("cc4", (64, 1), 0),
    ("u3T", (64, 64), 1), ("v3Th", (64, 64), 1), ("c3", (64, 1), 0),
    ("w6aT", (128, 1024), 1), ("w6bT", (64, 1024), 1), ("c6v", (128, 8), 0),
    ("w7gT", (128, 8, 512), 0), ("w7laT", (128, 512), 1),
    ("w7lbT", (64, 512), 1), ("c7v", (128, 4), 0),
    ("w8T", (128, 4, 256), 1), ("c8v", (128, 2), 0),
    ("w9T", (128, 2, 13), 1), ("b9v", (13, 1), 0),
    ("id64", (64, 64), 0), ("ones3", (3, 1), 0), ("ones64", (64, 1), 0),
]


def _wcols(bf):
    return sum(int(np.prod(s[1:])) if len(s) > 1 else 1
               for _, s, isb in _WEIGHT_SPECS if isb == bf)


# --------------------------------------------------------------------------
# device kernel builder
# --------------------------------------------------------------------------

def build_kernel():
    import concourse.bacc as bacc
    import concourse.mybir as mybir
    import concourse.tile as tile

    f32 = mybir.dt.float32
    bf = mybir.dt.bfloat16
    i16 = mybir.dt.int16
    u16 = mybir.dt.uint16
    AF = mybir.ActivationFunctionType
    ALU = mybir.AluOpType
    PAIRS = [[0, 1], [2, 3], [4, 5], [6, 7]]

    nc = bacc.Bacc("TRN2", target_bir_lowering=False, num_devices=8)

    pts_full = nc.dram_tensor("pts_full", [3, N], f32, kind="ExternalInput")
    pts_mine = nc.dram_tensor("pts_mine", [3, H], f32, kind="ExternalInput")
    pts_mine_bf = nc.dram_tensor("pts_mine_bf", [3, H], bf,
                                 kind="ExternalInput")
    choff_d = nc.dram_tensor("choff", [128, 64], u16, kind="ExternalInput")
    wpack32_d = nc.dram_tensor("wpack32", [128, _wcols(0)], f32,
                               kind="ExternalInput")
    wpack16_d = nc.dram_tensor("wpack16", [128, _wcols(1)], bf,
                               kind="ExternalInput")
    out_d = nc.dram_tensor("out", [13, H], f32, kind="ExternalOutput")

    with tile.TileContext(nc) as tc:
        with (
            tc.tile_pool(name="wp", bufs=1) as wp,
            tc.tile_pool(name="per", bufs=1) as per,
            tc.tile_pool(name="psd", bufs=2, space="PSUM") as psd,
            tc.tile_pool(name="pse", bufs=2, space="PSUM") as pse,
            tc.tile_pool(name="dram", bufs=1, space="DRAM") as drp,
        ):
            # ---- persistent tiles ----
            rhsF = per.tile([66, N], bf, tag="rhsF", name="rhsF")
            rhsF1 = per.tile([4, N], f32, tag="rhsF1", name="rhsF1")
            lhsTm = per.tile([66, H], bf, tag="lhsTm", name="lhsTm")
            lhsTm1 = per.tile([4, H], f32, tag="lhsTm1", name="lhsTm1")
            u_t = per.tile([64, N], f32, tag="u", name="u_t")
            x2r = per.tile([64, H], bf, tag="x2r", name="x2r")
            xcat_a = per.tile([128, H], bf, tag="xcata", name="xcat_a")
            xcat_b = per.tile([64, H], bf, tag="xcatb", name="xcat_b")
            widx = per.tile([64, KNN * 128], i16, tag="widx", name="widx")
            acc3 = per.tile([64, H], f32, tag="acc3", name="acc3")
            xg_sb = per.tile([128, 8], f32, tag="xg", name="xg_sb")
            b7_sb = per.tile([128, 4], f32, tag="b7", name="b7_sb")
            choff_sb = per.tile([128, 64], u16, tag="choff", name="choff_sb")
            zero128 = per.tile([128, 64], f32, tag="z128", name="zero128")

            idx_dram = drp.tile([H, KNN], i16, tag="idxd", name="idx_dram")

            x1h = xcat_a[0:64]
            x2h = xcat_a[64:128]
            x3h = xcat_b

            # critical inputs first; the row-3 ones of lhsTm1 come from a
            # 32-aligned memset that the pts DMA then partially overwrites
            nc.vector.memset(lhsTm1[0:4], 1.0)
            nc.sync.dma_start(lhsTm1[0:3], pts_mine[:])
            nc.sync.dma_start(x2r[0:3], pts_mine_bf[:])
            nc.sync.dma_start(choff_sb, choff_d[:])
            nc.vector.memset(zero128, 0.0)
            nc.vector.memset(lhsTm[64:66], 1.0)
            # all weights arrive in two packed DMAs; per-weight tiles are
            # views into the packed tiles
            wpt32 = wp.tile([128, _wcols(0)], f32, tag="wp32", name="wp32")
            nc.scalar.dma_start(wpt32, wpack32_d[:])
            wpt16 = wp.tile([128, _wcols(1)], bf, tag="wp16", name="wp16")
            nc.scalar.dma_start(wpt16, wpack16_d[:])
            wsb = {}
            col = {0: 0, 1: 0}
            for name, shape, isb in _WEIGHT_SPECS:
                w = int(np.prod(shape[1:])) if len(shape) > 1 else 1
                src = wpt16 if isb else wpt32
                v = src[0 : shape[0], col[isb] : col[isb] + w]
                if len(shape) == 3:
                    v = v.rearrange("p (a b) -> p a b", a=shape[1])
                wsb[name] = v
                col[isb] += w

            LAYER = {
                1: dict(cin=3, uT="u1T", vTh="v1Th", c="c1", w2="w2T",
                        cc="cc2"),
                2: dict(cin=64, uT="u2T", vTh="v2Th", c="c2", w2="w4T",
                        cc="cc4"),
                3: dict(cin=64, uT="u3T", vTh="v3Th", c="c3", w2=None,
                        cc=None),
            }
            XOUT = {1: x1h, 2: x2h, 3: x3h}

            with (
                tc.tile_pool(name="dsb", bufs=4) as dsbp,
                tc.tile_pool(name="pp", bufs=2) as ppl,
                tc.tile_pool(name="tk", bufs=2) as tkp,
                tc.tile_pool(name="gp", bufs=2) as gp,
                tc.tile_pool(name="yp", bufs=3) as ypl,
                tc.tile_pool(name="za", bufs=1) as zap,
            ):
                cands = {}

                def prep_block1(lo):
                    """layer-1 lead: rhsF1 = [2*pts; -|pts|^2] and
                    u_t = u1T.T @ pts for columns [lo, lo+1024)."""
                    stage = ppl.tile([3, 1024], f32, tag="pts", name="stage")
                    nc.sync.dma_start(stage, pts_full[:, lo : lo + 1024])
                    nc.scalar.mul(rhsF1[0:3, lo : lo + 1024], stage, 2.0)
                    xsq = ppl.tile([64, 1024], f32, tag="xsq", name="xsq1")
                    nc.scalar.activation(xsq[0:3], stage, AF.Square)
                    sqrow = ppl.tile([1, 1024], f32, tag="sqr", name="sqrow1")
                    for j in range(2):
                        jl = slice(j * 512, (j + 1) * 512)
                        pq = psd.tile([128, 1024], f32, tag="d", name="pq")
                        nc.tensor.matmul(pq[0:1, 0:512], wsb["ones3"],
                                         xsq[0:3, jl])
                        nc.scalar.mul(sqrow[:, jl], pq[0:1, 0:512], -1.0)
                        pu = psd.tile([128, 1024], f32, tag="d", name="pu")
                        nc.tensor.matmul(pu[0:64, 0:512], wsb["u1T"],
                                         stage[:, jl])
                        nc.scalar.copy(u_t[:, lo + j * 512 : lo + (j + 1) * 512],
                                       pu[0:64, 0:512])
                    nc.sync.dma_start(rhsF1[3:4, lo : lo + 1024], sqrow)

                def prep_block(lid, lo):
                    """u_t[:, lo:lo+1024] = uT.T @ rhsF[0:cin] (bf16)."""
                    P = LAYER[lid]
                    cin = P["cin"]
                    for j in range(2):
                        sl = slice(lo + j * 512, lo + (j + 1) * 512)
                        pu = psd.tile([128, 1024], f32, tag="d", name="pu")
                        nc.tensor.matmul(pu[0:64, 0:512], wsb[P["uT"]],
                                         rhsF[0:cin, sl])
                        nc.scalar.copy(u_t[:, sl], pu[0:64, 0:512])

                def dist_quarter(lid, t, blk):
                    """distance psum + copy + top8 scan for tile t, column
                    block blk (1024 cols), ranking for layer lid."""
                    if t not in cands:
                        cands[t] = (
                            tkp.tile([128, 64], f32, tag="cand", name="cand",
                                     bufs=12),
                            tkp.tile([128, 64], u16, tag="cidx", name="cidx",
                                     bufs=12),
                        )
                    cand, cidx = cands[t]
                    lo = blk * 1024
                    pd = psd.tile([128, 1024], f32, tag="d", name="pd")
                    for q in range(2):
                        cs = slice(lo + q * 512, lo + (q + 1) * 512)
                        if lid == 1:
                            nc.tensor.matmul(
                                pd[:, q * 512 : (q + 1) * 512],
                                lhsTm1[:, t * 128 : (t + 1) * 128],
                                rhsF1[:, cs],
                            )
                        else:
                            nc.tensor.matmul(
                                pd[:, q * 512 : (q + 1) * 512],
                                lhsTm[:, t * 128 : (t + 1) * 128],
                                rhsF[:, cs],
                            )
                    dsb = dsbp.tile([128, 1024], f32, tag="dsb", name="dsb")
                    nc.scalar.copy(dsb, pd)
                    for q in range(2):
                        cc = blk * 2 + q
                        nc.vector.max(
                            out=cand[:, cc * 8 : (cc + 1) * 8],
                            in_=dsb[:, q * 512 : (q + 1) * 512],
                        )
                        nc.vector.max_index(
                            cidx[:, cc * 8 : (cc + 1) * 8],
                            cand[:, cc * 8 : (cc + 1) * 8],
                            dsb[:, q * 512 : (q + 1) * 512],
                        )

                def peel_compact(t):
                    """top-20 selection for tile t from its 64 candidates."""
                    cand, cidx = cands.pop(t)
                    nc.vector.tensor_add(cidx, cidx, choff_sb)
                    candw = tkp.tile([128, 64], f32, tag="candw", name="candw")
                    nc.vector.tensor_copy(candw, cand)
                    t8 = tkp.tile([128, 24], f32, tag="t8", name="t8")
                    nc.vector.max(out=t8[:, 0:8], in_=candw)
                    nc.vector.match_replace(out=candw, in_to_replace=t8[:, 0:8],
                                            in_values=candw, imm_value=NEG)
                    nc.vector.max(out=t8[:, 8:16], in_=candw)
                    nc.vector.match_replace(out=candw,
                                            in_to_replace=t8[:, 8:16],
                                            in_values=candw, imm_value=NEG)
                    nc.vector.max(out=t8[:, 16:24], in_=candw)
                    mask = tkp.tile([128, 64], f32, tag="mask", name="mask")
                    nc.vector.tensor_scalar(mask, cand, t8[:, 19:20], None,
                                            op0=ALU.is_ge)
                    cums = tkp.tile([128, 64], f32, tag="cums", name="cums")
                    nc.vector.tensor_tensor_scan(cums, mask, zero128, 0.0,
                                                 op0=ALU.add, op1=ALU.add)
                    # slot = cums*mask - 1  (-1 marks non-selected: ignored
                    # by local_scatter)
                    nc.vector.tensor_mul(cums, cums, mask)
                    nc.vector.tensor_scalar_add(cums, cums, -1.0)
                    slot = tkp.tile([128, 64], i16, tag="slot", name="slot")
                    nc.vector.tensor_copy(slot, cums)
                    sel = tkp.tile([128, 24], u16, tag="sel", name="sel")
                    nc.gpsimd.local_scatter(out_ap=sel, data_ap=cidx,
                                            idxs_ap=slot, channels=128,
                                            num_elems=24, num_idxs=64)
                    nc.sync.dma_start(
                        idx_dram[t * 128 : (t + 1) * 128, :],
                        sel[:, 0:KNN].bitcast(i16),
                    )

                # gather calls: (k0, nk, widx column base within the group).
                # The first call covers a single neighbor slot so the edge
                # pipeline starts as soon as possible after the peels.
                CALLS = [(0, 1, 0), (1, 4, 64), (5, 4, 320), (9, 4, 576),
                         (13, 4, 832), (17, 3, 1088)]

                def reformat(grp):
                    """widx[p, grp*G + base + r*nk + kk]
                         = idx_dram[1024*grp + 16*r + p, k0 + kk],
                    replicated over the four 16-partition groups; one DMA per
                    (gather call, replica) so the first call's indices land
                    first and later transfers overlap the gathers."""
                    for k0, nk, base in CALLS:
                        src = idx_dram[grp * 1024 : (grp + 1) * 1024,
                                       k0 : k0 + nk].rearrange(
                            "(r p) k -> p r k", p=16
                        )
                        for rep in range(4):
                            dst = widx[rep * 16 : (rep + 1) * 16,
                                       grp * G + base
                                       : grp * G + base + 64 * nk].rearrange(
                                "p (r k) -> p r k", k=nk
                            )
                            nc.sync.dma_start(dst, src)

                def gather_call(grp, call):
                    k0, nk, base = CALLS[call]
                    g = gp.tile([64, 4096], f32, tag="g", name="g")
                    nc.gpsimd.ap_gather(
                        out_ap=g[:, 0 : 1024 * nk],
                        in_ap=u_t,
                        idxs_ap=widx[:, grp * G + base
                                     : grp * G + base + 64 * nk],
                        channels=64,
                        num_elems=N,
                        d=1,
                        num_idxs=1024 * nk,
                    )
                    # columns are (r, kk)-interleaved: col = nk*16*r + 16*kk + p
                    return g[:, 0 : 1024 * nk].rearrange(
                        "c (r f p) -> c f r p", f=nk, p=16
                    )

                def edge_k(lid, grp, gv, kk, k, zacc):
                    """one neighbor slot: y = Prelu(I u_j + V x_i + c);
                    z psum = W2 y; zacc = max(zacc, z).  Layer 3 needs only
                    the running max of the gathered u."""
                    P = LAYER[lid]
                    cin = P["cin"]
                    gk = gv[:, kk]  # (64, 64, 16): r-major, p-minor
                    if lid == 3:
                        gsl = slice(grp * 1024, (grp + 1) * 1024)
                        a3 = acc3[:, gsl].rearrange("c (r p) -> c r p", p=16)
                        if k == 0:
                            nc.vector.tensor_copy(a3, gk)
                        else:
                            nc.vector.tensor_max(a3, a3, gk)
                        return
                    for q in range(2):
                        rsl = slice(q * 32, (q + 1) * 32)
                        csl = slice(q * 512, (q + 1) * 512)
                        msl = slice(grp * 1024 + q * 512,
                                    grp * 1024 + (q + 1) * 512)
                        yp = pse.tile([64, 512], f32, tag="e", name="yp")
                        nc.tensor.matmul(yp, wsb["id64"], gk[:, rsl],
                                         start=True, stop=False)
                        nc.tensor.matmul(yp, wsb[P["vTh"]], x2r[0:cin, msl],
                                         start=False, stop=True)
                        y = ypl.tile([64, 512], bf, tag="y", name="y")
                        nc.scalar.activation(y, yp, AF.Prelu, bias=wsb[P["c"]],
                                             alpha=0.2)
                        zp = pse.tile([64, 512], f32, tag="c2", name="zp")
                        nc.tensor.matmul(zp, wsb[P["w2"]], y)
                        if k == 0:
                            nc.vector.tensor_copy(zacc[:, csl], zp)
                        else:
                            nc.vector.tensor_max(zacc[:, csl], zacc[:, csl],
                                                 zp)

                def edge_fin(lid, grp, zacc):
                    P = LAYER[lid]
                    gsl = slice(grp * 1024, (grp + 1) * 1024)
                    x_out = XOUT[lid]
                    if lid == 3:
                        for q in range(2):
                            sl = slice(grp * 1024 + q * 512,
                                       grp * 1024 + (q + 1) * 512)
                            pe_ = pse.tile([64, 512], f32, tag="e", name="pe3")
                            nc.tensor.matmul(pe_, wsb["id64"], acc3[:, sl],
                                             start=True, stop=False)
                            nc.tensor.matmul(pe_, wsb[P["vTh"]],
                                             x2r[0 : P["cin"], sl],
                                             start=False, stop=True)
                            nc.scalar.activation(x_out[:, sl], pe_, AF.Prelu,
                                                 bias=wsb[P["c"]], alpha=0.2)
                    else:
                        nc.scalar.activation(x_out[:, gsl], zacc, AF.Prelu,
                                             bias=wsb[P["cc"]], alpha=0.2)

                ccs = {}

                def exchange_start(lid, grp):
                    """export x_out group plus the hi/lo bf16 pair carrying
                    its fp32 -|x|^2 row, and launch the pairwise AllGather."""
                    gsl = slice(grp * 1024, (grp + 1) * 1024)
                    x_half = XOUT[lid]
                    xsq = ppl.tile([64, 1024], f32, tag="xsq", name="xsqe")
                    nc.scalar.activation(xsq, x_half[:, gsl], AF.Square)
                    sqrow = ppl.tile([1, 1024], f32, tag="sqr", name="sqrowe")
                    for j in range(2):
                        jl = slice(j * 512, (j + 1) * 512)
                        pq = psd.tile([128, 1024], f32, tag="d", name="pqe")
                        nc.tensor.matmul(pq[0:1, 0:512], wsb["ones64"],
                                         xsq[:, jl])
                        nc.scalar.mul(sqrow[:, jl], pq[0:1, 0:512], -1.0)
                    hi = ppl.tile([1, 1024], bf, tag="hi", name="hi")
                    nc.scalar.copy(hi, sqrow)
                    lo = ppl.tile([1, 1024], bf, tag="lo", name="lo")
                    nc.vector.tensor_tensor(lo, sqrow, hi, op=ALU.subtract)
                    ccin = drp.tile([66, 1024], bf, tag="cci%d" % grp,
                                    name="ccin%d" % grp)
                    nc.scalar.dma_start(ccin[0:64], x_half[:, gsl])
                    nc.scalar.dma_start(ccin[64:65], hi)
                    nc.scalar.dma_start(ccin[65:66], lo)
                    ccout = drp.tile([132, 1024], bf, tag="cco%d" % grp,
                                     name="ccout%d" % grp)
                    nc.gpsimd.collective_compute(
                        "AllGather", ALU.bypass, replica_groups=PAIRS,
                        ins=[ccin], outs=[ccout],
                    )
                    return ccout

                def exchange_finish(ccout, grp):
                    lo0 = grp * 1024
                    lo1 = 2048 + grp * 1024
                    nc.sync.dma_start(rhsF[0:64, lo0 : lo0 + 1024], ccout[0:64])
                    nc.sync.dma_start(rhsF[64:66, lo0 : lo0 + 1024],
                                      ccout[64:66])
                    nc.sync.dma_start(rhsF[0:64, lo1 : lo1 + 1024],
                                      ccout[66:130])
                    nc.sync.dma_start(rhsF[64:66, lo1 : lo1 + 1024],
                                      ccout[130:132])

                def lhs_prep(lid, grp):
                    """lhsTm/x2r columns for group grp from layer lid's out."""
                    gsl = slice(grp * 1024, (grp + 1) * 1024)
                    x_out = XOUT[lid]
                    nc.scalar.mul(lhsTm[0:64, gsl], x_out[:, gsl], 2.0)
                    nc.scalar.mul(x2r[0:64, gsl], x_out[:, gsl], 2.0)

                def edge_steps(lid, grp):
                    """emission steps for the edge phase of (lid, grp); the
                    final step also launches the AllGather for layers 1-2."""
                    steps = []
                    state = {}
                    zacc = None
                    if lid != 3:
                        zacc = zap.tile([64, 1024], f32, tag="za", name="zacc")

                    def mk_gather(call):
                        def f():
                            state[call] = gather_call(grp, call)
                        return f

                    def mk_k(call, kk, k):
                        def f():
                            edge_k(lid, grp, state[call], kk, k, zacc)
                        return f

                    def fin():
                        edge_fin(lid, grp, zacc)
                        if lid < 3:
                            ccs[grp] = exchange_start(lid, grp)
                            lhs_prep(lid, grp)

                    for call, (k0, nk, base) in enumerate(CALLS):
                        steps.append(mk_gather(call))
                        for kk in range(nk):
                            steps.append(mk_k(call, kk, k0 + kk))
                    steps.append(fin)
                    return steps

                carry = []

                def phase_B(lid):
                    """grp0 edge (leading, so its AllGather fires early)
                    followed by grp1 tiles' dist/topk."""
                    a = []
                    for t in range(8, 16):
                        for blk in range(4):
                            a.append(lambda t=t, blk=blk:
                                     dist_quarter(lid, t, blk))
                        a.append(lambda t=t: peel_compact(t))
                    for f in a[:4]:
                        f()
                    for f in edge_steps(lid, 0):
                        f()
                    for f in a[4:]:
                        f()
                    reformat(1)

                def phase_C(lid):
                    """grp1 edge followed by the next layer's lead work (or
                    conv6's first half for layer 3)."""
                    nxt = lid + 1 if lid < 3 else None
                    if nxt is not None:
                        exchange_finish(ccs[0], 0)
                    es = edge_steps(lid, 1)
                    if nxt is None:
                        _mix([lambda ob=ob: conv6_chunk(ob, 0)
                              for ob in range(8)], es)
                        return
                    for f in es:
                        f()
                    prep_block(nxt, 0)
                    prep_block(nxt, 2048)
                    for t in range(8):
                        dist_quarter(nxt, t, 0)
                    for t in range(3):
                        dist_quarter(nxt, t, 2)
                    for t in range(3, 8):
                        carry.append(lambda t=t: dist_quarter(nxt, t, 2))

                def phase_A(lid):
                    """finish the leftover and second-half dist quarters of
                    tiles 0..8 for layer lid, then peel."""
                    exchange_finish(ccs[1], 1)
                    for f in carry:
                        f()
                    carry.clear()
                    prep_block(lid, 1024)
                    prep_block(lid, 3072)
                    for t in range(8):
                        dist_quarter(lid, t, 1)
                        dist_quarter(lid, t, 3)
                    for t in range(8):
                        peel_compact(t)
                    reformat(0)

                def conv6_chunk(ob, jp):
                    obs = slice(ob * 128, (ob + 1) * 128)
                    pf = psd.tile([128, 1024], f32, tag="d", name="pf6")
                    for q in range(2):
                        sl = slice(jp * 1024 + q * 512,
                                   jp * 1024 + (q + 1) * 512)
                        po = slice(q * 512, (q + 1) * 512)
                        nc.tensor.matmul(pf[:, po], wsb["w6aT"][:, obs],
                                         xcat_a[:, sl], start=True, stop=False)
                        nc.tensor.matmul(pf[:, po], wsb["w6bT"][:, obs],
                                         xcat_b[:, sl], start=False, stop=True)
                    h6 = gp.tile([128, 1024], f32, tag="h6", name="h6", bufs=2)
                    nc.scalar.activation(h6, pf, AF.Prelu,
                                         bias=wsb["c6v"][:, ob : ob + 1],
                                         alpha=0.2)
                    xgt = xgts[ob]
                    nc.vector.reduce_max(xgt[:, jp : jp + 1], h6,
                                         axis=mybir.AxisListType.X)

                xgts = [tkp.tile([128, 2], f32, tag="xgt%d" % ob,
                                 name="xgt%d" % ob) for ob in range(8)]

                # ================= layer 1 lead-in =================
                for blk in range(4):
                    prep_block1(blk * 1024)
                    for t in range(8):
                        dist_quarter(1, t, blk)
                for t in range(8):
                    peel_compact(t)
                reformat(0)

                # ================= the three layers =================
                phase_B(1)
                phase_C(1)
                phase_A(2)
                phase_B(2)
                phase_C(2)
                phase_A(3)
                phase_B(3)
                phase_C(3)

                # ============ conv6 second half + global max pool ============
                for ob in range(8):
                    conv6_chunk(ob, 1)
                    nc.vector.reduce_max(xg_sb[:, ob : ob + 1], xgts[ob],
                                         axis=mybir.AxisListType.X)
                ccg_i = drp.tile([128, 8], f32, tag="cgi", name="ccg_in")
                nc.scalar.dma_start(ccg_i, xg_sb)
                ccg_o = drp.tile([128, 8], f32, tag="cgo", name="ccg_out")
                nc.gpsimd.collective_compute(
                    "AllReduce", ALU.max, replica_groups=PAIRS,
                    ins=[ccg_i], outs=[ccg_o],
                )

            # layer scratch pools released here; final stage below.
            with tc.tile_pool(name="fin", bufs=1) as fin:
                # conv7 local part without bias (overlaps the AllReduce)
                h7 = fin.tile([128, 4 * H], bf, tag="h7", name="h7")
                for ob in range(4):
                    obs = slice(ob * 128, (ob + 1) * 128)
                    for jp in range(2):
                        pf = psd.tile([128, 1024], f32, tag="d", name="pf7")
                        for q in range(2):
                            sl = slice(jp * 1024 + q * 512,
                                       jp * 1024 + (q + 1) * 512)
                            po = slice(q * 512, (q + 1) * 512)
                            nc.tensor.matmul(pf[:, po], wsb["w7laT"][:, obs],
                                             xcat_a[:, sl],
                                             start=True, stop=False)
                            nc.tensor.matmul(pf[:, po], wsb["w7lbT"][:, obs],
                                             xcat_b[:, sl],
                                             start=False, stop=True)
                        nc.scalar.copy(
                            h7[:, ob * H + jp * 1024 : ob * H + (jp + 1) * 1024],
                            pf,
                        )

                nc.sync.dma_start(xg_sb, ccg_o)

                # conv7 effective bias: c7 + W7g @ xg
                for ob in range(4):
                    pb = psd.tile([128, 1024], f32, tag="d", name="pb7")
                    for kb in range(8):
                        nc.tensor.matmul(
                            pb[:, 0:1],
                            wsb["w7gT"][:, kb, ob * 128 : (ob + 1) * 128],
                            xg_sb[:, kb : kb + 1],
                            start=(kb == 0),
                            stop=(kb == 7),
                        )
                    nc.scalar.activation(b7_sb[:, ob : ob + 1], pb[:, 0:1],
                                         AF.Identity,
                                         bias=wsb["c7v"][:, ob : ob + 1])

                h7a = fin.tile([128, 4 * H], bf, tag="h7a", name="h7a")
                for ob in range(4):
                    for jp in range(4):
                        sl = slice(ob * H + jp * 512, ob * H + (jp + 1) * 512)
                        nc.scalar.activation(
                            h7a[:, sl], h7[:, sl], AF.Prelu,
                            bias=b7_sb[:, ob : ob + 1], alpha=0.2,
                        )
                h8 = fin.tile([128, 2 * H], bf, tag="h8", name="h8")
                for ob in range(2):
                    for jp in range(2):
                        pf = psd.tile([128, 1024], f32, tag="d", name="pf8")
                        for q in range(2):
                            po = slice(q * 512, (q + 1) * 512)
                            co = jp * 1024 + q * 512
                            for kb in range(4):
                                nc.tensor.matmul(
                                    pf[:, po],
                                    wsb["w8T"][:, kb, ob * 128 : (ob + 1) * 128],
                                    h7a[:, kb * H + co : kb * H + co + 512],
                                    start=(kb == 0),
                                    stop=(kb == 3),
                                )
                        nc.scalar.activation(
                            h8[:, ob * H + jp * 1024 : ob * H + (jp + 1) * 1024],
                            pf, AF.Prelu, bias=wsb["c8v"][:, ob : ob + 1],
                            alpha=0.2,
                        )
                o_sb = fin.tile([13, H], f32, tag="osb", name="o_sb")
                for jp in range(2):
                    pf = psd.tile([128, 1024], f32, tag="d", name="pf9")
                    for q in range(2):
                        po = slice(q * 512, (q + 1) * 512)
                        co = jp * 1024 + q * 512
                        for kb in range(2):
                            nc.tensor.matmul(
                                pf[0:13, po],
                                wsb["w9T"][:, kb, :],
                                h8[:, kb * H + co : kb * H + co + 512],
                                start=(kb == 0),
                                stop=(kb == 1),
                            )
                    nc.scalar.activation(
                        o_sb[:, jp * 1024 : (jp + 1) * 1024], pf[0:13],
                        AF.Identity, bias=wsb["b9v"],
                    )
                nc.sync.dma_start(out_d[:], o_sb)

    nc.compile()
    return nc


def _mix(a_steps, b_steps, lead=0):
    """Emit `lead` a-steps, then alternate a/b 1:1 until b is exhausted, then
    the remaining a-steps."""
    ia = 0
    for _ in range(min(lead, len(a_steps))):
        a_steps[ia]()
        ia += 1
    for ib in range(len(b_steps)):
        if ia < len(a_steps):
            a_steps[ia]()
            ia += 1
        b_steps[ib]()
    while ia < len(a_steps):
        a_steps[ia]()
        ia += 1


def make_in_maps(inputs):
    """Per-core input dicts from the full problem inputs."""
    wd = _prep_weights(inputs)
    wpack32 = np.zeros((128, _wcols(0)), dtype=np.float32)
    import ml_dtypes

    wpack16 = np.zeros((128, _wcols(1)), dtype=ml_dtypes.bfloat16)
    col = {0: 0, 1: 0}
    for name, shape, isb in _WEIGHT_SPECS:
        w = int(np.prod(shape[1:])) if len(shape) > 1 else 1
        dst = wpack16 if isb else wpack32
        v = wd[name].reshape(shape[0], w)
        dst[0 : shape[0], col[isb] : col[isb] + w] = (
            _bf16(v) if isb else v.astype(np.float32)
        )
        col[isb] += w
    pts = np.asarray(inputs["points"], dtype=np.float32)
    in_maps = []
    for c in range(8):
        b, h = c // 2, c % 2
        m = {"wpack32": wpack32, "wpack16": wpack16}
        m["choff"] = np.ascontiguousarray(
            np.tile(np.repeat(np.arange(8, dtype=np.uint16) * 512, 8), (128, 1)))
        m["pts_full"] = np.ascontiguousarray(pts[b])
        pm = np.ascontiguousarray(pts[b][:, h * H : (h + 1) * H])
        m["pts_mine"] = pm
        m["pts_mine_bf"] = _bf16(pm)
        in_maps.append(m)
    return in_maps


def kernel(**inputs):
    from concourse.bass_utils import run_bass_kernel_spmd

    if "nc" not in _CACHE:
        _CACHE["nc"] = build_kernel()
    nc = _CACHE["nc"]
    in_maps = make_in_maps(inputs)
    res = run_bass_kernel_spmd(nc, in_maps, core_ids=list(range(8)))
    out = np.zeros((B, 13, N), dtype=np.float32)
    for c in range(8):
        b, h = c // 2, c % 2
        out[b][:, h * H : (h + 1) * H] = res.results[c]["out"]
    return out


# revision 51
# speedup vs baseline: 1.7313x; 1.0022x over previous
"""DGCNN part-segmentation forward pass on 8 Trainium2 NeuronCores.

Sharding: data-parallel over the batch (B=4) x 2-way split of the N=4096
points within each batch element.  Core c handles batch element c//2,
point rows [(c%2)*2048, (c%2+1)*2048).  The two cores of a pair exchange
their half of each EdgeConv output with pairwise AllGathers (one per
1024-point group, launched as soon as that group's output is ready) and
a pairwise AllReduce-max for the global pooling feature.

Precision: the feature path (EdgeConv transforms, convs 6-9) runs in
bfloat16 with fp32 PSUM accumulation.  Layer 1's kNN runs on raw fp32
points (3-D points have many near-ties; bf16 inputs measurably flip
neighbors), via a small separate fp32 lhs/rhs pair.  Layers 2-3 rank
neighbors of the bf16 features exactly: the distance matmul contracts
66 bf16 rows = 64 feature channels + a hi/lo bf16 pair carrying the
fp32 -|x_j|^2 row at ~16-bit precision.  The sq rows travel inside the
AllGather so no distance prep remains on the post-exchange critical
path.

The emission order is software-pipelined: engine queues are in-order,
so each phase leads with a few distance tiles, then runs the edge-conv
phase solid (its AllGather fires early), then the bulk of the
distance/top-k scans; the next layer's first distance quarters overlap
the current layer's second edge phase.

Per EdgeConv layer:
  top-20: per 128-row tile: 8x top-8 per 512-column chunk (DVE Max) ->
          64 candidates; 3 peeling rounds (max8 + match_replace) ->
          top-24 values; threshold mask + prefix-sum compaction +
          gpsimd local_scatter -> global indices of the top-20
  gather: batched GPSIMD ap_gather of the bf16 u = U x transform; the
          first call covers one neighbor slot so the edge pipeline
          starts right after the peels, later calls cover 4 slots
  edge  : psum = I*u_gathered + V*x_i (bf16 matmuls), LeakyReLU (Prelu)
          on the scalar engine, conv2 bf16 matmul; the k-max runs on
          the raw conv2 psums (LeakyReLU is monotone) and one Prelu
          finalizes each group.
"""

import sys

sys.path.insert(0, "/opt/trn_rl_repo")

import numpy as np

B = 4
N = 4096
H = 2048  # points per core (half of a batch element)
KNN = 20
EPS = 1e-5
NEG = -3.0e38
G = KNN * 64  # widx columns per point group

_CACHE = {}


# --------------------------------------------------------------------------
# host-side weight preparation
# --------------------------------------------------------------------------

def _bf16(x):
    import ml_dtypes

    return np.ascontiguousarray(x, dtype=np.float32).astype(ml_dtypes.bfloat16)


def _fold_bn(w, b, g, be):
    s = (g / np.sqrt(np.float32(1.0) + np.float32(EPS))).astype(np.float32)
    return (w * s[:, None]).astype(np.float32), (s * b + be).astype(np.float32)


def _prep_weights(inp):
    f = np.float32
    W, C = {}, {}
    for i in range(1, 9):
        W[i], C[i] = _fold_bn(
            inp["w%d" % i], inp["b%d" % i], inp["g%d" % i], inp["be%d" % i]
        )
    d = {}
    # edge conv layers: split into U (neighbor part) and V (center part).
    # Layer 1's x2r tile holds raw points, layers 2-3 hold 2*x, so the V
    # transpose is halved only for layers 2-3.
    for lid, wi, vscale in ((1, 1, 1.0), (2, 3, 0.5), (3, 5, 0.5)):
        w = W[wi]
        cin = w.shape[1] // 2
        U = w[:, :cin]
        V = w[:, cin:] - w[:, :cin]
        d["u%dT" % lid] = np.ascontiguousarray(U.T)
        d["v%dTh" % lid] = np.ascontiguousarray((V * f(vscale)).T)
        d["c%d" % lid] = C[wi].reshape(64, 1)
    d["w2T"] = np.ascontiguousarray(W[2].T)
    d["cc2"] = C[2].reshape(64, 1)
    d["w4T"] = np.ascontiguousarray(W[4].T)
    d["cc4"] = C[4].reshape(64, 1)
    # conv6 (192 -> 1024)
    w6T = np.ascontiguousarray(W[6].T)  # (192, 1024)
    d["w6aT"] = np.ascontiguousarray(w6T[:128])
    d["w6bT"] = np.ascontiguousarray(w6T[128:])
    d["c6v"] = np.ascontiguousarray(C[6].reshape(8, 128).T)  # (128, 8)
    # conv7 (1216 -> 512): xg part (1024) and local part (192)
    w7 = W[7]
    w7gT = np.ascontiguousarray(w7[:, :1024].T)  # (1024, 512)
    d["w7gT"] = np.ascontiguousarray(w7gT.reshape(8, 128, 512).transpose(1, 0, 2))
    w7lT = np.ascontiguousarray(w7[:, 1024:].T)  # (192, 512)
    d["w7laT"] = np.ascontiguousarray(w7lT[:128])
    d["w7lbT"] = np.ascontiguousarray(w7lT[128:])
    d["c7v"] = np.ascontiguousarray(C[7].reshape(4, 128).T)  # (128, 4)
    # conv8 (512 -> 256)
    w8T = np.ascontiguousarray(W[8].T)  # (512, 256)
    d["w8T"] = np.ascontiguousarray(w8T.reshape(4, 128, 256).transpose(1, 0, 2))
    d["c8v"] = np.ascontiguousarray(C[8].reshape(2, 128).T)  # (128, 2)
    # conv9 (256 -> 13), plain linear
    w9T = np.ascontiguousarray(inp["w9"].T.astype(f))  # (256, 13)
    d["w9T"] = np.ascontiguousarray(w9T.reshape(2, 128, 13).transpose(1, 0, 2))
    d["b9v"] = inp["b9"].astype(f).reshape(13, 1)
    # constants
    d["id64"] = np.eye(64, dtype=f)
    d["ones3"] = np.ones((3, 1), dtype=f)
    d["ones64"] = np.ones((64, 1), dtype=f)
    return d


# name -> (shape, is_bf16)
_WEIGHT_SPECS = [
    ("u1T", (3, 64), 0), ("v1Th", (3, 64), 1), ("c1", (64, 1), 0),
    ("w2T", (64, 64), 1), ("cc2", (64, 1), 0),
    ("u2T", (64, 64), 1), ("v2Th", (64, 64), 1), ("c2", (64, 1), 0),
    ("w4T", (64, 64), 1), ("cc4", (64, 1), 0),
    ("u3T", (64, 64), 1), ("v3Th", (64, 64), 1), ("c3", (64, 1), 0),
    ("w6aT", (128, 1024), 1), ("w6bT", (64, 1024), 1), ("c6v", (128, 8), 0),
    ("w7gT", (128, 8, 512), 0), ("w7laT", (128, 512), 1),
    ("w7lbT", (64, 512), 1), ("c7v", (128, 4), 0),
    ("w8T", (128, 4, 256), 1), ("c8v", (128, 2), 0),
    ("w9T", (128, 2, 13), 1), ("b9v", (13, 1), 0),
    ("id64", (64, 64), 0), ("ones3", (3, 1), 0), ("ones64", (64, 1), 0),
]


def _wcols(bf):
    return sum(int(np.prod(s[1:])) if len(s) > 1 else 1
               for _, s, isb in _WEIGHT_SPECS if isb == bf)


# --------------------------------------------------------------------------
# device kernel builder
# --------------------------------------------------------------------------

def build_kernel():
    import concourse.bacc as bacc
    import concourse.mybir as mybir
    import concourse.tile as tile

    f32 = mybir.dt.float32
    bf = mybir.dt.bfloat16
    i16 = mybir.dt.int16
    u16 = mybir.dt.uint16
    AF = mybir.ActivationFunctionType
    ALU = mybir.AluOpType
    PAIRS = [[0, 1], [2, 3], [4, 5], [6, 7]]

    nc = bacc.Bacc("TRN2", target_bir_lowering=False, num_devices=8)

    pts_full = nc.dram_tensor("pts_full", [3, N], f32, kind="ExternalInput")
    pts_mine = nc.dram_tensor("pts_mine", [3, H], f32, kind="ExternalInput")
    pts_mine_bf = nc.dram_tensor("pts_mine_bf", [3, H], bf,
                                 kind="ExternalInput")
    choff_d = nc.dram_tensor("choff", [128, 64], u16, kind="ExternalInput")
    wboot_d = nc.dram_tensor("wboot", [3, 65], f32, kind="ExternalInput")
    wpack32_d = nc.dram_tensor("wpack32", [128, _wcols(0)], f32,
                               kind="ExternalInput")
    wpack16_d = nc.dram_tensor("wpack16", [128, _wcols(1)], bf,
                               kind="ExternalInput")
    out_d = nc.dram_tensor("out", [13, H], f32, kind="ExternalOutput")

    with tile.TileContext(nc) as tc:
        with (
            tc.tile_pool(name="wp", bufs=1) as wp,
            tc.tile_pool(name="per", bufs=1) as per,
            tc.tile_pool(name="psd", bufs=2, space="PSUM") as psd,
            tc.tile_pool(name="pse", bufs=2, space="PSUM") as pse,
            tc.tile_pool(name="dram", bufs=1, space="DRAM") as drp,
        ):
            # ---- persistent tiles ----
            rhsF = per.tile([66, N], bf, tag="rhsF", name="rhsF")
            rhsF1 = per.tile([4, N], f32, tag="rhsF1", name="rhsF1")
            lhsTm = per.tile([66, H], bf, tag="lhsTm", name="lhsTm")
            lhsTm1 = per.tile([4, H], f32, tag="lhsTm1", name="lhsTm1")
            u_t = per.tile([64, N], f32, tag="u", name="u_t")
            x2r = per.tile([64, H], bf, tag="x2r", name="x2r")
            xcat_a = per.tile([128, H], bf, tag="xcata", name="xcat_a")
            xcat_b = per.tile([64, H], bf, tag="xcatb", name="xcat_b")
            widx = per.tile([64, KNN * 128], i16, tag="widx", name="widx")
            acc3 = per.tile([64, H], f32, tag="acc3", name="acc3")
            xg_sb = per.tile([128, 8], f32, tag="xg", name="xg_sb")
            b7_sb = per.tile([128, 4], f32, tag="b7", name="b7_sb")
            choff_sb = per.tile([128, 64], u16, tag="choff", name="choff_sb")
            zero128 = per.tile([128, 64], f32, tag="z128", name="zero128")

            idx_dram = drp.tile([H, KNN], i16, tag="idxd", name="idx_dram")

            x1h = xcat_a[0:64]
            x2h = xcat_a[64:128]
            x3h = xcat_b

            # critical inputs first; the row-3 ones of lhsTm1 come from a
            # 32-aligned memset that the pts DMA then partially overwrites
            wbt = wp.tile([3, 65], f32, tag="wb", name="wboot_sb")
            nc.sync.dma_start(wbt, wboot_d[:])
            nc.vector.memset(lhsTm1[0:4], 1.0)
            nc.sync.dma_start(lhsTm1[0:3], pts_mine[:])
            nc.sync.dma_start(x2r[0:3], pts_mine_bf[:])
            nc.sync.dma_start(choff_sb, choff_d[:])
            nc.vector.memset(zero128, 0.0)
            nc.vector.memset(lhsTm[64:66], 1.0)
            # all weights arrive in two packed DMAs; per-weight tiles are
            # views into the packed tiles
            wpt32 = wp.tile([128, _wcols(0)], f32, tag="wp32", name="wp32")
            nc.scalar.dma_start(wpt32, wpack32_d[:])
            wpt16 = wp.tile([128, _wcols(1)], bf, tag="wp16", name="wp16")
            nc.scalar.dma_start(wpt16, wpack16_d[:])
            wsb = {}
            col = {0: 0, 1: 0}
            for name, shape, isb in _WEIGHT_SPECS:
                w = int(np.prod(shape[1:])) if len(shape) > 1 else 1
                src = wpt16 if isb else wpt32
                v = src[0 : shape[0], col[isb] : col[isb] + w]
                if len(shape) == 3:
                    v = v.rearrange("p (a b) -> p a b", a=shape[1])
                wsb[name] = v
                col[isb] += w
            wsb["u1T"] = wbt[0:3, 0:64]
            wsb["ones3"] = wbt[0:3, 64:65]

            LAYER = {
                1: dict(cin=3, uT="u1T", vTh="v1Th", c="c1", w2="w2T",
                        cc="cc2"),
                2: dict(cin=64, uT="u2T", vTh="v2Th", c="c2", w2="w4T",
                        cc="cc4"),
                3: dict(cin=64, uT="u3T", vTh="v3Th", c="c3", w2=None,
                        cc=None),
            }
            XOUT = {1: x1h, 2: x2h, 3: x3h}

            with (
                tc.tile_pool(name="dsb", bufs=4) as dsbp,
                tc.tile_pool(name="pp", bufs=2) as ppl,
                tc.tile_pool(name="tk", bufs=2) as tkp,
                tc.tile_pool(name="gp", bufs=2) as gp,
                tc.tile_pool(name="yp", bufs=3) as ypl,
                tc.tile_pool(name="za", bufs=1) as zap,
            ):
                cands = {}

                def prep_block1(lo):
                    """layer-1 lead: rhsF1 = [2*pts; -|pts|^2] and
                    u_t = u1T.T @ pts for columns [lo, lo+1024)."""
                    stage = ppl.tile([3, 1024], f32, tag="pts", name="stage")
                    nc.sync.dma_start(stage, pts_full[:, lo : lo + 1024])
                    nc.scalar.mul(rhsF1[0:3, lo : lo + 1024], stage, 2.0)
                    xsq = ppl.tile([64, 1024], f32, tag="xsq", name="xsq1")
                    nc.scalar.activation(xsq[0:3], stage, AF.Square)
                    sqrow = ppl.tile([1, 1024], f32, tag="sqr", name="sqrow1")
                    for j in range(2):
                        jl = slice(j * 512, (j + 1) * 512)
                        pq = psd.tile([128, 1024], f32, tag="d", name="pq")
                        nc.tensor.matmul(pq[0:1, 0:512], wsb["ones3"],
                                         xsq[0:3, jl])
                        nc.scalar.mul(sqrow[:, jl], pq[0:1, 0:512], -1.0)
                        pu = psd.tile([128, 1024], f32, tag="d", name="pu")
                        nc.tensor.matmul(pu[0:64, 0:512], wsb["u1T"],
                                         stage[:, jl])
                        nc.scalar.copy(u_t[:, lo + j * 512 : lo + (j + 1) * 512],
                                       pu[0:64, 0:512])
                    nc.sync.dma_start(rhsF1[3:4, lo : lo + 1024], sqrow)

                def prep_block(lid, lo):
                    """u_t[:, lo:lo+1024] = uT.T @ rhsF[0:cin] (bf16)."""
                    P = LAYER[lid]
                    cin = P["cin"]
                    for j in range(2):
                        sl = slice(lo + j * 512, lo + (j + 1) * 512)
                        pu = psd.tile([128, 1024], f32, tag="d", name="pu")
                        nc.tensor.matmul(pu[0:64, 0:512], wsb[P["uT"]],
                                         rhsF[0:cin, sl])
                        nc.scalar.copy(u_t[:, sl], pu[0:64, 0:512])

                def dist_quarter(lid, t, blk):
                    """distance psum + copy + top8 scan for tile t, column
                    block blk (1024 cols), ranking for layer lid."""
                    if t not in cands:
                        cands[t] = (
                            tkp.tile([128, 64], f32, tag="cand", name="cand",
                                     bufs=12),
                            tkp.tile([128, 64], u16, tag="cidx", name="cidx",
                                     bufs=12),
                        )
                    cand, cidx = cands[t]
                    lo = blk * 1024
                    pd = psd.tile([128, 1024], f32, tag="d", name="pd")
                    for q in range(2):
                        cs = slice(lo + q * 512, lo + (q + 1) * 512)
                        if lid == 1:
                            nc.tensor.matmul(
                                pd[:, q * 512 : (q + 1) * 512],
                                lhsTm1[:, t * 128 : (t + 1) * 128],
                                rhsF1[:, cs],
                            )
                        else:
                            nc.tensor.matmul(
                                pd[:, q * 512 : (q + 1) * 512],
                                lhsTm[:, t * 128 : (t + 1) * 128],
                                rhsF[:, cs],
                            )
                    dsb = dsbp.tile([128, 1024], f32, tag="dsb", name="dsb")
                    nc.scalar.copy(dsb, pd)
                    for q in range(2):
                        cc = blk * 2 + q
                        nc.vector.max(
                            out=cand[:, cc * 8 : (cc + 1) * 8],
                            in_=dsb[:, q * 512 : (q + 1) * 512],
                        )
                        nc.vector.max_index(
                            cidx[:, cc * 8 : (cc + 1) * 8],
                            cand[:, cc * 8 : (cc + 1) * 8],
                            dsb[:, q * 512 : (q + 1) * 512],
                        )

                def peel_compact(t):
                    """top-20 selection for tile t from its 64 candidates."""
                    cand, cidx = cands.pop(t)
                    nc.vector.tensor_add(cidx, cidx, choff_sb)
                    candw = tkp.tile([128, 64], f32, tag="candw", name="candw")
                    nc.vector.tensor_copy(candw, cand)
                    t8 = tkp.tile([128, 24], f32, tag="t8", name="t8")
                    nc.vector.max(out=t8[:, 0:8], in_=candw)
                    nc.vector.match_replace(out=candw, in_to_replace=t8[:, 0:8],
                                            in_values=candw, imm_value=NEG)
                    nc.vector.max(out=t8[:, 8:16], in_=candw)
                    nc.vector.match_replace(out=candw,
                                            in_to_replace=t8[:, 8:16],
                                            in_values=candw, imm_value=NEG)
                    nc.vector.max(out=t8[:, 16:24], in_=candw)
                    mask = tkp.tile([128, 64], f32, tag="mask", name="mask")
                    nc.vector.tensor_scalar(mask, cand, t8[:, 19:20], None,
                                            op0=ALU.is_ge)
                    cums = tkp.tile([128, 64], f32, tag="cums", name="cums")
                    nc.vector.tensor_tensor_scan(cums, mask, zero128, 0.0,
                                                 op0=ALU.add, op1=ALU.add)
                    # slot = cums*mask - 1  (-1 marks non-selected: ignored
                    # by local_scatter)
                    nc.vector.tensor_mul(cums, cums, mask)
                    nc.vector.tensor_scalar_add(cums, cums, -1.0)
                    slot = tkp.tile([128, 64], i16, tag="slot", name="slot")
                    nc.vector.tensor_copy(slot, cums)
                    sel = tkp.tile([128, 24], u16, tag="sel", name="sel")
                    nc.gpsimd.local_scatter(out_ap=sel, data_ap=cidx,
                                            idxs_ap=slot, channels=128,
                                            num_elems=24, num_idxs=64)
                    nc.sync.dma_start(
                        idx_dram[t * 128 : (t + 1) * 128, :],
                        sel[:, 0:KNN].bitcast(i16),
                    )

                # gather calls: (k0, nk, widx column base within the group).
                # The first call covers a single neighbor slot so the edge
                # pipeline starts as soon as possible after the peels.
                CALLS = [(0, 1, 0), (1, 4, 64), (5, 4, 320), (9, 4, 576),
                         (13, 4, 832), (17, 3, 1088)]

                def reformat(grp):
                    """widx[p, grp*G + base + r*nk + kk]
                         = idx_dram[1024*grp + 16*r + p, k0 + kk],
                    replicated over the four 16-partition groups; one DMA per
                    (gather call, replica) so the first call's indices land
                    first and later transfers overlap the gathers."""
                    for k0, nk, base in CALLS:
                        src = idx_dram[grp * 1024 : (grp + 1) * 1024,
                                       k0 : k0 + nk].rearrange(
                            "(r p) k -> p r k", p=16
                        )
                        for rep in range(4):
                            dst = widx[rep * 16 : (rep + 1) * 16,
                                       grp * G + base
                                       : grp * G + base + 64 * nk].rearrange(
                                "p (r k) -> p r k", k=nk
                            )
                            nc.sync.dma_start(dst, src)

                def gather_call(grp, call):
                    k0, nk, base = CALLS[call]
                    g = gp.tile([64, 4096], f32, tag="g", name="g")
                    nc.gpsimd.ap_gather(
                        out_ap=g[:, 0 : 1024 * nk],
                        in_ap=u_t,
                        idxs_ap=widx[:, grp * G + base
                                     : grp * G + base + 64 * nk],
                        channels=64,
                        num_elems=N,
                        d=1,
                        num_idxs=1024 * nk,
                    )
                    # columns are (r, kk)-interleaved: col = nk*16*r + 16*kk + p
                    return g[:, 0 : 1024 * nk].rearrange(
                        "c (r f p) -> c f r p", f=nk, p=16
                    )

                def edge_k(lid, grp, gv, kk, k, zacc):
                    """one neighbor slot: y = Prelu(I u_j + V x_i + c);
                    z psum = W2 y; zacc = max(zacc, z).  Layer 3 needs only
                    the running max of the gathered u."""
                    P = LAYER[lid]
                    cin = P["cin"]
                    gk = gv[:, kk]  # (64, 64, 16): r-major, p-minor
                    if lid == 3:
                        gsl = slice(grp * 1024, (grp + 1) * 1024)
                        a3 = acc3[:, gsl].rearrange("c (r p) -> c r p", p=16)
                        if k == 0:
                            nc.vector.tensor_copy(a3, gk)
                        else:
                            nc.vector.tensor_max(a3, a3, gk)
                        return
                    for q in range(2):
                        rsl = slice(q * 32, (q + 1) * 32)
                        csl = slice(q * 512, (q + 1) * 512)
                        msl = slice(grp * 1024 + q * 512,
                                    grp * 1024 + (q + 1) * 512)
                        yp = pse.tile([64, 512], f32, tag="e", name="yp")
                        nc.tensor.matmul(yp, wsb["id64"], gk[:, rsl],
                                         start=True, stop=False)
                        nc.tensor.matmul(yp, wsb[P["vTh"]], x2r[0:cin, msl],
                                         start=False, stop=True)
                        y = ypl.tile([64, 512], bf, tag="y", name="y")
                        nc.scalar.activation(y, yp, AF.Prelu, bias=wsb[P["c"]],
                                             alpha=0.2)
                        zp = pse.tile([64, 512], f32, tag="c2", name="zp")
                        nc.tensor.matmul(zp, wsb[P["w2"]], y)
                        if k == 0:
                            nc.vector.tensor_copy(zacc[:, csl], zp)
                        else:
                            nc.vector.tensor_max(zacc[:, csl], zacc[:, csl],
                                                 zp)

                def edge_fin(lid, grp, zacc):
                    P = LAYER[lid]
                    gsl = slice(grp * 1024, (grp + 1) * 1024)
                    x_out = XOUT[lid]
                    if lid == 3:
                        for q in range(2):
                            sl = slice(grp * 1024 + q * 512,
                                       grp * 1024 + (q + 1) * 512)
                            pe_ = pse.tile([64, 512], f32, tag="e", name="pe3")
                            nc.tensor.matmul(pe_, wsb["id64"], acc3[:, sl],
                                             start=True, stop=False)
                            nc.tensor.matmul(pe_, wsb[P["vTh"]],
                                             x2r[0 : P["cin"], sl],
                                             start=False, stop=True)
                            nc.scalar.activation(x_out[:, sl], pe_, AF.Prelu,
                                                 bias=wsb[P["c"]], alpha=0.2)
                    else:
                        nc.scalar.activation(x_out[:, gsl], zacc, AF.Prelu,
                                             bias=wsb[P["cc"]], alpha=0.2)

                ccs = {}

                def exchange_start(lid, grp):
                    """export x_out group plus the hi/lo bf16 pair carrying
                    its fp32 -|x|^2 row, and launch the pairwise AllGather."""
                    gsl = slice(grp * 1024, (grp + 1) * 1024)
                    x_half = XOUT[lid]
                    xsq = ppl.tile([64, 1024], f32, tag="xsq", name="xsqe")
                    nc.scalar.activation(xsq, x_half[:, gsl], AF.Square)
                    sqrow = ppl.tile([1, 1024], f32, tag="sqr", name="sqrowe")
                    for j in range(2):
                        jl = slice(j * 512, (j + 1) * 512)
                        pq = psd.tile([128, 1024], f32, tag="d", name="pqe")
                        nc.tensor.matmul(pq[0:1, 0:512], wsb["ones64"],
                                         xsq[:, jl])
                        nc.scalar.mul(sqrow[:, jl], pq[0:1, 0:512], -1.0)
                    hi = ppl.tile([1, 1024], bf, tag="hi", name="hi")
                    nc.scalar.copy(hi, sqrow)
                    lo = ppl.tile([1, 1024], bf, tag="lo", name="lo")
                    nc.vector.tensor_tensor(lo, sqrow, hi, op=ALU.subtract)
                    ccin = drp.tile([66, 1024], bf, tag="cci%d" % grp,
                                    name="ccin%d" % grp)
                    nc.scalar.dma_start(ccin[0:64], x_half[:, gsl])
                    nc.scalar.dma_start(ccin[64:65], hi)
                    nc.scalar.dma_start(ccin[65:66], lo)
                    ccout = drp.tile([132, 1024], bf, tag="cco%d" % grp,
                                     name="ccout%d" % grp)
                    nc.gpsimd.collective_compute(
                        "AllGather", ALU.bypass, replica_groups=PAIRS,
                        ins=[ccin], outs=[ccout],
                    )
                    return ccout

                def exchange_finish(ccout, grp):
                    lo0 = grp * 1024
                    lo1 = 2048 + grp * 1024
                    nc.sync.dma_start(rhsF[0:64, lo0 : lo0 + 1024], ccout[0:64])
                    nc.sync.dma_start(rhsF[64:66, lo0 : lo0 + 1024],
                                      ccout[64:66])
                    nc.sync.dma_start(rhsF[0:64, lo1 : lo1 + 1024],
                                      ccout[66:130])
                    nc.sync.dma_start(rhsF[64:66, lo1 : lo1 + 1024],
                                      ccout[130:132])

                def lhs_prep(lid, grp):
                    """lhsTm/x2r columns for group grp from layer lid's out."""
                    gsl = slice(grp * 1024, (grp + 1) * 1024)
                    x_out = XOUT[lid]
                    nc.scalar.mul(lhsTm[0:64, gsl], x_out[:, gsl], 2.0)
                    nc.scalar.mul(x2r[0:64, gsl], x_out[:, gsl], 2.0)

                def edge_steps(lid, grp):
                    """emission steps for the edge phase of (lid, grp); the
                    final step also launches the AllGather for layers 1-2."""
                    steps = []
                    state = {}
                    zacc = None
                    if lid != 3:
                        zacc = zap.tile([64, 1024], f32, tag="za", name="zacc")

                    def mk_gather(call):
                        def f():
                            state[call] = gather_call(grp, call)
                        return f

                    def mk_k(call, kk, k):
                        def f():
                            edge_k(lid, grp, state[call], kk, k, zacc)
                        return f

                    def fin():
                        edge_fin(lid, grp, zacc)
                        if lid < 3:
                            ccs[grp] = exchange_start(lid, grp)
                            lhs_prep(lid, grp)

                    for call, (k0, nk, base) in enumerate(CALLS):
                        steps.append(mk_gather(call))
                        for kk in range(nk):
                            steps.append(mk_k(call, kk, k0 + kk))
                    steps.append(fin)
                    return steps

                carry = []

                def phase_B(lid):
                    """grp0 edge (leading, so its AllGather fires early)
                    followed by grp1 tiles' dist/topk."""
                    a = []
                    for t in range(8, 16):
                        for blk in range(4):
                            a.append(lambda t=t, blk=blk:
                                     dist_quarter(lid, t, blk))
                        a.append(lambda t=t: peel_compact(t))
                    for f in a[:4]:
                        f()
                    for f in edge_steps(lid, 0):
                        f()
                    for f in a[4:]:
                        f()
                    reformat(1)

                def phase_C(lid):
                    """grp1 edge followed by the next layer's lead work (or
                    conv6's first half for layer 3)."""
                    nxt = lid + 1 if lid < 3 else None
                    if nxt is not None:
                        exchange_finish(ccs[0], 0)
                    es = edge_steps(lid, 1)
                    if nxt is None:
                        _mix([lambda ob=ob: conv6_chunk(ob, 0)
                              for ob in range(8)], es)
                        return
                    for f in es:
                        f()
                    prep_block(nxt, 0)
                    prep_block(nxt, 2048)
                    for t in range(8):
                        dist_quarter(nxt, t, 0)
                    for t in range(3):
                        dist_quarter(nxt, t, 2)
                    for t in range(3, 8):
                        carry.append(lambda t=t: dist_quarter(nxt, t, 2))

                def phase_A(lid):
                    """finish the leftover and second-half dist quarters of
                    tiles 0..8 for layer lid, then peel."""
                    exchange_finish(ccs[1], 1)
                    for f in carry:
                        f()
                    carry.clear()
                    prep_block(lid, 1024)
                    prep_block(lid, 3072)
                    for t in range(8):
                        dist_quarter(lid, t, 1)
                        dist_quarter(lid, t, 3)
                    for t in range(8):
                        peel_compact(t)
                    reformat(0)

                def conv6_chunk(ob, jp):
                    obs = slice(ob * 128, (ob + 1) * 128)
                    pf = psd.tile([128, 1024], f32, tag="d", name="pf6")
                    for q in range(2):
                        sl = slice(jp * 1024 + q * 512,
                                   jp * 1024 + (q + 1) * 512)
                        po = slice(q * 512, (q + 1) * 512)
                        nc.tensor.matmul(pf[:, po], wsb["w6aT"][:, obs],
                                         xcat_a[:, sl], start=True, stop=False)
                        nc.tensor.matmul(pf[:, po], wsb["w6bT"][:, obs],
                                         xcat_b[:, sl], start=False, stop=True)
                    h6 = gp.tile([128, 1024], f32, tag="h6", name="h6", bufs=2)
                    nc.scalar.activation(h6, pf, AF.Prelu,
                                         bias=wsb["c6v"][:, ob : ob + 1],
                                         alpha=0.2)
                    xgt = xgts[ob]
                    nc.vector.reduce_max(xgt[:, jp : jp + 1], h6,
                                         axis=mybir.AxisListType.X)

                xgts = [tkp.tile([128, 2], f32, tag="xgt%d" % ob,
                                 name="xgt%d" % ob) for ob in range(8)]

                # ================= layer 1 lead-in =================
                for blk in range(4):
                    prep_block1(blk * 1024)
                    for t in range(8):
                        dist_quarter(1, t, blk)
                for t in range(8):
                    peel_compact(t)
                reformat(0)

                # ================= the three layers =================
                phase_B(1)
                phase_C(1)
                phase_A(2)
                phase_B(2)
                phase_C(2)
                phase_A(3)
                phase_B(3)
                phase_C(3)

                # ============ conv6 second half + global max pool ============
                for ob in range(8):
                    conv6_chunk(ob, 1)
                    nc.vector.reduce_max(xg_sb[:, ob : ob + 1], xgts[ob],
                                         axis=mybir.AxisListType.X)
                ccg_i = drp.tile([128, 8], f32, tag="cgi", name="ccg_in")
                nc.scalar.dma_start(ccg_i, xg_sb)
                ccg_o = drp.tile([128, 8], f32, tag="cgo", name="ccg_out")
                nc.gpsimd.collective_compute(
                    "AllReduce", ALU.max, replica_groups=PAIRS,
                    ins=[ccg_i], outs=[ccg_o],
                )

            # layer scratch pools released here; final stage below.
            with tc.tile_pool(name="fin", bufs=1) as fin:
                # conv7 local part without bias (overlaps the AllReduce)
                h7 = fin.tile([128, 4 * H], bf, tag="h7", name="h7")
                for ob in range(4):
                    obs = slice(ob * 128, (ob + 1) * 128)
                    for jp in range(2):
                        pf = psd.tile([128, 1024], f32, tag="d", name="pf7")
                        for q in range(2):
                            sl = slice(jp * 1024 + q * 512,
                                       jp * 1024 + (q + 1) * 512)
                            po = slice(q * 512, (q + 1) * 512)
                            nc.tensor.matmul(pf[:, po], wsb["w7laT"][:, obs],
                                             xcat_a[:, sl],
                                             start=True, stop=False)
                            nc.tensor.matmul(pf[:, po], wsb["w7lbT"][:, obs],
                                             xcat_b[:, sl],
                                             start=False, stop=True)
                        nc.scalar.copy(
                            h7[:, ob * H + jp * 1024 : ob * H + (jp + 1) * 1024],
                            pf,
                        )

                nc.sync.dma_start(xg_sb, ccg_o)

                # conv7 effective bias: c7 + W7g @ xg
                for ob in range(4):
                    pb = psd.tile([128, 1024], f32, tag="d", name="pb7")
                    for kb in range(8):
                        nc.tensor.matmul(
                            pb[:, 0:1],
                            wsb["w7gT"][:, kb, ob * 128 : (ob + 1) * 128],
                            xg_sb[:, kb : kb + 1],
                            start=(kb == 0),
                            stop=(kb == 7),
                        )
                    nc.scalar.activation(b7_sb[:, ob : ob + 1], pb[:, 0:1],
                                         AF.Identity,
                                         bias=wsb["c7v"][:, ob : ob + 1])

                h7a = fin.tile([128, 4 * H], bf, tag="h7a", name="h7a")
                for ob in range(4):
                    for jp in range(4):
                        sl = slice(ob * H + jp * 512, ob * H + (jp + 1) * 512)
                        nc.scalar.activation(
                            h7a[:, sl], h7[:, sl], AF.Prelu,
                            bias=b7_sb[:, ob : ob + 1], alpha=0.2,
                        )
                h8 = fin.tile([128, 2 * H], bf, tag="h8", name="h8")
                for ob in range(2):
                    for jp in range(2):
                        pf = psd.tile([128, 1024], f32, tag="d", name="pf8")
                        for q in range(2):
                            po = slice(q * 512, (q + 1) * 512)
                            co = jp * 1024 + q * 512
                            for kb in range(4):
                                nc.tensor.matmul(
                                    pf[:, po],
                                    wsb["w8T"][:, kb, ob * 128 : (ob + 1) * 128],
                                    h7a[:, kb * H + co : kb * H + co + 512],
                                    start=(kb == 0),
                                    stop=(kb == 3),
                                )
                        nc.scalar.activation(
                            h8[:, ob * H + jp * 1024 : ob * H + (jp + 1) * 1024],
                            pf, AF.Prelu, bias=wsb["c8v"][:, ob : ob + 1],
                            alpha=0.2,
                        )
                o_sb = fin.tile([13, H], f32, tag="osb", name="o_sb")
                for jp in range(2):
                    pf = psd.tile([128, 1024], f32, tag="d", name="pf9")
                    for q in range(2):
                        po = slice(q * 512, (q + 1) * 512)
                        co = jp * 1024 + q * 512
                        for kb in range(2):
                            nc.tensor.matmul(
                                pf[0:13, po],
                                wsb["w9T"][:, kb, :],
                                h8[:, kb * H + co : kb * H + co + 512],
                                start=(kb == 0),
                                stop=(kb == 1),
                            )
                    nc.scalar.activation(
                        o_sb[:, jp * 1024 : (jp + 1) * 1024], pf[0:13],
                        AF.Identity, bias=wsb["b9v"],
                    )
                nc.sync.dma_start(out_d[:], o_sb)

    nc.compile()
    return nc


def _mix(a_steps, b_steps, lead=0):
    """Emit `lead` a-steps, then alternate a/b 1:1 until b is exhausted, then
    the remaining a-steps."""
    ia = 0
    for _ in range(min(lead, len(a_steps))):
        a_steps[ia]()
        ia += 1
    for ib in range(len(b_steps)):
        if ia < len(a_steps):
            a_steps[ia]()
            ia += 1
        b_steps[ib]()
    while ia < len(a_steps):
        a_steps[ia]()
        ia += 1


def make_in_maps(inputs):
    """Per-core input dicts from the full problem inputs."""
    wd = _prep_weights(inputs)
    wpack32 = np.zeros((128, _wcols(0)), dtype=np.float32)
    import ml_dtypes

    wpack16 = np.zeros((128, _wcols(1)), dtype=ml_dtypes.bfloat16)
    col = {0: 0, 1: 0}
    for name, shape, isb in _WEIGHT_SPECS:
        w = int(np.prod(shape[1:])) if len(shape) > 1 else 1
        dst = wpack16 if isb else wpack32
        v = wd[name].reshape(shape[0], w)
        dst[0 : shape[0], col[isb] : col[isb] + w] = (
            _bf16(v) if isb else v.astype(np.float32)
        )
        col[isb] += w
    pts = np.asarray(inputs["points"], dtype=np.float32)
    in_maps = []
    for c in range(8):
        b, h = c // 2, c % 2
        m = {"wpack32": wpack32, "wpack16": wpack16}
        m["wboot"] = np.ascontiguousarray(
            np.concatenate([wd["u1T"], wd["ones3"]], axis=1).astype(np.float32))
        m["choff"] = np.ascontiguousarray(
            np.tile(np.repeat(np.arange(8, dtype=np.uint16) * 512, 8), (128, 1)))
        m["pts_full"] = np.ascontiguousarray(pts[b])
        pm = np.ascontiguousarray(pts[b][:, h * H : (h + 1) * H])
        m["pts_mine"] = pm
        m["pts_mine_bf"] = _bf16(pm)
        in_maps.append(m)
    return in_maps


def kernel(**inputs):
    from concourse.bass_utils import run_bass_kernel_spmd

    if "nc" not in _CACHE:
        _CACHE["nc"] = build_kernel()
    nc = _CACHE["nc"]
    in_maps = make_in_maps(inputs)
    res = run_bass_kernel_spmd(nc, in_maps, core_ids=list(range(8)))
    out = np.zeros((B, 13, N), dtype=np.float32)
    for c in range(8):
        b, h = c // 2, c % 2
        out[b][:, h * H : (h + 1) * H] = res.results[c]["out"]
    return out


# revision 52
# speedup vs baseline: 1.7440x; 1.0073x over previous
"""DGCNN part-segmentation forward pass on 8 Trainium2 NeuronCores.

Sharding: data-parallel over the batch (B=4) x 2-way split of the N=4096
points within each batch element.  Core c handles batch element c//2,
point rows [(c%2)*2048, (c%2+1)*2048).  The two cores of a pair exchange
their half of each EdgeConv output with pairwise AllGathers (one per
1024-point group, launched as soon as that group's output is ready) and
a pairwise AllReduce-max for the global pooling feature.

Precision: the feature path (EdgeConv transforms, convs 6-9) runs in
bfloat16 with fp32 PSUM accumulation.  Layer 1's kNN runs on raw fp32
points (3-D points have many near-ties; bf16 inputs measurably flip
neighbors), via a small separate fp32 lhs/rhs pair.  Layers 2-3 rank
neighbors of the bf16 features exactly: the distance matmul contracts
66 bf16 rows = 64 feature channels + a hi/lo bf16 pair carrying the
fp32 -|x_j|^2 row at ~16-bit precision.  The sq rows travel inside the
AllGather so no distance prep remains on the post-exchange critical
path.

The emission order is software-pipelined: engine queues are in-order,
so each phase leads with a few distance tiles, then runs the edge-conv
phase solid (its AllGather fires early), then the bulk of the
distance/top-k scans; the next layer's first distance quarters overlap
the current layer's second edge phase.

Per EdgeConv layer:
  top-20: per 128-row tile: 8x top-8 per 512-column chunk (DVE Max) ->
          64 candidates; 3 peeling rounds (max8 + match_replace) ->
          top-24 values; threshold mask + prefix-sum compaction +
          gpsimd local_scatter -> global indices of the top-20
  gather: batched GPSIMD ap_gather of the bf16 u = U x transform; the
          first call covers one neighbor slot so the edge pipeline
          starts right after the peels, later calls cover 4 slots
  edge  : psum = I*u_gathered + V*x_i (bf16 matmuls), LeakyReLU (Prelu)
          on the scalar engine, conv2 bf16 matmul; the k-max runs on
          the raw conv2 psums (LeakyReLU is monotone) and one Prelu
          finalizes each group.
"""

import sys

sys.path.insert(0, "/opt/trn_rl_repo")

import numpy as np

B = 4
N = 4096
H = 2048  # points per core (half of a batch element)
KNN = 20
EPS = 1e-5
NEG = -3.0e38
G = KNN * 64  # widx columns per point group

_CACHE = {}


# --------------------------------------------------------------------------
# host-side weight preparation
# --------------------------------------------------------------------------

def _bf16(x):
    import ml_dtypes

    return np.ascontiguousarray(x, dtype=np.float32).astype(ml_dtypes.bfloat16)


def _fold_bn(w, b, g, be):
    s = (g / np.sqrt(np.float32(1.0) + np.float32(EPS))).astype(np.float32)
    return (w * s[:, None]).astype(np.float32), (s * b + be).astype(np.float32)


def _prep_weights(inp):
    f = np.float32
    W, C = {}, {}
    for i in range(1, 9):
        W[i], C[i] = _fold_bn(
            inp["w%d" % i], inp["b%d" % i], inp["g%d" % i], inp["be%d" % i]
        )
    d = {}
    # edge conv layers: split into U (neighbor part) and V (center part).
    # Layer 1's x2r tile holds raw points, layers 2-3 hold 2*x, so the V
    # transpose is halved only for layers 2-3.
    for lid, wi, vscale in ((1, 1, 1.0), (2, 3, 0.5), (3, 5, 0.5)):
        w = W[wi]
        cin = w.shape[1] // 2
        U = w[:, :cin]
        V = w[:, cin:] - w[:, :cin]
        d["u%dT" % lid] = np.ascontiguousarray(U.T)
        d["v%dTh" % lid] = np.ascontiguousarray((V * f(vscale)).T)
        d["c%d" % lid] = C[wi].reshape(64, 1)
    d["w2T"] = np.ascontiguousarray(W[2].T)
    d["cc2"] = C[2].reshape(64, 1)
    d["w4T"] = np.ascontiguousarray(W[4].T)
    d["cc4"] = C[4].reshape(64, 1)
    # conv6 (192 -> 1024)
    w6T = np.ascontiguousarray(W[6].T)  # (192, 1024)
    d["w6aT"] = np.ascontiguousarray(w6T[:128])
    d["w6bT"] = np.ascontiguousarray(w6T[128:])
    d["c6v"] = np.ascontiguousarray(C[6].reshape(8, 128).T)  # (128, 8)
    # conv7 (1216 -> 512): xg part (1024) and local part (192)
    w7 = W[7]
    w7gT = np.ascontiguousarray(w7[:, :1024].T)  # (1024, 512)
    d["w7gT"] = np.ascontiguousarray(w7gT.reshape(8, 128, 512).transpose(1, 0, 2))
    w7lT = np.ascontiguousarray(w7[:, 1024:].T)  # (192, 512)
    d["w7laT"] = np.ascontiguousarray(w7lT[:128])
    d["w7lbT"] = np.ascontiguousarray(w7lT[128:])
    d["c7v"] = np.ascontiguousarray(C[7].reshape(4, 128).T)  # (128, 4)
    # conv8 (512 -> 256)
    w8T = np.ascontiguousarray(W[8].T)  # (512, 256)
    d["w8T"] = np.ascontiguousarray(w8T.reshape(4, 128, 256).transpose(1, 0, 2))
    d["c8v"] = np.ascontiguousarray(C[8].reshape(2, 128).T)  # (128, 2)
    # conv9 (256 -> 13), plain linear
    w9T = np.ascontiguousarray(inp["w9"].T.astype(f))  # (256, 13)
    d["w9T"] = np.ascontiguousarray(w9T.reshape(2, 128, 13).transpose(1, 0, 2))
    d["b9v"] = inp["b9"].astype(f).reshape(13, 1)
    # constants
    d["id64"] = np.eye(64, dtype=f)
    d["ones3"] = np.ones((3, 1), dtype=f)
    d["ones64"] = np.ones((64, 1), dtype=f)
    return d


# name -> (shape, is_bf16)
_WEIGHT_SPECS = [
    ("u1T", (3, 64), 0), ("v1Th", (3, 64), 1), ("c1", (64, 1), 0),
    ("w2T", (64, 64), 1), ("cc2", (64, 1), 0),
    ("u2T", (64, 64), 1), ("v2Th", (64, 64), 1), ("c2", (64, 1), 0),
    ("w4T", (64, 64), 1), ("cc4", (64, 1), 0),
    ("u3T", (64, 64), 1), ("v3Th", (64, 64), 1), ("c3", (64, 1), 0),
    ("w6aT", (128, 1024), 1), ("w6bT", (64, 1024), 1), ("c6v", (128, 8), 0),
    ("w7gT", (128, 8, 512), 0), ("w7laT", (128, 512), 1),
    ("w7lbT", (64, 512), 1), ("c7v", (128, 4), 0),
    ("w8T", (128, 4, 256), 1), ("c8v", (128, 2), 0),
    ("w9T", (128, 2, 13), 1), ("b9v", (13, 1), 0),
    ("id64", (64, 64), 0), ("ones3", (3, 1), 0), ("ones64", (64, 1), 0),
]


def _wcols(bf):
    return sum(int(np.prod(s[1:])) if len(s) > 1 else 1
               for _, s, isb in _WEIGHT_SPECS if isb == bf)


# --------------------------------------------------------------------------
# device kernel builder
# --------------------------------------------------------------------------

def build_kernel():
    import concourse.bacc as bacc
    import concourse.mybir as mybir
    import concourse.tile as tile

    f32 = mybir.dt.float32
    bf = mybir.dt.bfloat16
    i16 = mybir.dt.int16
    u16 = mybir.dt.uint16
    AF = mybir.ActivationFunctionType
    ALU = mybir.AluOpType
    PAIRS = [[0, 1], [2, 3], [4, 5], [6, 7]]

    nc = bacc.Bacc("TRN2", target_bir_lowering=False, num_devices=8)

    pts_full = nc.dram_tensor("pts_full", [3, N], f32, kind="ExternalInput")
    pts_mine = nc.dram_tensor("pts_mine", [3, H], f32, kind="ExternalInput")
    pts_mine_bf = nc.dram_tensor("pts_mine_bf", [3, H], bf,
                                 kind="ExternalInput")
    choff_d = nc.dram_tensor("choff", [128, 64], u16, kind="ExternalInput")
    wboot_d = nc.dram_tensor("wboot", [3, 65], f32, kind="ExternalInput")
    wpack32_d = nc.dram_tensor("wpack32", [128, _wcols(0)], f32,
                               kind="ExternalInput")
    wpack16_d = nc.dram_tensor("wpack16", [128, _wcols(1)], bf,
                               kind="ExternalInput")
    out_d = nc.dram_tensor("out", [13, H], f32, kind="ExternalOutput")

    with tile.TileContext(nc) as tc:
        with (
            tc.tile_pool(name="wp", bufs=1) as wp,
            tc.tile_pool(name="per", bufs=1) as per,
            tc.tile_pool(name="psd", bufs=2, space="PSUM") as psd,
            tc.tile_pool(name="pse", bufs=2, space="PSUM") as pse,
            tc.tile_pool(name="dram", bufs=1, space="DRAM") as drp,
        ):
            # ---- persistent tiles ----
            rhsF = per.tile([66, N], bf, tag="rhsF", name="rhsF")
            rhsF1 = per.tile([4, N], f32, tag="rhsF1", name="rhsF1")
            lhsTm = per.tile([66, H], bf, tag="lhsTm", name="lhsTm")
            lhsTm1 = per.tile([4, H], f32, tag="lhsTm1", name="lhsTm1")
            u_t = per.tile([64, N], f32, tag="u", name="u_t")
            x2r = per.tile([64, H], bf, tag="x2r", name="x2r")
            xcat_a = per.tile([128, H], bf, tag="xcata", name="xcat_a")
            xcat_b = per.tile([64, H], bf, tag="xcatb", name="xcat_b")
            widx = per.tile([64, KNN * 128], i16, tag="widx", name="widx")
            acc3 = per.tile([64, H], f32, tag="acc3", name="acc3")
            xg_sb = per.tile([128, 8], f32, tag="xg", name="xg_sb")
            b7_sb = per.tile([128, 4], f32, tag="b7", name="b7_sb")
            choff_sb = per.tile([128, 64], u16, tag="choff", name="choff_sb")
            zero128 = per.tile([128, 64], f32, tag="z128", name="zero128")

            idx_dram = drp.tile([H, KNN], i16, tag="idxd", name="idx_dram")

            x1h = xcat_a[0:64]
            x2h = xcat_a[64:128]
            x3h = xcat_b

            # critical inputs first; the row-3 ones of lhsTm1 come from a
            # 32-aligned memset that the pts DMA then partially overwrites
            wbt = wp.tile([3, 65], f32, tag="wb", name="wboot_sb")
            nc.sync.dma_start(wbt, wboot_d[:])
            nc.vector.memset(lhsTm1[0:4], 1.0)
            nc.sync.dma_start(lhsTm1[0:3], pts_mine[:])
            nc.sync.dma_start(x2r[0:3], pts_mine_bf[:])
            nc.sync.dma_start(choff_sb, choff_d[:])
            nc.vector.memset(zero128, 0.0)
            nc.vector.memset(lhsTm[64:66], 1.0)
            # all weights arrive in two packed DMAs; per-weight tiles are
            # views into the packed tiles
            wpt32 = wp.tile([128, _wcols(0)], f32, tag="wp32", name="wp32")
            nc.scalar.dma_start(wpt32, wpack32_d[:])
            wpt16 = wp.tile([128, _wcols(1)], bf, tag="wp16", name="wp16")
            nc.scalar.dma_start(wpt16, wpack16_d[:])
            wsb = {}
            col = {0: 0, 1: 0}
            for name, shape, isb in _WEIGHT_SPECS:
                w = int(np.prod(shape[1:])) if len(shape) > 1 else 1
                src = wpt16 if isb else wpt32
                v = src[0 : shape[0], col[isb] : col[isb] + w]
                if len(shape) == 3:
                    v = v.rearrange("p (a b) -> p a b", a=shape[1])
                wsb[name] = v
                col[isb] += w
            wsb["u1T"] = wbt[0:3, 0:64]
            wsb["ones3"] = wbt[0:3, 64:65]

            LAYER = {
                1: dict(cin=3, uT="u1T", vTh="v1Th", c="c1", w2="w2T",
                        cc="cc2"),
                2: dict(cin=64, uT="u2T", vTh="v2Th", c="c2", w2="w4T",
                        cc="cc4"),
                3: dict(cin=64, uT="u3T", vTh="v3Th", c="c3", w2=None,
                        cc=None),
            }
            XOUT = {1: x1h, 2: x2h, 3: x3h}

            with (
                tc.tile_pool(name="dsb", bufs=4) as dsbp,
                tc.tile_pool(name="pp", bufs=2) as ppl,
                tc.tile_pool(name="tk", bufs=2) as tkp,
                tc.tile_pool(name="gp", bufs=2) as gp,
                tc.tile_pool(name="yp", bufs=3) as ypl,
                tc.tile_pool(name="za", bufs=1) as zap,
            ):
                cands = {}

                def prep_block1(lo):
                    """layer-1 lead: rhsF1 = [2*pts; -|pts|^2] and
                    u_t = u1T.T @ pts for columns [lo, lo+1024)."""
                    stage = ppl.tile([3, 1024], f32, tag="pts", name="stage")
                    nc.sync.dma_start(stage, pts_full[:, lo : lo + 1024])
                    nc.scalar.mul(rhsF1[0:3, lo : lo + 1024], stage, 2.0)
                    xsq = ppl.tile([64, 1024], f32, tag="xsq", name="xsq1")
                    nc.scalar.activation(xsq[0:3], stage, AF.Square)
                    sqrow = ppl.tile([1, 1024], f32, tag="sqr", name="sqrow1")
                    for j in range(2):
                        jl = slice(j * 512, (j + 1) * 512)
                        pq = psd.tile([128, 1024], f32, tag="d", name="pq")
                        nc.tensor.matmul(pq[0:1, 0:512], wsb["ones3"],
                                         xsq[0:3, jl])
                        nc.scalar.mul(sqrow[:, jl], pq[0:1, 0:512], -1.0)
                        pu = psd.tile([128, 1024], f32, tag="d", name="pu")
                        nc.tensor.matmul(pu[0:64, 0:512], wsb["u1T"],
                                         stage[:, jl])
                        nc.scalar.copy(u_t[:, lo + j * 512 : lo + (j + 1) * 512],
                                       pu[0:64, 0:512])
                    nc.sync.dma_start(rhsF1[3:4, lo : lo + 1024], sqrow)

                def prep_block(lid, lo):
                    """u_t[:, lo:lo+1024] = uT.T @ rhsF[0:cin] (bf16)."""
                    P = LAYER[lid]
                    cin = P["cin"]
                    for j in range(2):
                        sl = slice(lo + j * 512, lo + (j + 1) * 512)
                        pu = psd.tile([128, 1024], f32, tag="d", name="pu")
                        nc.tensor.matmul(pu[0:64, 0:512], wsb[P["uT"]],
                                         rhsF[0:cin, sl])
                        nc.scalar.copy(u_t[:, sl], pu[0:64, 0:512])

                def dist_quarter(lid, t, blk):
                    """distance psum + copy + top8 scan for tile t, column
                    block blk (1024 cols), ranking for layer lid."""
                    if t not in cands:
                        cands[t] = (
                            tkp.tile([128, 64], f32, tag="cand", name="cand",
                                     bufs=12),
                            tkp.tile([128, 64], u16, tag="cidx", name="cidx",
                                     bufs=12),
                        )
                    cand, cidx = cands[t]
                    lo = blk * 1024
                    pd = psd.tile([128, 1024], f32, tag="d", name="pd")
                    for q in range(2):
                        cs = slice(lo + q * 512, lo + (q + 1) * 512)
                        if lid == 1:
                            nc.tensor.matmul(
                                pd[:, q * 512 : (q + 1) * 512],
                                lhsTm1[:, t * 128 : (t + 1) * 128],
                                rhsF1[:, cs],
                            )
                        else:
                            nc.tensor.matmul(
                                pd[:, q * 512 : (q + 1) * 512],
                                lhsTm[:, t * 128 : (t + 1) * 128],
                                rhsF[:, cs],
                            )
                    dsb = dsbp.tile([128, 1024], f32, tag="dsb", name="dsb")
                    nc.scalar.copy(dsb, pd)
                    for q in range(2):
                        cc = blk * 2 + q
                        nc.vector.max(
                            out=cand[:, cc * 8 : (cc + 1) * 8],
                            in_=dsb[:, q * 512 : (q + 1) * 512],
                        )
                        nc.vector.max_index(
                            cidx[:, cc * 8 : (cc + 1) * 8],
                            cand[:, cc * 8 : (cc + 1) * 8],
                            dsb[:, q * 512 : (q + 1) * 512],
                        )

                def peel_compact(t):
                    """top-20 selection for tile t from its 64 candidates."""
                    cand, cidx = cands.pop(t)
                    nc.vector.tensor_add(cidx, cidx, choff_sb)
                    candw = tkp.tile([128, 64], f32, tag="candw", name="candw")
                    nc.vector.tensor_copy(candw, cand)
                    t8 = tkp.tile([128, 24], f32, tag="t8", name="t8")
                    nc.vector.max(out=t8[:, 0:8], in_=candw)
                    nc.vector.match_replace(out=candw, in_to_replace=t8[:, 0:8],
                                            in_values=candw, imm_value=NEG)
                    nc.vector.max(out=t8[:, 8:16], in_=candw)
                    nc.vector.match_replace(out=candw,
                                            in_to_replace=t8[:, 8:16],
                                            in_values=candw, imm_value=NEG)
                    nc.vector.max(out=t8[:, 16:24], in_=candw)
                    mask = tkp.tile([128, 64], f32, tag="mask", name="mask")
                    nc.vector.tensor_scalar(mask, cand, t8[:, 19:20], None,
                                            op0=ALU.is_ge)
                    cums = tkp.tile([128, 64], f32, tag="cums", name="cums")
                    nc.vector.tensor_tensor_scan(cums, mask, zero128, 0.0,
                                                 op0=ALU.add, op1=ALU.add)
                    # slot = cums*mask - 1  (-1 marks non-selected: ignored
                    # by local_scatter)
                    nc.vector.tensor_mul(cums, cums, mask)
                    nc.vector.tensor_scalar_add(cums, cums, -1.0)
                    slot = tkp.tile([128, 64], i16, tag="slot", name="slot")
                    nc.vector.tensor_copy(slot, cums)
                    sel = tkp.tile([128, 24], u16, tag="sel", name="sel")
                    nc.gpsimd.local_scatter(out_ap=sel, data_ap=cidx,
                                            idxs_ap=slot, channels=128,
                                            num_elems=24, num_idxs=64)
                    nc.sync.dma_start(
                        idx_dram[t * 128 : (t + 1) * 128, :],
                        sel[:, 0:KNN].bitcast(i16),
                    )

                # gather calls: (k0, nk, widx column base within the group).
                # The first call covers a single neighbor slot so the edge
                # pipeline starts as soon as possible after the peels.
                CALLS = [(0, 1, 0), (1, 4, 64), (5, 4, 320), (9, 4, 576),
                         (13, 4, 832), (17, 3, 1088)]

                def reformat(grp):
                    """widx[p, grp*G + base + r*nk + kk]
                         = idx_dram[1024*grp + 16*r + p, k0 + kk],
                    replicated over the four 16-partition groups; one DMA per
                    (gather call, replica) so the first call's indices land
                    first and later transfers overlap the gathers."""
                    for k0, nk, base in CALLS:
                        src = idx_dram[grp * 1024 : (grp + 1) * 1024,
                                       k0 : k0 + nk].rearrange(
                            "(r p) k -> p r k", p=16
                        )
                        for rep in range(4):
                            dst = widx[rep * 16 : (rep + 1) * 16,
                                       grp * G + base
                                       : grp * G + base + 64 * nk].rearrange(
                                "p (r k) -> p r k", k=nk
                            )
                            nc.sync.dma_start(dst, src)

                def gather_call(grp, call):
                    k0, nk, base = CALLS[call]
                    g = gp.tile([64, 4096], f32, tag="g", name="g")
                    nc.gpsimd.ap_gather(
                        out_ap=g[:, 0 : 1024 * nk],
                        in_ap=u_t,
                        idxs_ap=widx[:, grp * G + base
                                     : grp * G + base + 64 * nk],
                        channels=64,
                        num_elems=N,
                        d=1,
                        num_idxs=1024 * nk,
                    )
                    # columns are (r, kk)-interleaved: col = nk*16*r + 16*kk + p
                    return g[:, 0 : 1024 * nk].rearrange(
                        "c (r f p) -> c f r p", f=nk, p=16
                    )

                def edge_k(lid, grp, gv, kk, k, zacc):
                    """one neighbor slot: y = Prelu(I u_j + V x_i + c);
                    z psum = W2 y; zacc = max(zacc, z).  Layer 3 needs only
                    the running max of the gathered u."""
                    P = LAYER[lid]
                    cin = P["cin"]
                    gk = gv[:, kk]  # (64, 64, 16): r-major, p-minor
                    if lid == 3:
                        gsl = slice(grp * 1024, (grp + 1) * 1024)
                        a3 = acc3[:, gsl].rearrange("c (r p) -> c r p", p=16)
                        if k == 0:
                            nc.vector.tensor_copy(a3, gk)
                        else:
                            nc.vector.tensor_max(a3, a3, gk)
                        return
                    for q in range(2):
                        rsl = slice(q * 32, (q + 1) * 32)
                        csl = slice(q * 512, (q + 1) * 512)
                        msl = slice(grp * 1024 + q * 512,
                                    grp * 1024 + (q + 1) * 512)
                        yp = pse.tile([64, 512], f32, tag="e", name="yp")
                        nc.tensor.matmul(yp, wsb["id64"], gk[:, rsl],
                                         start=True, stop=False)
                        nc.tensor.matmul(yp, wsb[P["vTh"]], x2r[0:cin, msl],
                                         start=False, stop=True)
                        y = ypl.tile([64, 512], bf, tag="y", name="y")
                        nc.scalar.activation(y, yp, AF.Prelu, bias=wsb[P["c"]],
                                             alpha=0.2)
                        zp = pse.tile([64, 512], f32, tag="c2", name="zp")
                        nc.tensor.matmul(zp, wsb[P["w2"]], y)
                        if k == 0:
                            nc.vector.tensor_copy(zacc[:, csl], zp)
                        else:
                            nc.vector.tensor_max(zacc[:, csl], zacc[:, csl],
                                                 zp)

                def edge_fin(lid, grp, zacc):
                    P = LAYER[lid]
                    gsl = slice(grp * 1024, (grp + 1) * 1024)
                    x_out = XOUT[lid]
                    if lid == 3:
                        for q in range(2):
                            sl = slice(grp * 1024 + q * 512,
                                       grp * 1024 + (q + 1) * 512)
                            pe_ = pse.tile([64, 512], f32, tag="e", name="pe3")
                            nc.tensor.matmul(pe_, wsb["id64"], acc3[:, sl],
                                             start=True, stop=False)
                            nc.tensor.matmul(pe_, wsb[P["vTh"]],
                                             x2r[0 : P["cin"], sl],
                                             start=False, stop=True)
                            nc.scalar.activation(x_out[:, sl], pe_, AF.Prelu,
                                                 bias=wsb[P["c"]], alpha=0.2)
                    else:
                        nc.scalar.activation(x_out[:, gsl], zacc, AF.Prelu,
                                             bias=wsb[P["cc"]], alpha=0.2)

                ccs = {}

                def exchange_start(lid, grp):
                    """export x_out group plus the hi/lo bf16 pair carrying
                    its fp32 -|x|^2 row, and launch the pairwise AllGather."""
                    gsl = slice(grp * 1024, (grp + 1) * 1024)
                    x_half = XOUT[lid]
                    xsq = ppl.tile([64, 1024], f32, tag="xsq", name="xsqe")
                    nc.scalar.activation(xsq, x_half[:, gsl], AF.Square)
                    sqrow = ppl.tile([1, 1024], f32, tag="sqr", name="sqrowe")
                    for j in range(2):
                        jl = slice(j * 512, (j + 1) * 512)
                        pq = psd.tile([128, 1024], f32, tag="d", name="pqe")
                        nc.tensor.matmul(pq[0:1, 0:512], wsb["ones64"],
                                         xsq[:, jl])
                        nc.scalar.mul(sqrow[:, jl], pq[0:1, 0:512], -1.0)
                    hi = ppl.tile([1, 1024], bf, tag="hi", name="hi")
                    nc.scalar.copy(hi, sqrow)
                    lo = ppl.tile([1, 1024], bf, tag="lo", name="lo")
                    nc.vector.tensor_tensor(lo, sqrow, hi, op=ALU.subtract)
                    ccin = drp.tile([66, 1024], bf, tag="cci%d" % grp,
                                    name="ccin%d" % grp)
                    nc.scalar.dma_start(ccin[0:64], x_half[:, gsl])
                    nc.scalar.dma_start(ccin[64:65], hi)
                    nc.scalar.dma_start(ccin[65:66], lo)
                    ccout = drp.tile([132, 1024], bf, tag="cco%d" % grp,
                                     name="ccout%d" % grp)
                    nc.gpsimd.collective_compute(
                        "AllGather", ALU.bypass, replica_groups=PAIRS,
                        ins=[ccin], outs=[ccout],
                    )
                    return ccout

                def exchange_finish(ccout, grp):
                    lo0 = grp * 1024
                    lo1 = 2048 + grp * 1024
                    nc.sync.dma_start(rhsF[0:64, lo0 : lo0 + 1024], ccout[0:64])
                    nc.sync.dma_start(rhsF[64:66, lo0 : lo0 + 1024],
                                      ccout[64:66])
                    nc.sync.dma_start(rhsF[0:64, lo1 : lo1 + 1024],
                                      ccout[66:130])
                    nc.sync.dma_start(rhsF[64:66, lo1 : lo1 + 1024],
                                      ccout[130:132])

                def lhs_prep(lid, grp):
                    """lhsTm/x2r columns for group grp from layer lid's out."""
                    gsl = slice(grp * 1024, (grp + 1) * 1024)
                    x_out = XOUT[lid]
                    nc.scalar.mul(lhsTm[0:64, gsl], x_out[:, gsl], 2.0)
                    nc.scalar.mul(x2r[0:64, gsl], x_out[:, gsl], 2.0)

                def edge_steps(lid, grp):
                    """emission steps for the edge phase of (lid, grp); the
                    final step also launches the AllGather for layers 1-2."""
                    steps = []
                    state = {}
                    zacc = None
                    if lid != 3:
                        zacc = zap.tile([64, 1024], f32, tag="za", name="zacc")

                    def mk_gather(call):
                        def f():
                            state[call] = gather_call(grp, call)
                        return f

                    def mk_k(call, kk, k):
                        def f():
                            edge_k(lid, grp, state[call], kk, k, zacc)
                        return f

                    def fin():
                        edge_fin(lid, grp, zacc)
                        if lid < 3:
                            ccs[grp] = exchange_start(lid, grp)
                            lhs_prep(lid, grp)

                    for call, (k0, nk, base) in enumerate(CALLS):
                        steps.append(mk_gather(call))
                        for kk in range(nk):
                            steps.append(mk_k(call, kk, k0 + kk))
                    steps.append(fin)
                    return steps

                carry = []

                def phase_B(lid):
                    """grp0 edge (leading, so its AllGather fires early)
                    followed by grp1 tiles' dist/topk."""
                    a = []
                    for t in range(8, 16):
                        for blk in range(4):
                            a.append(lambda t=t, blk=blk:
                                     dist_quarter(lid, t, blk))
                        a.append(lambda t=t: peel_compact(t))
                    for f in a[:4]:
                        f()
                    for f in edge_steps(lid, 0):
                        f()
                    for f in a[4:]:
                        f()
                    reformat(1)

                def phase_C(lid):
                    """grp1 edge followed by the next layer's lead work (or
                    conv6's first half for layer 3)."""
                    nxt = lid + 1 if lid < 3 else None
                    if nxt is not None:
                        exchange_finish(ccs[0], 0)
                    es = edge_steps(lid, 1)
                    if nxt is None:
                        _mix([lambda ob=ob: conv6_chunk(ob, 0)
                              for ob in range(8)], es)
                        return
                    for f in es:
                        f()
                    prep_block(nxt, 0)
                    prep_block(nxt, 2048)
                    for t in range(8):
                        dist_quarter(nxt, t, 0)
                    for t in range(3):
                        dist_quarter(nxt, t, 2)
                    for t in range(3, 8):
                        carry.append(lambda t=t: dist_quarter(nxt, t, 2))

                def phase_A(lid):
                    """finish the leftover and second-half dist quarters of
                    tiles 0..8 for layer lid, then peel."""
                    exchange_finish(ccs[1], 1)
                    for f in carry:
                        f()
                    carry.clear()
                    prep_block(lid, 1024)
                    prep_block(lid, 3072)
                    for t in range(8):
                        dist_quarter(lid, t, 1)
                        dist_quarter(lid, t, 3)
                    for t in range(8):
                        peel_compact(t)
                    reformat(0)

                def conv6_chunk(ob, jp):
                    obs = slice(ob * 128, (ob + 1) * 128)
                    pf = psd.tile([128, 1024], f32, tag="d", name="pf6")
                    for q in range(2):
                        sl = slice(jp * 1024 + q * 512,
                                   jp * 1024 + (q + 1) * 512)
                        po = slice(q * 512, (q + 1) * 512)
                        nc.tensor.matmul(pf[:, po], wsb["w6aT"][:, obs],
                                         xcat_a[:, sl], start=True, stop=False)
                        nc.tensor.matmul(pf[:, po], wsb["w6bT"][:, obs],
                                         xcat_b[:, sl], start=False, stop=True)
                    h6 = gp.tile([128, 1024], f32, tag="h6", name="h6", bufs=2)
                    nc.scalar.activation(h6, pf, AF.Prelu,
                                         bias=wsb["c6v"][:, ob : ob + 1],
                                         alpha=0.2)
                    xgt = xgts[ob]
                    nc.vector.reduce_max(xgt[:, jp : jp + 1], h6,
                                         axis=mybir.AxisListType.X)

                xgts = [tkp.tile([128, 2], f32, tag="xgt%d" % ob,
                                 name="xgt%d" % ob) for ob in range(8)]

                # ================= layer 1 lead-in =================
                for blk in range(4):
                    prep_block1(blk * 1024)
                    for t in range(8):
                        dist_quarter(1, t, blk)
                for t in range(8):
                    peel_compact(t)
                reformat(0)

                # ================= the three layers =================
                phase_B(1)
                phase_C(1)
                phase_A(2)
                phase_B(2)
                phase_C(2)
                phase_A(3)
                phase_B(3)
                phase_C(3)

                # ============ conv6 second half + global max pool ============
                for ob in range(8):
                    conv6_chunk(ob, 1)
                    nc.vector.reduce_max(xg_sb[:, ob : ob + 1], xgts[ob],
                                         axis=mybir.AxisListType.X)
                ccg_i = drp.tile([128, 8], f32, tag="cgi", name="ccg_in")
                nc.scalar.dma_start(ccg_i, xg_sb)
                ccg_o = drp.tile([256, 8], f32, tag="cgo", name="ccg_out")
                nc.gpsimd.collective_compute(
                    "AllGather", ALU.bypass, replica_groups=PAIRS,
                    ins=[ccg_i], outs=[ccg_o],
                )

            # layer scratch pools released here; final stage below.
            with tc.tile_pool(name="fin", bufs=1) as fin:
                # conv7 local part without bias (overlaps the AllReduce)
                h7 = fin.tile([128, 4 * H], bf, tag="h7", name="h7")
                for ob in range(4):
                    obs = slice(ob * 128, (ob + 1) * 128)
                    for jp in range(2):
                        pf = psd.tile([128, 1024], f32, tag="d", name="pf7")
                        for q in range(2):
                            sl = slice(jp * 1024 + q * 512,
                                       jp * 1024 + (q + 1) * 512)
                            po = slice(q * 512, (q + 1) * 512)
                            nc.tensor.matmul(pf[:, po], wsb["w7laT"][:, obs],
                                             xcat_a[:, sl],
                                             start=True, stop=False)
                            nc.tensor.matmul(pf[:, po], wsb["w7lbT"][:, obs],
                                             xcat_b[:, sl],
                                             start=False, stop=True)
                        nc.scalar.copy(
                            h7[:, ob * H + jp * 1024 : ob * H + (jp + 1) * 1024],
                            pf,
                        )

                # max of the pair's two xg halves (cheaper than AllReduce,
                # which carries a 1.875x cost multiplier)
                nc.sync.dma_start(xg_sb, ccg_o[0:128])
                xgp = fin.tile([128, 8], f32, tag="xgp", name="xgp")
                nc.sync.dma_start(xgp, ccg_o[128:256])
                nc.vector.tensor_max(xg_sb, xg_sb, xgp)

                # conv7 effective bias: c7 + W7g @ xg
                for ob in range(4):
                    pb = psd.tile([128, 1024], f32, tag="d", name="pb7")
                    for kb in range(8):
                        nc.tensor.matmul(
                            pb[:, 0:1],
                            wsb["w7gT"][:, kb, ob * 128 : (ob + 1) * 128],
                            xg_sb[:, kb : kb + 1],
                            start=(kb == 0),
                            stop=(kb == 7),
                        )
                    nc.scalar.activation(b7_sb[:, ob : ob + 1], pb[:, 0:1],
                                         AF.Identity,
                                         bias=wsb["c7v"][:, ob : ob + 1])

                h7a = fin.tile([128, 4 * H], bf, tag="h7a", name="h7a")
                for jp in range(4):
                    for ob in range(4):
                        sl = slice(ob * H + jp * 512, ob * H + (jp + 1) * 512)
                        nc.scalar.activation(
                            h7a[:, sl], h7[:, sl], AF.Prelu,
                            bias=b7_sb[:, ob : ob + 1], alpha=0.2,
                        )
                h8 = fin.tile([128, 2 * H], bf, tag="h8", name="h8")
                o_sb = fin.tile([13, H], f32, tag="osb", name="o_sb")
                for jp in range(2):
                    for ob in range(2):
                        pf = psd.tile([128, 1024], f32, tag="d", name="pf8")
                        for q in range(2):
                            po = slice(q * 512, (q + 1) * 512)
                            co = jp * 1024 + q * 512
                            for kb in range(4):
                                nc.tensor.matmul(
                                    pf[:, po],
                                    wsb["w8T"][:, kb, ob * 128 : (ob + 1) * 128],
                                    h7a[:, kb * H + co : kb * H + co + 512],
                                    start=(kb == 0),
                                    stop=(kb == 3),
                                )
                        nc.scalar.activation(
                            h8[:, ob * H + jp * 1024 : ob * H + (jp + 1) * 1024],
                            pf, AF.Prelu, bias=wsb["c8v"][:, ob : ob + 1],
                            alpha=0.2,
                        )
                    pf = psd.tile([128, 1024], f32, tag="d", name="pf9")
                    for q in range(2):
                        po = slice(q * 512, (q + 1) * 512)
                        co = jp * 1024 + q * 512
                        for kb in range(2):
                            nc.tensor.matmul(
                                pf[0:13, po],
                                wsb["w9T"][:, kb, :],
                                h8[:, kb * H + co : kb * H + co + 512],
                                start=(kb == 0),
                                stop=(kb == 1),
                            )
                    nc.scalar.activation(
                        o_sb[:, jp * 1024 : (jp + 1) * 1024], pf[0:13],
                        AF.Identity, bias=wsb["b9v"],
                    )
                nc.sync.dma_start(out_d[:], o_sb)

    nc.compile()
    return nc


def _mix(a_steps, b_steps, lead=0):
    """Emit `lead` a-steps, then alternate a/b 1:1 until b is exhausted, then
    the remaining a-steps."""
    ia = 0
    for _ in range(min(lead, len(a_steps))):
        a_steps[ia]()
        ia += 1
    for ib in range(len(b_steps)):
        if ia < len(a_steps):
            a_steps[ia]()
            ia += 1
        b_steps[ib]()
    while ia < len(a_steps):
        a_steps[ia]()
        ia += 1


def make_in_maps(inputs):
    """Per-core input dicts from the full problem inputs."""
    wd = _prep_weights(inputs)
    wpack32 = np.zeros((128, _wcols(0)), dtype=np.float32)
    import ml_dtypes

    wpack16 = np.zeros((128, _wcols(1)), dtype=ml_dtypes.bfloat16)
    col = {0: 0, 1: 0}
    for name, shape, isb in _WEIGHT_SPECS:
        w = int(np.prod(shape[1:])) if len(shape) > 1 else 1
        dst = wpack16 if isb else wpack32
        v = wd[name].reshape(shape[0], w)
        dst[0 : shape[0], col[isb] : col[isb] + w] = (
            _bf16(v) if isb else v.astype(np.float32)
        )
        col[isb] += w
    pts = np.asarray(inputs["points"], dtype=np.float32)
    in_maps = []
    for c in range(8):
        b, h = c // 2, c % 2
        m = {"wpack32": wpack32, "wpack16": wpack16}
        m["wboot"] = np.ascontiguousarray(
            np.concatenate([wd["u1T"], wd["ones3"]], axis=1).astype(np.float32))
        m["choff"] = np.ascontiguousarray(
            np.tile(np.repeat(np.arange(8, dtype=np.uint16) * 512, 8), (128, 1)))
        m["pts_full"] = np.ascontiguousarray(pts[b])
        pm = np.ascontiguousarray(pts[b][:, h * H : (h + 1) * H])
        m["pts_mine"] = pm
        m["pts_mine_bf"] = _bf16(pm)
        in_maps.append(m)
    return in_maps


def kernel(**inputs):
    from concourse.bass_utils import run_bass_kernel_spmd

    if "nc" not in _CACHE:
        _CACHE["nc"] = build_kernel()
    nc = _CACHE["nc"]
    in_maps = make_in_maps(inputs)
    res = run_bass_kernel_spmd(nc, in_maps, core_ids=list(range(8)))
    out = np.zeros((B, 13, N), dtype=np.float32)
    for c in range(8):
        b, h = c // 2, c % 2
        out[b][:, h * H : (h + 1) * H] = res.results[c]["out"]
    return out


# revision 53
# speedup vs baseline: 1.7765x; 1.0187x over previous
"""DGCNN part-segmentation forward pass on 8 Trainium2 NeuronCores.

Sharding: data-parallel over the batch (B=4) x 2-way split of the N=4096
points within each batch element.  Core c handles batch element c//2,
point rows [(c%2)*2048, (c%2+1)*2048).  The two cores of a pair exchange
their half of each EdgeConv output with pairwise AllGathers (one per
1024-point group, launched as soon as that group's output is ready) and
a pairwise AllReduce-max for the global pooling feature.

Precision: the feature path (EdgeConv transforms, convs 6-9) runs in
bfloat16 with fp32 PSUM accumulation.  Layer 1's kNN runs on raw fp32
points (3-D points have many near-ties; bf16 inputs measurably flip
neighbors), via a small separate fp32 lhs/rhs pair.  Layers 2-3 rank
neighbors of the bf16 features exactly: the distance matmul contracts
66 bf16 rows = 64 feature channels + a hi/lo bf16 pair carrying the
fp32 -|x_j|^2 row at ~16-bit precision.  The sq rows travel inside the
AllGather so no distance prep remains on the post-exchange critical
path.

The emission order is software-pipelined: engine queues are in-order,
so each phase leads with a few distance tiles, then runs the edge-conv
phase solid (its AllGather fires early), then the bulk of the
distance/top-k scans; the next layer's first distance quarters overlap
the current layer's second edge phase.

Per EdgeConv layer:
  top-20: per 128-row tile: 8x top-8 per 512-column chunk (DVE Max) ->
          64 candidates; 3 peeling rounds (max8 + match_replace) ->
          top-24 values; threshold mask + prefix-sum compaction +
          gpsimd local_scatter -> global indices of the top-20
  gather: batched GPSIMD ap_gather of the bf16 u = U x transform; the
          first call covers one neighbor slot so the edge pipeline
          starts right after the peels, later calls cover 4 slots
  edge  : psum = I*u_gathered + V*x_i (bf16 matmuls), LeakyReLU (Prelu)
          on the scalar engine, conv2 bf16 matmul; the k-max runs on
          the raw conv2 psums (LeakyReLU is monotone) and one Prelu
          finalizes each group.
"""

import sys

sys.path.insert(0, "/opt/trn_rl_repo")

import numpy as np

B = 4
N = 4096
H = 2048  # points per core (half of a batch element)
KNN = 20
EPS = 1e-5
NEG = -3.0e38
G = KNN * 64  # widx columns per point group

_CACHE = {}


# --------------------------------------------------------------------------
# host-side weight preparation
# --------------------------------------------------------------------------

def _bf16(x):
    import ml_dtypes

    return np.ascontiguousarray(x, dtype=np.float32).astype(ml_dtypes.bfloat16)


def _fold_bn(w, b, g, be):
    s = (g / np.sqrt(np.float32(1.0) + np.float32(EPS))).astype(np.float32)
    return (w * s[:, None]).astype(np.float32), (s * b + be).astype(np.float32)


def _prep_weights(inp):
    f = np.float32
    W, C = {}, {}
    for i in range(1, 9):
        W[i], C[i] = _fold_bn(
            inp["w%d" % i], inp["b%d" % i], inp["g%d" % i], inp["be%d" % i]
        )
    d = {}
    # edge conv layers: split into U (neighbor part) and V (center part).
    # Layer 1's x2r tile holds raw points, layers 2-3 hold 2*x, so the V
    # transpose is halved only for layers 2-3.
    for lid, wi, vscale in ((1, 1, 1.0), (2, 3, 0.5), (3, 5, 0.5)):
        w = W[wi]
        cin = w.shape[1] // 2
        U = w[:, :cin]
        V = w[:, cin:] - w[:, :cin]
        d["u%dT" % lid] = np.ascontiguousarray(U.T)
        d["v%dTh" % lid] = np.ascontiguousarray((V * f(vscale)).T)
        d["c%d" % lid] = C[wi].reshape(64, 1)
    d["w2T"] = np.ascontiguousarray(W[2].T)
    d["cc2"] = C[2].reshape(64, 1)
    d["w4T"] = np.ascontiguousarray(W[4].T)
    d["cc4"] = C[4].reshape(64, 1)
    # conv6 (192 -> 1024)
    w6T = np.ascontiguousarray(W[6].T)  # (192, 1024)
    d["w6aT"] = np.ascontiguousarray(w6T[:128])
    d["w6bT"] = np.ascontiguousarray(w6T[128:])
    d["c6v"] = np.ascontiguousarray(C[6].reshape(8, 128).T)  # (128, 8)
    # conv7 (1216 -> 512): xg part (1024) and local part (192)
    w7 = W[7]
    w7gT = np.ascontiguousarray(w7[:, :1024].T)  # (1024, 512)
    d["w7gT"] = np.ascontiguousarray(w7gT.reshape(8, 128, 512).transpose(1, 0, 2))
    w7lT = np.ascontiguousarray(w7[:, 1024:].T)  # (192, 512)
    d["w7laT"] = np.ascontiguousarray(w7lT[:128])
    d["w7lbT"] = np.ascontiguousarray(w7lT[128:])
    d["c7v"] = np.ascontiguousarray(C[7].reshape(4, 128).T)  # (128, 4)
    # conv8 (512 -> 256)
    w8T = np.ascontiguousarray(W[8].T)  # (512, 256)
    d["w8T"] = np.ascontiguousarray(w8T.reshape(4, 128, 256).transpose(1, 0, 2))
    d["c8v"] = np.ascontiguousarray(C[8].reshape(2, 128).T)  # (128, 2)
    # conv9 (256 -> 13), plain linear
    w9T = np.ascontiguousarray(inp["w9"].T.astype(f))  # (256, 13)
    d["w9T"] = np.ascontiguousarray(w9T.reshape(2, 128, 13).transpose(1, 0, 2))
    d["b9v"] = inp["b9"].astype(f).reshape(13, 1)
    # constants
    d["id64"] = np.eye(64, dtype=f)
    d["ones3"] = np.ones((3, 1), dtype=f)
    d["ones64"] = np.ones((64, 1), dtype=f)
    return d


# name -> (shape, is_bf16)
_WEIGHT_SPECS = [
    ("u1T", (3, 64), 0), ("v1Th", (3, 64), 1), ("c1", (64, 1), 0),
    ("w2T", (64, 64), 1), ("cc2", (64, 1), 0),
    ("u2T", (64, 64), 1), ("v2Th", (64, 64), 1), ("c2", (64, 1), 0),
    ("w4T", (64, 64), 1), ("cc4", (64, 1), 0),
    ("u3T", (64, 64), 1), ("v3Th", (64, 64), 1), ("c3", (64, 1), 0),
    ("w6aT", (128, 1024), 1), ("w6bT", (64, 1024), 1), ("c6v", (128, 8), 0),
    ("w7gT", (128, 8, 512), 0), ("w7laT", (128, 512), 1),
    ("w7lbT", (64, 512), 1), ("c7v", (128, 4), 0),
    ("w8T", (128, 4, 256), 1), ("c8v", (128, 2), 0),
    ("w9T", (128, 2, 13), 1), ("b9v", (13, 1), 0),
    ("id64", (64, 64), 0), ("ones3", (3, 1), 0), ("ones64", (64, 1), 0),
]


def _wcols(bf):
    return sum(int(np.prod(s[1:])) if len(s) > 1 else 1
               for _, s, isb in _WEIGHT_SPECS if isb == bf)


# --------------------------------------------------------------------------
# device kernel builder
# --------------------------------------------------------------------------

def build_kernel():
    import concourse.bacc as bacc
    import concourse.mybir as mybir
    import concourse.tile as tile

    f32 = mybir.dt.float32
    bf = mybir.dt.bfloat16
    i16 = mybir.dt.int16
    u16 = mybir.dt.uint16
    AF = mybir.ActivationFunctionType
    ALU = mybir.AluOpType
    PAIRS = [[0, 1], [2, 3], [4, 5], [6, 7]]

    nc = bacc.Bacc("TRN2", target_bir_lowering=False, num_devices=8)

    pts_full = nc.dram_tensor("pts_full", [3, N], f32, kind="ExternalInput")
    pts_mine = nc.dram_tensor("pts_mine", [3, H], f32, kind="ExternalInput")
    pts_mine_bf = nc.dram_tensor("pts_mine_bf", [3, H], bf,
                                 kind="ExternalInput")
    choff_d = nc.dram_tensor("choff", [128, 64], u16, kind="ExternalInput")
    wboot_d = nc.dram_tensor("wboot", [3, 65], f32, kind="ExternalInput")
    wpack32_d = nc.dram_tensor("wpack32", [128, _wcols(0)], f32,
                               kind="ExternalInput")
    wpack16_d = nc.dram_tensor("wpack16", [128, _wcols(1)], bf,
                               kind="ExternalInput")
    out_d = nc.dram_tensor("out", [13, H], f32, kind="ExternalOutput")

    with tile.TileContext(nc) as tc:
        with (
            tc.tile_pool(name="wp", bufs=1) as wp,
            tc.tile_pool(name="per", bufs=1) as per,
            tc.tile_pool(name="psd", bufs=2, space="PSUM") as psd,
            tc.tile_pool(name="pse", bufs=2, space="PSUM") as pse,
            tc.tile_pool(name="dram", bufs=1, space="DRAM") as drp,
        ):
            # ---- persistent tiles ----
            rhsF = per.tile([66, N], bf, tag="rhsF", name="rhsF")
            rhsF1 = per.tile([4, N], f32, tag="rhsF1", name="rhsF1")
            lhsTm = per.tile([66, H], bf, tag="lhsTm", name="lhsTm")
            lhsTm1 = per.tile([4, H], f32, tag="lhsTm1", name="lhsTm1")
            u_t = per.tile([64, N], f32, tag="u", name="u_t")
            x2r = per.tile([64, H], bf, tag="x2r", name="x2r")
            xcat_a = per.tile([128, H], bf, tag="xcata", name="xcat_a")
            xcat_b = per.tile([64, H], bf, tag="xcatb", name="xcat_b")
            widx = per.tile([64, KNN * 128], i16, tag="widx", name="widx")
            acc3 = per.tile([64, H], f32, tag="acc3", name="acc3")
            xg_sb = per.tile([128, 8], f32, tag="xg", name="xg_sb")
            b7_sb = per.tile([128, 4], f32, tag="b7", name="b7_sb")
            choff_sb = per.tile([128, 64], u16, tag="choff", name="choff_sb")
            zero128 = per.tile([128, 64], f32, tag="z128", name="zero128")

            idx_dram = drp.tile([H, KNN], i16, tag="idxd", name="idx_dram")

            x1h = xcat_a[0:64]
            x2h = xcat_a[64:128]
            x3h = xcat_b

            # critical inputs first; the row-3 ones of lhsTm1 come from a
            # 32-aligned memset that the pts DMA then partially overwrites
            wbt = wp.tile([3, 65], f32, tag="wb", name="wboot_sb")
            nc.sync.dma_start(wbt, wboot_d[:])
            nc.vector.memset(lhsTm1[0:4], 1.0)
            nc.sync.dma_start(lhsTm1[0:3], pts_mine[:])
            nc.sync.dma_start(x2r[0:3], pts_mine_bf[:])
            nc.sync.dma_start(choff_sb, choff_d[:])
            nc.vector.memset(zero128, 0.0)
            nc.vector.memset(lhsTm[64:66], 1.0)
            # all weights arrive in two packed DMAs; per-weight tiles are
            # views into the packed tiles
            wpt32 = wp.tile([128, _wcols(0)], f32, tag="wp32", name="wp32")
            nc.scalar.dma_start(wpt32, wpack32_d[:])
            wpt16 = wp.tile([128, _wcols(1)], bf, tag="wp16", name="wp16")
            nc.scalar.dma_start(wpt16, wpack16_d[:])
            wsb = {}
            col = {0: 0, 1: 0}
            for name, shape, isb in _WEIGHT_SPECS:
                w = int(np.prod(shape[1:])) if len(shape) > 1 else 1
                src = wpt16 if isb else wpt32
                v = src[0 : shape[0], col[isb] : col[isb] + w]
                if len(shape) == 3:
                    v = v.rearrange("p (a b) -> p a b", a=shape[1])
                wsb[name] = v
                col[isb] += w
            wsb["u1T"] = wbt[0:3, 0:64]
            wsb["ones3"] = wbt[0:3, 64:65]

            LAYER = {
                1: dict(cin=3, uT="u1T", vTh="v1Th", c="c1", w2="w2T",
                        cc="cc2"),
                2: dict(cin=64, uT="u2T", vTh="v2Th", c="c2", w2="w4T",
                        cc="cc4"),
                3: dict(cin=64, uT="u3T", vTh="v3Th", c="c3", w2=None,
                        cc=None),
            }
            XOUT = {1: x1h, 2: x2h, 3: x3h}

            with (
                tc.tile_pool(name="dsb", bufs=4) as dsbp,
                tc.tile_pool(name="pp", bufs=2) as ppl,
                tc.tile_pool(name="tk", bufs=2) as tkp,
                tc.tile_pool(name="gp", bufs=2) as gp,
                tc.tile_pool(name="yp", bufs=3) as ypl,
                tc.tile_pool(name="za", bufs=1) as zap,
            ):
                cands = {}

                def prep_block1(lo):
                    """layer-1 lead: rhsF1 = [2*pts; -|pts|^2] and
                    u_t = u1T.T @ pts for columns [lo, lo+1024)."""
                    stage = ppl.tile([3, 1024], f32, tag="pts", name="stage")
                    nc.sync.dma_start(stage, pts_full[:, lo : lo + 1024])
                    nc.scalar.mul(rhsF1[0:3, lo : lo + 1024], stage, 2.0)
                    xsq = ppl.tile([64, 1024], f32, tag="xsq", name="xsq1")
                    nc.scalar.activation(xsq[0:3], stage, AF.Square)
                    sqrow = ppl.tile([1, 1024], f32, tag="sqr", name="sqrow1")
                    for j in range(2):
                        jl = slice(j * 512, (j + 1) * 512)
                        pq = psd.tile([128, 1024], f32, tag="d", name="pq")
                        nc.tensor.matmul(pq[0:1, 0:512], wsb["ones3"],
                                         xsq[0:3, jl])
                        nc.scalar.mul(sqrow[:, jl], pq[0:1, 0:512], -1.0)
                        pu = psd.tile([128, 1024], f32, tag="d", name="pu")
                        nc.tensor.matmul(pu[0:64, 0:512], wsb["u1T"],
                                         stage[:, jl])
                        nc.scalar.copy(u_t[:, lo + j * 512 : lo + (j + 1) * 512],
                                       pu[0:64, 0:512])
                    nc.sync.dma_start(rhsF1[3:4, lo : lo + 1024], sqrow)

                def prep_block(lid, lo):
                    """u_t[:, lo:lo+1024] = uT.T @ rhsF[0:cin] (bf16)."""
                    P = LAYER[lid]
                    cin = P["cin"]
                    for j in range(2):
                        sl = slice(lo + j * 512, lo + (j + 1) * 512)
                        pu = psd.tile([128, 1024], f32, tag="d", name="pu")
                        nc.tensor.matmul(pu[0:64, 0:512], wsb[P["uT"]],
                                         rhsF[0:cin, sl])
                        nc.scalar.copy(u_t[:, sl], pu[0:64, 0:512])

                def dist_quarter(lid, t, blk):
                    """distance psum + copy + top8 scan for tile t, column
                    block blk (1024 cols), ranking for layer lid."""
                    if t not in cands:
                        cands[t] = (
                            tkp.tile([128, 64], f32, tag="cand", name="cand",
                                     bufs=12),
                            tkp.tile([128, 64], u16, tag="cidx", name="cidx",
                                     bufs=12),
                        )
                    cand, cidx = cands[t]
                    lo = blk * 1024
                    pd = psd.tile([128, 1024], f32, tag="d", name="pd")
                    for q in range(2):
                        cs = slice(lo + q * 512, lo + (q + 1) * 512)
                        if lid == 1:
                            nc.tensor.matmul(
                                pd[:, q * 512 : (q + 1) * 512],
                                lhsTm1[:, t * 128 : (t + 1) * 128],
                                rhsF1[:, cs],
                            )
                        else:
                            nc.tensor.matmul(
                                pd[:, q * 512 : (q + 1) * 512],
                                lhsTm[:, t * 128 : (t + 1) * 128],
                                rhsF[:, cs],
                            )
                    dsb = dsbp.tile([128, 1024], f32, tag="dsb", name="dsb")
                    nc.scalar.copy(dsb, pd)
                    for q in range(2):
                        cc = blk * 2 + q
                        nc.vector.max(
                            out=cand[:, cc * 8 : (cc + 1) * 8],
                            in_=dsb[:, q * 512 : (q + 1) * 512],
                        )
                        nc.vector.max_index(
                            cidx[:, cc * 8 : (cc + 1) * 8],
                            cand[:, cc * 8 : (cc + 1) * 8],
                            dsb[:, q * 512 : (q + 1) * 512],
                        )

                def peel_compact(t):
                    """top-20 selection for tile t from its 64 candidates."""
                    cand, cidx = cands.pop(t)
                    nc.vector.tensor_add(cidx, cidx, choff_sb)
                    candw = tkp.tile([128, 64], f32, tag="candw", name="candw")
                    nc.vector.tensor_copy(candw, cand)
                    t8 = tkp.tile([128, 24], f32, tag="t8", name="t8")
                    nc.vector.max(out=t8[:, 0:8], in_=candw)
                    nc.vector.match_replace(out=candw, in_to_replace=t8[:, 0:8],
                                            in_values=candw, imm_value=NEG)
                    nc.vector.max(out=t8[:, 8:16], in_=candw)
                    nc.vector.match_replace(out=candw,
                                            in_to_replace=t8[:, 8:16],
                                            in_values=candw, imm_value=NEG)
                    nc.vector.max(out=t8[:, 16:24], in_=candw)
                    mask = tkp.tile([128, 64], f32, tag="mask", name="mask")
                    nc.vector.tensor_scalar(mask, cand, t8[:, 19:20], None,
                                            op0=ALU.is_ge)
                    cums = tkp.tile([128, 64], f32, tag="cums", name="cums")
                    nc.vector.tensor_tensor_scan(cums, mask, zero128, 0.0,
                                                 op0=ALU.add, op1=ALU.add)
                    # slot = cums*mask - 1  (-1 marks non-selected: ignored
                    # by local_scatter)
                    nc.vector.tensor_mul(cums, cums, mask)
                    nc.vector.tensor_scalar_add(cums, cums, -1.0)
                    slot = tkp.tile([128, 64], i16, tag="slot", name="slot")
                    nc.vector.tensor_copy(slot, cums)
                    sel = tkp.tile([128, 24], u16, tag="sel", name="sel")
                    nc.gpsimd.local_scatter(out_ap=sel, data_ap=cidx,
                                            idxs_ap=slot, channels=128,
                                            num_elems=24, num_idxs=64)
                    nc.sync.dma_start(
                        idx_dram[t * 128 : (t + 1) * 128, :],
                        sel[:, 0:KNN].bitcast(i16),
                    )

                # gather calls: (k0, nk, widx column base within the group).
                # The first call covers a single neighbor slot so the edge
                # pipeline starts as soon as possible after the peels.
                CALLS = [(0, 1, 0), (1, 4, 64), (5, 4, 320), (9, 4, 576),
                         (13, 4, 832), (17, 3, 1088)]

                def reformat(grp):
                    """widx[p, grp*G + base + r*nk + kk]
                         = idx_dram[1024*grp + 16*r + p, k0 + kk],
                    replicated over the four 16-partition groups; one DMA per
                    (gather call, replica) so the first call's indices land
                    first and later transfers overlap the gathers."""
                    for k0, nk, base in CALLS:
                        src = idx_dram[grp * 1024 : (grp + 1) * 1024,
                                       k0 : k0 + nk].rearrange(
                            "(r p) k -> p r k", p=16
                        )
                        for rep in range(4):
                            dst = widx[rep * 16 : (rep + 1) * 16,
                                       grp * G + base
                                       : grp * G + base + 64 * nk].rearrange(
                                "p (r k) -> p r k", k=nk
                            )
                            nc.sync.dma_start(dst, src)

                def gather_call(grp, call):
                    k0, nk, base = CALLS[call]
                    g = gp.tile([64, 4096], f32, tag="g", name="g")
                    nc.gpsimd.ap_gather(
                        out_ap=g[:, 0 : 1024 * nk],
                        in_ap=u_t,
                        idxs_ap=widx[:, grp * G + base
                                     : grp * G + base + 64 * nk],
                        channels=64,
                        num_elems=N,
                        d=1,
                        num_idxs=1024 * nk,
                    )
                    # columns are (r, kk)-interleaved: col = nk*16*r + 16*kk + p
                    return g[:, 0 : 1024 * nk].rearrange(
                        "c (r f p) -> c f r p", f=nk, p=16
                    )

                def edge_k(lid, grp, gv, kk, k, zacc):
                    """one neighbor slot: y = Prelu(I u_j + V x_i + c);
                    z psum = W2 y; zacc = max(zacc, z).  Layer 3 needs only
                    the running max of the gathered u."""
                    P = LAYER[lid]
                    cin = P["cin"]
                    gk = gv[:, kk]  # (64, 64, 16): r-major, p-minor
                    if lid == 3:
                        gsl = slice(grp * 1024, (grp + 1) * 1024)
                        a3 = acc3[:, gsl].rearrange("c (r p) -> c r p", p=16)
                        if k == 0:
                            nc.vector.tensor_copy(a3, gk)
                        else:
                            nc.vector.tensor_max(a3, a3, gk)
                        return
                    for q in range(2):
                        rsl = slice(q * 32, (q + 1) * 32)
                        csl = slice(q * 512, (q + 1) * 512)
                        msl = slice(grp * 1024 + q * 512,
                                    grp * 1024 + (q + 1) * 512)
                        yp = pse.tile([64, 512], f32, tag="e", name="yp")
                        nc.tensor.matmul(yp, wsb["id64"], gk[:, rsl],
                                         start=True, stop=False)
                        nc.tensor.matmul(yp, wsb[P["vTh"]], x2r[0:cin, msl],
                                         start=False, stop=True)
                        y = ypl.tile([64, 512], bf, tag="y", name="y")
                        nc.scalar.activation(y, yp, AF.Prelu, bias=wsb[P["c"]],
                                             alpha=0.2)
                        zp = pse.tile([64, 512], f32, tag="c2", name="zp")
                        nc.tensor.matmul(zp, wsb[P["w2"]], y)
                        if k == 0:
                            nc.vector.tensor_copy(zacc[:, csl], zp)
                        else:
                            nc.vector.tensor_max(zacc[:, csl], zacc[:, csl],
                                                 zp)

                def edge_fin(lid, grp, zacc):
                    P = LAYER[lid]
                    gsl = slice(grp * 1024, (grp + 1) * 1024)
                    x_out = XOUT[lid]
                    if lid == 3:
                        for q in range(2):
                            sl = slice(grp * 1024 + q * 512,
                                       grp * 1024 + (q + 1) * 512)
                            pe_ = pse.tile([64, 512], f32, tag="e", name="pe3")
                            nc.tensor.matmul(pe_, wsb["id64"], acc3[:, sl],
                                             start=True, stop=False)
                            nc.tensor.matmul(pe_, wsb[P["vTh"]],
                                             x2r[0 : P["cin"], sl],
                                             start=False, stop=True)
                            nc.scalar.activation(x_out[:, sl], pe_, AF.Prelu,
                                                 bias=wsb[P["c"]], alpha=0.2)
                    else:
                        nc.scalar.activation(x_out[:, gsl], zacc, AF.Prelu,
                                             bias=wsb[P["cc"]], alpha=0.2)

                ccs = {}

                def exchange_start(lid, grp):
                    """export x_out group plus the hi/lo bf16 pair carrying
                    its fp32 -|x|^2 row, and launch the pairwise AllGather."""
                    gsl = slice(grp * 1024, (grp + 1) * 1024)
                    x_half = XOUT[lid]
                    xsq = ppl.tile([64, 1024], f32, tag="xsq", name="xsqe")
                    nc.scalar.activation(xsq, x_half[:, gsl], AF.Square)
                    sqrow = ppl.tile([1, 1024], f32, tag="sqr", name="sqrowe")
                    for j in range(2):
                        jl = slice(j * 512, (j + 1) * 512)
                        pq = psd.tile([128, 1024], f32, tag="d", name="pqe")
                        nc.tensor.matmul(pq[0:1, 0:512], wsb["ones64"],
                                         xsq[:, jl])
                        nc.scalar.mul(sqrow[:, jl], pq[0:1, 0:512], -1.0)
                    hi = ppl.tile([1, 1024], bf, tag="hi", name="hi")
                    nc.scalar.copy(hi, sqrow)
                    lo = ppl.tile([1, 1024], bf, tag="lo", name="lo")
                    nc.vector.tensor_tensor(lo, sqrow, hi, op=ALU.subtract)
                    ccin = drp.tile([66, 1024], bf, tag="cci%d" % grp,
                                    name="ccin%d" % grp)
                    nc.scalar.dma_start(ccin[0:64], x_half[:, gsl])
                    nc.scalar.dma_start(ccin[64:65], hi)
                    nc.scalar.dma_start(ccin[65:66], lo)
                    ccout = drp.tile([132, 1024], bf, tag="cco%d" % grp,
                                     name="ccout%d" % grp)
                    nc.gpsimd.collective_compute(
                        "AllGather", ALU.bypass, replica_groups=PAIRS,
                        ins=[ccin], outs=[ccout],
                    )
                    return ccout

                def exchange_finish(ccout, grp):
                    lo0 = grp * 1024
                    lo1 = 2048 + grp * 1024
                    nc.sync.dma_start(rhsF[0:64, lo0 : lo0 + 1024], ccout[0:64])
                    nc.sync.dma_start(rhsF[64:66, lo0 : lo0 + 1024],
                                      ccout[64:66])
                    nc.sync.dma_start(rhsF[0:64, lo1 : lo1 + 1024],
                                      ccout[66:130])
                    nc.sync.dma_start(rhsF[64:66, lo1 : lo1 + 1024],
                                      ccout[130:132])

                def lhs_prep(lid, grp):
                    """lhsTm/x2r columns for group grp from layer lid's out."""
                    gsl = slice(grp * 1024, (grp + 1) * 1024)
                    x_out = XOUT[lid]
                    nc.scalar.mul(lhsTm[0:64, gsl], x_out[:, gsl], 2.0)
                    nc.scalar.mul(x2r[0:64, gsl], x_out[:, gsl], 2.0)

                def edge_steps(lid, grp):
                    """emission steps for the edge phase of (lid, grp); the
                    final step also launches the AllGather for layers 1-2."""
                    steps = []
                    state = {}
                    zacc = None
                    if lid != 3:
                        zacc = zap.tile([64, 1024], f32, tag="za", name="zacc")

                    def mk_gather(call):
                        def f():
                            state[call] = gather_call(grp, call)
                        return f

                    def mk_k(call, kk, k):
                        def f():
                            edge_k(lid, grp, state[call], kk, k, zacc)
                        return f

                    def fin():
                        edge_fin(lid, grp, zacc)
                        if lid < 3:
                            ccs[grp] = exchange_start(lid, grp)
                            lhs_prep(lid, grp)

                    for call, (k0, nk, base) in enumerate(CALLS):
                        steps.append(mk_gather(call))
                        for kk in range(nk):
                            steps.append(mk_k(call, kk, k0 + kk))
                    steps.append(fin)
                    return steps

                carry = []

                def phase_B(lid):
                    """grp0 edge (leading, so its AllGather fires early)
                    followed by grp1 tiles' dist/topk."""
                    a = []
                    for t in range(8, 16):
                        for blk in range(4):
                            a.append(lambda t=t, blk=blk:
                                     dist_quarter(lid, t, blk))
                        a.append(lambda t=t: peel_compact(t))
                    for f in a[:8]:
                        f()
                    for f in edge_steps(lid, 0):
                        f()
                    for f in a[8:]:
                        f()
                    reformat(1)

                def phase_C(lid):
                    """grp1 edge followed by the next layer's lead work (or
                    conv6's first half for layer 3)."""
                    nxt = lid + 1 if lid < 3 else None
                    if nxt is not None:
                        exchange_finish(ccs[0], 0)
                    es = edge_steps(lid, 1)
                    if nxt is None:
                        _mix([lambda ob=ob: conv6_chunk(ob, 0)
                              for ob in range(8)], es)
                        return
                    # lead with a few next-layer dist quarters: rhsF for the
                    # g0 blocks landed mid-phase-B, so these are ready and
                    # cover the reformat+gather spin-up of the g1 edge
                    for t in range(3):
                        dist_quarter(nxt, t, 0)
                    for f in es:
                        f()
                    prep_block(nxt, 0)
                    prep_block(nxt, 2048)
                    for t in range(3, 8):
                        dist_quarter(nxt, t, 0)
                    for t in range(3):
                        dist_quarter(nxt, t, 2)
                    for t in range(3, 8):
                        carry.append(lambda t=t: dist_quarter(nxt, t, 2))

                def phase_A(lid):
                    """finish the leftover and second-half dist quarters of
                    tiles 0..8 for layer lid, then peel."""
                    exchange_finish(ccs[1], 1)
                    for f in carry:
                        f()
                    carry.clear()
                    prep_block(lid, 1024)
                    prep_block(lid, 3072)
                    for t in range(8):
                        dist_quarter(lid, t, 1)
                        dist_quarter(lid, t, 3)
                    for t in range(8):
                        peel_compact(t)
                    reformat(0)

                def conv6_chunk(ob, jp):
                    obs = slice(ob * 128, (ob + 1) * 128)
                    pf = psd.tile([128, 1024], f32, tag="d", name="pf6")
                    for q in range(2):
                        sl = slice(jp * 1024 + q * 512,
                                   jp * 1024 + (q + 1) * 512)
                        po = slice(q * 512, (q + 1) * 512)
                        nc.tensor.matmul(pf[:, po], wsb["w6aT"][:, obs],
                                         xcat_a[:, sl], start=True, stop=False)
                        nc.tensor.matmul(pf[:, po], wsb["w6bT"][:, obs],
                                         xcat_b[:, sl], start=False, stop=True)
                    h6 = gp.tile([128, 1024], f32, tag="h6", name="h6", bufs=2)
                    nc.scalar.activation(h6, pf, AF.Prelu,
                                         bias=wsb["c6v"][:, ob : ob + 1],
                                         alpha=0.2)
                    xgt = xgts[ob]
                    nc.vector.reduce_max(xgt[:, jp : jp + 1], h6,
                                         axis=mybir.AxisListType.X)

                xgts = [tkp.tile([128, 2], f32, tag="xgt%d" % ob,
                                 name="xgt%d" % ob) for ob in range(8)]

                # ================= layer 1 lead-in =================
                for blk in range(4):
                    prep_block1(blk * 1024)
                    for t in range(8):
                        dist_quarter(1, t, blk)
                for t in range(8):
                    peel_compact(t)
                reformat(0)

                # ================= the three layers =================
                phase_B(1)
                phase_C(1)
                phase_A(2)
                phase_B(2)
                phase_C(2)
                phase_A(3)
                phase_B(3)
                phase_C(3)

                # ============ conv6 second half + global max pool ============
                for ob in range(8):
                    conv6_chunk(ob, 1)
                    nc.vector.reduce_max(xg_sb[:, ob : ob + 1], xgts[ob],
                                         axis=mybir.AxisListType.X)
                ccg_i = drp.tile([128, 8], f32, tag="cgi", name="ccg_in")
                nc.scalar.dma_start(ccg_i, xg_sb)
                ccg_o = drp.tile([256, 8], f32, tag="cgo", name="ccg_out")
                nc.gpsimd.collective_compute(
                    "AllGather", ALU.bypass, replica_groups=PAIRS,
                    ins=[ccg_i], outs=[ccg_o],
                )

            # layer scratch pools released here; final stage below.
            with tc.tile_pool(name="fin", bufs=1) as fin:
                # conv7 local part without bias (overlaps the AllReduce)
                h7 = fin.tile([128, 4 * H], bf, tag="h7", name="h7")
                for ob in range(4):
                    obs = slice(ob * 128, (ob + 1) * 128)
                    for jp in range(2):
                        pf = psd.tile([128, 1024], f32, tag="d", name="pf7")
                        for q in range(2):
                            sl = slice(jp * 1024 + q * 512,
                                       jp * 1024 + (q + 1) * 512)
                            po = slice(q * 512, (q + 1) * 512)
                            nc.tensor.matmul(pf[:, po], wsb["w7laT"][:, obs],
                                             xcat_a[:, sl],
                                             start=True, stop=False)
                            nc.tensor.matmul(pf[:, po], wsb["w7lbT"][:, obs],
                                             xcat_b[:, sl],
                                             start=False, stop=True)
                        nc.scalar.copy(
                            h7[:, ob * H + jp * 1024 : ob * H + (jp + 1) * 1024],
                            pf,
                        )

                # max of the pair's two xg halves (cheaper than AllReduce,
                # which carries a 1.875x cost multiplier)
                nc.sync.dma_start(xg_sb, ccg_o[0:128])
                xgp = fin.tile([128, 8], f32, tag="xgp", name="xgp")
                nc.sync.dma_start(xgp, ccg_o[128:256])
                nc.vector.tensor_max(xg_sb, xg_sb, xgp)

                # conv7 effective bias: c7 + W7g @ xg
                for ob in range(4):
                    pb = psd.tile([128, 1024], f32, tag="d", name="pb7")
                    for kb in range(8):
                        nc.tensor.matmul(
                            pb[:, 0:1],
                            wsb["w7gT"][:, kb, ob * 128 : (ob + 1) * 128],
                            xg_sb[:, kb : kb + 1],
                            start=(kb == 0),
                            stop=(kb == 7),
                        )
                    nc.scalar.activation(b7_sb[:, ob : ob + 1], pb[:, 0:1],
                                         AF.Identity,
                                         bias=wsb["c7v"][:, ob : ob + 1])

                h7a = fin.tile([128, 4 * H], bf, tag="h7a", name="h7a")
                for jp in range(4):
                    for ob in range(4):
                        sl = slice(ob * H + jp * 512, ob * H + (jp + 1) * 512)
                        nc.scalar.activation(
                            h7a[:, sl], h7[:, sl], AF.Prelu,
                            bias=b7_sb[:, ob : ob + 1], alpha=0.2,
                        )
                h8 = fin.tile([128, 2 * H], bf, tag="h8", name="h8")
                o_sb = fin.tile([13, H], f32, tag="osb", name="o_sb")
                for jp in range(2):
                    for ob in range(2):
                        pf = psd.tile([128, 1024], f32, tag="d", name="pf8")
                        for q in range(2):
                            po = slice(q * 512, (q + 1) * 512)
                            co = jp * 1024 + q * 512
                            for kb in range(4):
                                nc.tensor.matmul(
                                    pf[:, po],
                                    wsb["w8T"][:, kb, ob * 128 : (ob + 1) * 128],
                                    h7a[:, kb * H + co : kb * H + co + 512],
                                    start=(kb == 0),
                                    stop=(kb == 3),
                                )
                        nc.scalar.activation(
                            h8[:, ob * H + jp * 1024 : ob * H + (jp + 1) * 1024],
                            pf, AF.Prelu, bias=wsb["c8v"][:, ob : ob + 1],
                            alpha=0.2,
                        )
                    pf = psd.tile([128, 1024], f32, tag="d", name="pf9")
                    for q in range(2):
                        po = slice(q * 512, (q + 1) * 512)
                        co = jp * 1024 + q * 512
                        for kb in range(2):
                            nc.tensor.matmul(
                                pf[0:13, po],
                                wsb["w9T"][:, kb, :],
                                h8[:, kb * H + co : kb * H + co + 512],
                                start=(kb == 0),
                                stop=(kb == 1),
                            )
                    nc.scalar.activation(
                        o_sb[:, jp * 1024 : (jp + 1) * 1024], pf[0:13],
                        AF.Identity, bias=wsb["b9v"],
                    )
                nc.sync.dma_start(out_d[:], o_sb)

    nc.compile()
    return nc


def _mix(a_steps, b_steps, lead=0):
    """Emit `lead` a-steps, then alternate a/b 1:1 until b is exhausted, then
    the remaining a-steps."""
    ia = 0
    for _ in range(min(lead, len(a_steps))):
        a_steps[ia]()
        ia += 1
    for ib in range(len(b_steps)):
        if ia < len(a_steps):
            a_steps[ia]()
            ia += 1
        b_steps[ib]()
    while ia < len(a_steps):
        a_steps[ia]()
        ia += 1


def make_in_maps(inputs):
    """Per-core input dicts from the full problem inputs."""
    wd = _prep_weights(inputs)
    wpack32 = np.zeros((128, _wcols(0)), dtype=np.float32)
    import ml_dtypes

    wpack16 = np.zeros((128, _wcols(1)), dtype=ml_dtypes.bfloat16)
    col = {0: 0, 1: 0}
    for name, shape, isb in _WEIGHT_SPECS:
        w = int(np.prod(shape[1:])) if len(shape) > 1 else 1
        dst = wpack16 if isb else wpack32
        v = wd[name].reshape(shape[0], w)
        dst[0 : shape[0], col[isb] : col[isb] + w] = (
            _bf16(v) if isb else v.astype(np.float32)
        )
        col[isb] += w
    pts = np.asarray(inputs["points"], dtype=np.float32)
    in_maps = []
    for c in range(8):
        b, h = c // 2, c % 2
        m = {"wpack32": wpack32, "wpack16": wpack16}
        m["wboot"] = np.ascontiguousarray(
            np.concatenate([wd["u1T"], wd["ones3"]], axis=1).astype(np.float32))
        m["choff"] = np.ascontiguousarray(
            np.tile(np.repeat(np.arange(8, dtype=np.uint16) * 512, 8), (128, 1)))
        m["pts_full"] = np.ascontiguousarray(pts[b])
        pm = np.ascontiguousarray(pts[b][:, h * H : (h + 1) * H])
        m["pts_mine"] = pm
        m["pts_mine_bf"] = _bf16(pm)
        in_maps.append(m)
    return in_maps


def kernel(**inputs):
    from concourse.bass_utils import run_bass_kernel_spmd

    if "nc" not in _CACHE:
        _CACHE["nc"] = build_kernel()
    nc = _CACHE["nc"]
    in_maps = make_in_maps(inputs)
    res = run_bass_kernel_spmd(nc, in_maps, core_ids=list(range(8)))
    out = np.zeros((B, 13, N), dtype=np.float32)
    for c in range(8):
        b, h = c // 2, c % 2
        out[b][:, h * H : (h + 1) * H] = res.results[c]["out"]
    return out


# revision 54
# speedup vs baseline: 1.8023x; 1.0145x over previous
"""DGCNN part-segmentation forward pass on 8 Trainium2 NeuronCores.

Sharding: data-parallel over the batch (B=4) x 2-way split of the N=4096
points within each batch element.  Core c handles batch element c//2,
point rows [(c%2)*2048, (c%2+1)*2048).  The two cores of a pair exchange
their half of each EdgeConv output with pairwise AllGathers (one per
1024-point group, launched as soon as that group's output is ready) and
a pairwise AllReduce-max for the global pooling feature.

Precision: the feature path (EdgeConv transforms, convs 6-9) runs in
bfloat16 with fp32 PSUM accumulation.  Layer 1's kNN runs on raw fp32
points (3-D points have many near-ties; bf16 inputs measurably flip
neighbors), via a small separate fp32 lhs/rhs pair.  Layers 2-3 rank
neighbors of the bf16 features exactly: the distance matmul contracts
66 bf16 rows = 64 feature channels + a hi/lo bf16 pair carrying the
fp32 -|x_j|^2 row at ~16-bit precision.  The sq rows travel inside the
AllGather so no distance prep remains on the post-exchange critical
path.

The emission order is software-pipelined: engine queues are in-order,
so each phase leads with a few distance tiles, then runs the edge-conv
phase solid (its AllGather fires early), then the bulk of the
distance/top-k scans; the next layer's first distance quarters overlap
the current layer's second edge phase.

Per EdgeConv layer:
  top-20: per 128-row tile: 8x top-8 per 512-column chunk (DVE Max) ->
          64 candidates; 3 peeling rounds (max8 + match_replace) ->
          top-24 values; threshold mask + prefix-sum compaction +
          gpsimd local_scatter -> global indices of the top-20
  gather: batched GPSIMD ap_gather of the bf16 u = U x transform; the
          first call covers one neighbor slot so the edge pipeline
          starts right after the peels, later calls cover 4 slots
  edge  : psum = I*u_gathered + V*x_i (bf16 matmuls), LeakyReLU (Prelu)
          on the scalar engine, conv2 bf16 matmul; the k-max runs on
          the raw conv2 psums (LeakyReLU is monotone) and one Prelu
          finalizes each group.
"""

import sys

sys.path.insert(0, "/opt/trn_rl_repo")

import numpy as np

B = 4
N = 4096
H = 2048  # points per core (half of a batch element)
KNN = 20
EPS = 1e-5
NEG = -3.0e38
G = KNN * 64  # widx columns per point group

_CACHE = {}


# --------------------------------------------------------------------------
# host-side weight preparation
# --------------------------------------------------------------------------

def _bf16(x):
    import ml_dtypes

    return np.ascontiguousarray(x, dtype=np.float32).astype(ml_dtypes.bfloat16)


def _fold_bn(w, b, g, be):
    s = (g / np.sqrt(np.float32(1.0) + np.float32(EPS))).astype(np.float32)
    return (w * s[:, None]).astype(np.float32), (s * b + be).astype(np.float32)


def _prep_weights(inp):
    f = np.float32
    W, C = {}, {}
    for i in range(1, 9):
        W[i], C[i] = _fold_bn(
            inp["w%d" % i], inp["b%d" % i], inp["g%d" % i], inp["be%d" % i]
        )
    d = {}
    # edge conv layers: split into U (neighbor part) and V (center part).
    # Layer 1's x2r tile holds raw points, layers 2-3 hold 2*x, so the V
    # transpose is halved only for layers 2-3.
    for lid, wi, vscale in ((1, 1, 1.0), (2, 3, 0.5), (3, 5, 0.5)):
        w = W[wi]
        cin = w.shape[1] // 2
        U = w[:, :cin]
        V = w[:, cin:] - w[:, :cin]
        d["u%dT" % lid] = np.ascontiguousarray(U.T)
        d["v%dTh" % lid] = np.ascontiguousarray((V * f(vscale)).T)
        d["c%d" % lid] = C[wi].reshape(64, 1)
    d["w2T"] = np.ascontiguousarray(W[2].T)
    d["cc2"] = C[2].reshape(64, 1)
    d["w4T"] = np.ascontiguousarray(W[4].T)
    d["cc4"] = C[4].reshape(64, 1)
    # conv6 (192 -> 1024)
    w6T = np.ascontiguousarray(W[6].T)  # (192, 1024)
    d["w6aT"] = np.ascontiguousarray(w6T[:128])
    d["w6bT"] = np.ascontiguousarray(w6T[128:])
    d["c6v"] = np.ascontiguousarray(C[6].reshape(8, 128).T)  # (128, 8)
    # conv7 (1216 -> 512): xg part (1024) and local part (192)
    w7 = W[7]
    w7gT = np.ascontiguousarray(w7[:, :1024].T)  # (1024, 512)
    d["w7gT"] = np.ascontiguousarray(w7gT.reshape(8, 128, 512).transpose(1, 0, 2))
    w7lT = np.ascontiguousarray(w7[:, 1024:].T)  # (192, 512)
    d["w7laT"] = np.ascontiguousarray(w7lT[:128])
    d["w7lbT"] = np.ascontiguousarray(w7lT[128:])
    d["c7v"] = np.ascontiguousarray(C[7].reshape(4, 128).T)  # (128, 4)
    # conv8 (512 -> 256)
    w8T = np.ascontiguousarray(W[8].T)  # (512, 256)
    d["w8T"] = np.ascontiguousarray(w8T.reshape(4, 128, 256).transpose(1, 0, 2))
    d["c8v"] = np.ascontiguousarray(C[8].reshape(2, 128).T)  # (128, 2)
    # conv9 (256 -> 13), plain linear
    w9T = np.ascontiguousarray(inp["w9"].T.astype(f))  # (256, 13)
    d["w9T"] = np.ascontiguousarray(w9T.reshape(2, 128, 13).transpose(1, 0, 2))
    d["b9v"] = inp["b9"].astype(f).reshape(13, 1)
    # constants
    d["id64"] = np.eye(64, dtype=f)
    d["ones3"] = np.ones((3, 1), dtype=f)
    d["ones64"] = np.ones((64, 1), dtype=f)
    return d


# name -> (shape, is_bf16)
_WEIGHT_SPECS = [
    ("u1T", (3, 64), 0), ("v1Th", (3, 64), 1), ("c1", (64, 1), 0),
    ("w2T", (64, 64), 1), ("cc2", (64, 1), 0),
    ("u2T", (64, 64), 1), ("v2Th", (64, 64), 1), ("c2", (64, 1), 0),
    ("w4T", (64, 64), 1), ("cc4", (64, 1), 0),
    ("u3T", (64, 64), 1), ("v3Th", (64, 64), 1), ("c3", (64, 1), 0),
    ("w6aT", (128, 1024), 1), ("w6bT", (64, 1024), 1), ("c6v", (128, 8), 0),
    ("w7gT", (128, 8, 512), 0), ("w7laT", (128, 512), 1),
    ("w7lbT", (64, 512), 1), ("c7v", (128, 4), 0),
    ("w8T", (128, 4, 256), 1), ("c8v", (128, 2), 0),
    ("w9T", (128, 2, 13), 1), ("b9v", (13, 1), 0),
    ("id64", (64, 64), 0), ("ones3", (3, 1), 0), ("ones64", (64, 1), 0),
]


def _wcols(bf):
    return sum(int(np.prod(s[1:])) if len(s) > 1 else 1
               for _, s, isb in _WEIGHT_SPECS if isb == bf)


# --------------------------------------------------------------------------
# device kernel builder
# --------------------------------------------------------------------------

def build_kernel():
    import concourse.bacc as bacc
    import concourse.mybir as mybir
    import concourse.tile as tile

    f32 = mybir.dt.float32
    bf = mybir.dt.bfloat16
    i16 = mybir.dt.int16
    u16 = mybir.dt.uint16
    AF = mybir.ActivationFunctionType
    ALU = mybir.AluOpType
    PAIRS = [[0, 1], [2, 3], [4, 5], [6, 7]]

    nc = bacc.Bacc("TRN2", target_bir_lowering=False, num_devices=8)

    pts_full = nc.dram_tensor("pts_full", [3, N], f32, kind="ExternalInput")
    pts_mine = nc.dram_tensor("pts_mine", [3, H], f32, kind="ExternalInput")
    pts_mine_bf = nc.dram_tensor("pts_mine_bf", [3, H], bf,
                                 kind="ExternalInput")
    choff_d = nc.dram_tensor("choff", [128, 64], u16, kind="ExternalInput")
    wboot_d = nc.dram_tensor("wboot", [3, 65], f32, kind="ExternalInput")
    wpack32_d = nc.dram_tensor("wpack32", [128, _wcols(0)], f32,
                               kind="ExternalInput")
    wpack16_d = nc.dram_tensor("wpack16", [128, _wcols(1)], bf,
                               kind="ExternalInput")
    out_d = nc.dram_tensor("out", [13, H], f32, kind="ExternalOutput")

    with tile.TileContext(nc) as tc:
        with (
            tc.tile_pool(name="wp", bufs=1) as wp,
            tc.tile_pool(name="per", bufs=1) as per,
            tc.tile_pool(name="psd", bufs=2, space="PSUM") as psd,
            tc.tile_pool(name="pse", bufs=2, space="PSUM") as pse,
            tc.tile_pool(name="dram", bufs=1, space="DRAM") as drp,
        ):
            # ---- persistent tiles ----
            rhsF = per.tile([66, N], bf, tag="rhsF", name="rhsF")
            rhsF1 = per.tile([4, N], f32, tag="rhsF1", name="rhsF1")
            lhsTm = per.tile([66, H], bf, tag="lhsTm", name="lhsTm")
            lhsTm1 = per.tile([4, H], f32, tag="lhsTm1", name="lhsTm1")
            u_t = per.tile([64, N], f32, tag="u", name="u_t")
            x2r = per.tile([64, H], bf, tag="x2r", name="x2r")
            xcat_a = per.tile([128, H], bf, tag="xcata", name="xcat_a")
            xcat_b = per.tile([64, H], bf, tag="xcatb", name="xcat_b")
            widx = per.tile([64, KNN * 128], i16, tag="widx", name="widx")
            acc3 = per.tile([64, H], f32, tag="acc3", name="acc3")
            xg_sb = per.tile([128, 8], f32, tag="xg", name="xg_sb")
            b7_sb = per.tile([128, 4], f32, tag="b7", name="b7_sb")
            choff_sb = per.tile([128, 64], u16, tag="choff", name="choff_sb")
            zero128 = per.tile([128, 64], f32, tag="z128", name="zero128")

            idx_dram = drp.tile([H, KNN], i16, tag="idxd", name="idx_dram")

            x1h = xcat_a[0:64]
            x2h = xcat_a[64:128]
            x3h = xcat_b

            # critical inputs first; the row-3 ones of lhsTm1 come from a
            # 32-aligned memset that the pts DMA then partially overwrites
            wbt = wp.tile([3, 65], f32, tag="wb", name="wboot_sb")
            nc.sync.dma_start(wbt, wboot_d[:])
            nc.vector.memset(lhsTm1[0:4], 1.0)
            nc.sync.dma_start(lhsTm1[0:3], pts_mine[:])
            nc.sync.dma_start(x2r[0:3], pts_mine_bf[:])
            nc.sync.dma_start(choff_sb, choff_d[:])
            nc.vector.memset(zero128, 0.0)
            nc.vector.memset(lhsTm[64:66], 1.0)
            # all weights arrive in two packed DMAs; per-weight tiles are
            # views into the packed tiles
            wpt32 = wp.tile([128, _wcols(0)], f32, tag="wp32", name="wp32")
            nc.scalar.dma_start(wpt32, wpack32_d[:])
            wpt16 = wp.tile([128, _wcols(1)], bf, tag="wp16", name="wp16")
            nc.scalar.dma_start(wpt16, wpack16_d[:])
            wsb = {}
            col = {0: 0, 1: 0}
            for name, shape, isb in _WEIGHT_SPECS:
                w = int(np.prod(shape[1:])) if len(shape) > 1 else 1
                src = wpt16 if isb else wpt32
                v = src[0 : shape[0], col[isb] : col[isb] + w]
                if len(shape) == 3:
                    v = v.rearrange("p (a b) -> p a b", a=shape[1])
                wsb[name] = v
                col[isb] += w
            wsb["u1T"] = wbt[0:3, 0:64]
            wsb["ones3"] = wbt[0:3, 64:65]

            LAYER = {
                1: dict(cin=3, uT="u1T", vTh="v1Th", c="c1", w2="w2T",
                        cc="cc2"),
                2: dict(cin=64, uT="u2T", vTh="v2Th", c="c2", w2="w4T",
                        cc="cc4"),
                3: dict(cin=64, uT="u3T", vTh="v3Th", c="c3", w2=None,
                        cc=None),
            }
            XOUT = {1: x1h, 2: x2h, 3: x3h}

            with (
                tc.tile_pool(name="dsb", bufs=4) as dsbp,
                tc.tile_pool(name="pp", bufs=2) as ppl,
                tc.tile_pool(name="tk", bufs=2) as tkp,
                tc.tile_pool(name="gp", bufs=2) as gp,
                tc.tile_pool(name="yp", bufs=3) as ypl,
                tc.tile_pool(name="za", bufs=1) as zap,
            ):
                cands = {}

                def prep_block1(lo):
                    """layer-1 lead: rhsF1 = [2*pts; -|pts|^2] and
                    u_t = u1T.T @ pts for columns [lo, lo+1024)."""
                    stage = ppl.tile([3, 1024], f32, tag="pts", name="stage")
                    nc.sync.dma_start(stage, pts_full[:, lo : lo + 1024])
                    nc.scalar.mul(rhsF1[0:3, lo : lo + 1024], stage, 2.0)
                    xsq = ppl.tile([64, 1024], f32, tag="xsq", name="xsq1")
                    nc.scalar.activation(xsq[0:3], stage, AF.Square)
                    sqrow = ppl.tile([1, 1024], f32, tag="sqr", name="sqrow1")
                    for j in range(2):
                        jl = slice(j * 512, (j + 1) * 512)
                        pq = psd.tile([128, 1024], f32, tag="d", name="pq")
                        nc.tensor.matmul(pq[0:1, 0:512], wsb["ones3"],
                                         xsq[0:3, jl])
                        nc.scalar.mul(sqrow[:, jl], pq[0:1, 0:512], -1.0)
                        pu = psd.tile([128, 1024], f32, tag="d", name="pu")
                        nc.tensor.matmul(pu[0:64, 0:512], wsb["u1T"],
                                         stage[:, jl])
                        nc.scalar.copy(u_t[:, lo + j * 512 : lo + (j + 1) * 512],
                                       pu[0:64, 0:512])
                    nc.sync.dma_start(rhsF1[3:4, lo : lo + 1024], sqrow)

                def prep_block(lid, lo):
                    """u_t[:, lo:lo+1024] = uT.T @ rhsF[0:cin] (bf16)."""
                    P = LAYER[lid]
                    cin = P["cin"]
                    for j in range(2):
                        sl = slice(lo + j * 512, lo + (j + 1) * 512)
                        pu = psd.tile([128, 1024], f32, tag="d", name="pu")
                        nc.tensor.matmul(pu[0:64, 0:512], wsb[P["uT"]],
                                         rhsF[0:cin, sl])
                        nc.scalar.copy(u_t[:, sl], pu[0:64, 0:512])

                def dist_quarter(lid, t, blk):
                    """distance psum + copy + top8 scan for tile t, column
                    block blk (1024 cols), ranking for layer lid."""
                    if t not in cands:
                        cands[t] = (
                            tkp.tile([128, 64], f32, tag="cand", name="cand",
                                     bufs=12),
                            tkp.tile([128, 64], u16, tag="cidx", name="cidx",
                                     bufs=12),
                        )
                    cand, cidx = cands[t]
                    lo = blk * 1024
                    pd = psd.tile([128, 1024], f32, tag="d", name="pd")
                    for q in range(2):
                        cs = slice(lo + q * 512, lo + (q + 1) * 512)
                        if lid == 1:
                            nc.tensor.matmul(
                                pd[:, q * 512 : (q + 1) * 512],
                                lhsTm1[:, t * 128 : (t + 1) * 128],
                                rhsF1[:, cs],
                            )
                        else:
                            nc.tensor.matmul(
                                pd[:, q * 512 : (q + 1) * 512],
                                lhsTm[:, t * 128 : (t + 1) * 128],
                                rhsF[:, cs],
                            )
                    dsb = dsbp.tile([128, 1024], f32, tag="dsb", name="dsb")
                    nc.scalar.copy(dsb, pd)
                    for q in range(2):
                        cc = blk * 2 + q
                        nc.vector.max(
                            out=cand[:, cc * 8 : (cc + 1) * 8],
                            in_=dsb[:, q * 512 : (q + 1) * 512],
                        )
                        nc.vector.max_index(
                            cidx[:, cc * 8 : (cc + 1) * 8],
                            cand[:, cc * 8 : (cc + 1) * 8],
                            dsb[:, q * 512 : (q + 1) * 512],
                        )

                def peel_compact(t):
                    """top-20 selection for tile t from its 64 candidates."""
                    cand, cidx = cands.pop(t)
                    nc.vector.tensor_add(cidx, cidx, choff_sb)
                    candw = tkp.tile([128, 64], f32, tag="candw", name="candw")
                    nc.vector.tensor_copy(candw, cand)
                    t8 = tkp.tile([128, 24], f32, tag="t8", name="t8")
                    nc.vector.max(out=t8[:, 0:8], in_=candw)
                    nc.vector.match_replace(out=candw, in_to_replace=t8[:, 0:8],
                                            in_values=candw, imm_value=NEG)
                    nc.vector.max(out=t8[:, 8:16], in_=candw)
                    nc.vector.match_replace(out=candw,
                                            in_to_replace=t8[:, 8:16],
                                            in_values=candw, imm_value=NEG)
                    nc.vector.max(out=t8[:, 16:24], in_=candw)
                    mask = tkp.tile([128, 64], f32, tag="mask", name="mask")
                    nc.vector.tensor_scalar(mask, cand, t8[:, 19:20], None,
                                            op0=ALU.is_ge)
                    cums = tkp.tile([128, 64], f32, tag="cums", name="cums")
                    nc.vector.tensor_tensor_scan(cums, mask, zero128, 0.0,
                                                 op0=ALU.add, op1=ALU.add)
                    # slot = cums*mask - 1  (-1 marks non-selected: ignored
                    # by local_scatter)
                    nc.vector.tensor_mul(cums, cums, mask)
                    nc.vector.tensor_scalar_add(cums, cums, -1.0)
                    slot = tkp.tile([128, 64], i16, tag="slot", name="slot")
                    nc.vector.tensor_copy(slot, cums)
                    sel = tkp.tile([128, 24], u16, tag="sel", name="sel")
                    nc.gpsimd.local_scatter(out_ap=sel, data_ap=cidx,
                                            idxs_ap=slot, channels=128,
                                            num_elems=24, num_idxs=64)
                    nc.sync.dma_start(
                        idx_dram[t * 128 : (t + 1) * 128, :],
                        sel[:, 0:KNN].bitcast(i16),
                    )

                # gather calls: (k0, nk, widx column base within the group).
                # The first call covers a single neighbor slot so the edge
                # pipeline starts as soon as possible after the peels.
                CALLS = [(0, 1, 0), (1, 4, 64), (5, 4, 320), (9, 4, 576),
                         (13, 4, 832), (17, 3, 1088)]

                def reformat(grp):
                    """widx[p, grp*G + base + r*nk + kk]
                         = idx_dram[1024*grp + 16*r + p, k0 + kk],
                    replicated over the four 16-partition groups; one DMA per
                    (gather call, replica) so the first call's indices land
                    first and later transfers overlap the gathers."""
                    for k0, nk, base in CALLS:
                        src = idx_dram[grp * 1024 : (grp + 1) * 1024,
                                       k0 : k0 + nk].rearrange(
                            "(r p) k -> p r k", p=16
                        )
                        for rep in range(4):
                            dst = widx[rep * 16 : (rep + 1) * 16,
                                       grp * G + base
                                       : grp * G + base + 64 * nk].rearrange(
                                "p (r k) -> p r k", k=nk
                            )
                            nc.sync.dma_start(dst, src)

                def gather_call(grp, call):
                    k0, nk, base = CALLS[call]
                    g = gp.tile([64, 4096], f32, tag="g", name="g")
                    nc.gpsimd.ap_gather(
                        out_ap=g[:, 0 : 1024 * nk],
                        in_ap=u_t,
                        idxs_ap=widx[:, grp * G + base
                                     : grp * G + base + 64 * nk],
                        channels=64,
                        num_elems=N,
                        d=1,
                        num_idxs=1024 * nk,
                    )
                    # columns are (r, kk)-interleaved: col = nk*16*r + 16*kk + p
                    return g[:, 0 : 1024 * nk].rearrange(
                        "c (r f p) -> c f r p", f=nk, p=16
                    )

                def edge_k(lid, grp, gv, kk, k, zacc):
                    """one neighbor slot: y = Prelu(I u_j + V x_i + c);
                    z psum = W2 y; zacc = max(zacc, z).  Layer 3 needs only
                    the running max of the gathered u."""
                    P = LAYER[lid]
                    cin = P["cin"]
                    gk = gv[:, kk]  # (64, 64, 16): r-major, p-minor
                    if lid == 3:
                        gsl = slice(grp * 1024, (grp + 1) * 1024)
                        a3 = acc3[:, gsl].rearrange("c (r p) -> c r p", p=16)
                        if k == 0:
                            nc.vector.tensor_copy(a3, gk)
                        else:
                            nc.vector.tensor_max(a3, a3, gk)
                        return
                    for q in range(2):
                        rsl = slice(q * 32, (q + 1) * 32)
                        csl = slice(q * 512, (q + 1) * 512)
                        msl = slice(grp * 1024 + q * 512,
                                    grp * 1024 + (q + 1) * 512)
                        yp = pse.tile([64, 512], f32, tag="e", name="yp")
                        nc.tensor.matmul(yp, wsb["id64"], gk[:, rsl],
                                         start=True, stop=False)
                        nc.tensor.matmul(yp, wsb[P["vTh"]], x2r[0:cin, msl],
                                         start=False, stop=True)
                        y = ypl.tile([64, 512], bf, tag="y", name="y")
                        nc.scalar.activation(y, yp, AF.Prelu, bias=wsb[P["c"]],
                                             alpha=0.2)
                        zp = pse.tile([64, 512], f32, tag="c2", name="zp")
                        nc.tensor.matmul(zp, wsb[P["w2"]], y)
                        if k == 0:
                            nc.vector.tensor_copy(zacc[:, csl], zp)
                        else:
                            nc.vector.tensor_max(zacc[:, csl], zacc[:, csl],
                                                 zp)

                def edge_fin(lid, grp, zacc):
                    P = LAYER[lid]
                    gsl = slice(grp * 1024, (grp + 1) * 1024)
                    x_out = XOUT[lid]
                    if lid == 3:
                        for q in range(2):
                            sl = slice(grp * 1024 + q * 512,
                                       grp * 1024 + (q + 1) * 512)
                            pe_ = pse.tile([64, 512], f32, tag="e", name="pe3")
                            nc.tensor.matmul(pe_, wsb["id64"], acc3[:, sl],
                                             start=True, stop=False)
                            nc.tensor.matmul(pe_, wsb[P["vTh"]],
                                             x2r[0 : P["cin"], sl],
                                             start=False, stop=True)
                            nc.scalar.activation(x_out[:, sl], pe_, AF.Prelu,
                                                 bias=wsb[P["c"]], alpha=0.2)
                    else:
                        nc.scalar.activation(x_out[:, gsl], zacc, AF.Prelu,
                                             bias=wsb[P["cc"]], alpha=0.2)

                ccs = {}

                def exchange_start(lid, grp):
                    """export x_out group plus the hi/lo bf16 pair carrying
                    its fp32 -|x|^2 row, and launch the pairwise AllGather."""
                    gsl = slice(grp * 1024, (grp + 1) * 1024)
                    x_half = XOUT[lid]
                    xsq = ppl.tile([64, 1024], f32, tag="xsq", name="xsqe")
                    nc.scalar.activation(xsq, x_half[:, gsl], AF.Square)
                    sqrow = ppl.tile([1, 1024], f32, tag="sqr", name="sqrowe")
                    for j in range(2):
                        jl = slice(j * 512, (j + 1) * 512)
                        pq = psd.tile([128, 1024], f32, tag="d", name="pqe")
                        nc.tensor.matmul(pq[0:1, 0:512], wsb["ones64"],
                                         xsq[:, jl])
                        nc.scalar.mul(sqrow[:, jl], pq[0:1, 0:512], -1.0)
                    hi = ppl.tile([1, 1024], bf, tag="hi", name="hi")
                    nc.scalar.copy(hi, sqrow)
                    lo = ppl.tile([1, 1024], bf, tag="lo", name="lo")
                    nc.vector.tensor_tensor(lo, sqrow, hi, op=ALU.subtract)
                    ccin = drp.tile([66, 1024], bf, tag="cci%d" % grp,
                                    name="ccin%d" % grp)
                    nc.scalar.dma_start(ccin[0:64], x_half[:, gsl])
                    nc.scalar.dma_start(ccin[64:65], hi)
                    nc.scalar.dma_start(ccin[65:66], lo)
                    ccout = drp.tile([132, 1024], bf, tag="cco%d" % grp,
                                     name="ccout%d" % grp)
                    nc.gpsimd.collective_compute(
                        "AllGather", ALU.bypass, replica_groups=PAIRS,
                        ins=[ccin], outs=[ccout],
                    )
                    return ccout

                def exchange_finish(ccout, grp):
                    lo0 = grp * 1024
                    lo1 = 2048 + grp * 1024
                    nc.sync.dma_start(rhsF[0:64, lo0 : lo0 + 1024], ccout[0:64])
                    nc.sync.dma_start(rhsF[64:66, lo0 : lo0 + 1024],
                                      ccout[64:66])
                    nc.sync.dma_start(rhsF[0:64, lo1 : lo1 + 1024],
                                      ccout[66:130])
                    nc.sync.dma_start(rhsF[64:66, lo1 : lo1 + 1024],
                                      ccout[130:132])

                def lhs_prep(lid, grp):
                    """lhsTm/x2r columns for group grp from layer lid's out."""
                    gsl = slice(grp * 1024, (grp + 1) * 1024)
                    x_out = XOUT[lid]
                    nc.scalar.mul(lhsTm[0:64, gsl], x_out[:, gsl], 2.0)
                    nc.scalar.mul(x2r[0:64, gsl], x_out[:, gsl], 2.0)

                def edge_steps(lid, grp):
                    """emission steps for the edge phase of (lid, grp); the
                    final step also launches the AllGather for layers 1-2."""
                    steps = []
                    state = {}
                    zacc = None
                    if lid != 3:
                        zacc = zap.tile([64, 1024], f32, tag="za", name="zacc")

                    def mk_gather(call):
                        def f():
                            state[call] = gather_call(grp, call)
                        return f

                    def mk_k(call, kk, k):
                        def f():
                            edge_k(lid, grp, state[call], kk, k, zacc)
                        return f

                    def fin():
                        edge_fin(lid, grp, zacc)
                        if lid < 3:
                            ccs[grp] = exchange_start(lid, grp)
                            lhs_prep(lid, grp)

                    for call, (k0, nk, base) in enumerate(CALLS):
                        steps.append(mk_gather(call))
                        for kk in range(nk):
                            steps.append(mk_k(call, kk, k0 + kk))
                    steps.append(fin)
                    return steps

                carry = []

                def phase_B(lid):
                    """grp0 edge (leading, so its AllGather fires early)
                    followed by grp1 tiles' dist/topk."""
                    a = []
                    for t in range(8, 16):
                        for blk in range(4):
                            a.append(lambda t=t, blk=blk:
                                     dist_quarter(lid, t, blk))
                        a.append(lambda t=t: peel_compact(t))
                    for f in a[:10]:
                        f()
                    for f in edge_steps(lid, 0):
                        f()
                    for f in a[10:]:
                        f()
                    reformat(1)

                def phase_C(lid):
                    """grp1 edge followed by the next layer's lead work (or
                    conv6's first half for layer 3)."""
                    nxt = lid + 1 if lid < 3 else None
                    if nxt is not None:
                        exchange_finish(ccs[0], 0)
                    es = edge_steps(lid, 1)
                    if nxt is None:
                        _mix([lambda ob=ob: conv6_chunk(ob, 0)
                              for ob in range(8)], es)
                        return
                    # lead with a few next-layer dist quarters: rhsF for the
                    # g0 blocks landed mid-phase-B, so these are ready and
                    # cover the reformat+gather spin-up of the g1 edge
                    for t in range(4):
                        dist_quarter(nxt, t, 0)
                    for f in es:
                        f()
                    prep_block(nxt, 0)
                    prep_block(nxt, 2048)
                    for t in range(4, 8):
                        dist_quarter(nxt, t, 0)
                    for t in range(3):
                        dist_quarter(nxt, t, 2)
                    for t in range(3, 8):
                        carry.append(lambda t=t: dist_quarter(nxt, t, 2))

                def phase_A(lid):
                    """finish the leftover and second-half dist quarters of
                    tiles 0..8 for layer lid, then peel."""
                    exchange_finish(ccs[1], 1)
                    for f in carry:
                        f()
                    carry.clear()
                    prep_block(lid, 1024)
                    prep_block(lid, 3072)
                    for t in range(8):
                        dist_quarter(lid, t, 1)
                        dist_quarter(lid, t, 3)
                    for t in range(8):
                        peel_compact(t)
                    reformat(0)

                def conv6_chunk(ob, jp):
                    obs = slice(ob * 128, (ob + 1) * 128)
                    pf = psd.tile([128, 1024], f32, tag="d", name="pf6")
                    for q in range(2):
                        sl = slice(jp * 1024 + q * 512,
                                   jp * 1024 + (q + 1) * 512)
                        po = slice(q * 512, (q + 1) * 512)
                        nc.tensor.matmul(pf[:, po], wsb["w6aT"][:, obs],
                                         xcat_a[:, sl], start=True, stop=False)
                        nc.tensor.matmul(pf[:, po], wsb["w6bT"][:, obs],
                                         xcat_b[:, sl], start=False, stop=True)
                    h6 = gp.tile([128, 1024], f32, tag="h6", name="h6", bufs=2)
                    nc.scalar.activation(h6, pf, AF.Prelu,
                                         bias=wsb["c6v"][:, ob : ob + 1],
                                         alpha=0.2)
                    xgt = xgts[ob]
                    nc.vector.reduce_max(xgt[:, jp : jp + 1], h6,
                                         axis=mybir.AxisListType.X)

                xgts = [tkp.tile([128, 2], f32, tag="xgt%d" % ob,
                                 name="xgt%d" % ob) for ob in range(8)]

                # ================= layer 1 lead-in =================
                for blk in range(4):
                    prep_block1(blk * 1024)
                    for t in range(8):
                        dist_quarter(1, t, blk)
                for t in range(8):
                    peel_compact(t)
                reformat(0)

                # ================= the three layers =================
                phase_B(1)
                phase_C(1)
                phase_A(2)
                phase_B(2)
                phase_C(2)
                phase_A(3)
                phase_B(3)
                phase_C(3)

                # ============ conv6 second half + global max pool ============
                for ob in range(8):
                    conv6_chunk(ob, 1)
                    nc.vector.reduce_max(xg_sb[:, ob : ob + 1], xgts[ob],
                                         axis=mybir.AxisListType.X)
                ccg_i = drp.tile([128, 8], f32, tag="cgi", name="ccg_in")
                nc.scalar.dma_start(ccg_i, xg_sb)
                ccg_o = drp.tile([256, 8], f32, tag="cgo", name="ccg_out")
                nc.gpsimd.collective_compute(
                    "AllGather", ALU.bypass, replica_groups=PAIRS,
                    ins=[ccg_i], outs=[ccg_o],
                )

            # layer scratch pools released here; final stage below.
            with tc.tile_pool(name="fin", bufs=1) as fin:
                # conv7 local part without bias (overlaps the AllReduce)
                h7 = fin.tile([128, 4 * H], bf, tag="h7", name="h7")
                for ob in range(4):
                    obs = slice(ob * 128, (ob + 1) * 128)
                    for jp in range(2):
                        pf = psd.tile([128, 1024], f32, tag="d", name="pf7")
                        for q in range(2):
                            sl = slice(jp * 1024 + q * 512,
                                       jp * 1024 + (q + 1) * 512)
                            po = slice(q * 512, (q + 1) * 512)
                            nc.tensor.matmul(pf[:, po], wsb["w7laT"][:, obs],
                                             xcat_a[:, sl],
                                             start=True, stop=False)
                            nc.tensor.matmul(pf[:, po], wsb["w7lbT"][:, obs],
                                             xcat_b[:, sl],
                                             start=False, stop=True)
                        nc.scalar.copy(
                            h7[:, ob * H + jp * 1024 : ob * H + (jp + 1) * 1024],
                            pf,
                        )

                # max of the pair's two xg halves (cheaper than AllReduce,
                # which carries a 1.875x cost multiplier)
                nc.sync.dma_start(xg_sb, ccg_o[0:128])
                xgp = fin.tile([128, 8], f32, tag="xgp", name="xgp")
                nc.sync.dma_start(xgp, ccg_o[128:256])
                nc.vector.tensor_max(xg_sb, xg_sb, xgp)

                # conv7 effective bias: c7 + W7g @ xg
                for ob in range(4):
                    pb = psd.tile([128, 1024], f32, tag="d", name="pb7")
                    for kb in range(8):
                        nc.tensor.matmul(
                            pb[:, 0:1],
                            wsb["w7gT"][:, kb, ob * 128 : (ob + 1) * 128],
                            xg_sb[:, kb : kb + 1],
                            start=(kb == 0),
                            stop=(kb == 7),
                        )
                    nc.scalar.activation(b7_sb[:, ob : ob + 1], pb[:, 0:1],
                                         AF.Identity,
                                         bias=wsb["c7v"][:, ob : ob + 1])

                h7a = fin.tile([128, 4 * H], bf, tag="h7a", name="h7a")
                for jp in range(4):
                    for ob in range(4):
                        sl = slice(ob * H + jp * 512, ob * H + (jp + 1) * 512)
                        nc.scalar.activation(
                            h7a[:, sl], h7[:, sl], AF.Prelu,
                            bias=b7_sb[:, ob : ob + 1], alpha=0.2,
                        )
                h8 = fin.tile([128, 2 * H], bf, tag="h8", name="h8")
                o_sb = fin.tile([13, H], f32, tag="osb", name="o_sb")
                for jp in range(2):
                    for ob in range(2):
                        pf = psd.tile([128, 1024], f32, tag="d", name="pf8")
                        for q in range(2):
                            po = slice(q * 512, (q + 1) * 512)
                            co = jp * 1024 + q * 512
                            for kb in range(4):
                                nc.tensor.matmul(
                                    pf[:, po],
                                    wsb["w8T"][:, kb, ob * 128 : (ob + 1) * 128],
                                    h7a[:, kb * H + co : kb * H + co + 512],
                                    start=(kb == 0),
                                    stop=(kb == 3),
                                )
                        nc.scalar.activation(
                            h8[:, ob * H + jp * 1024 : ob * H + (jp + 1) * 1024],
                            pf, AF.Prelu, bias=wsb["c8v"][:, ob : ob + 1],
                            alpha=0.2,
                        )
                    pf = psd.tile([128, 1024], f32, tag="d", name="pf9")
                    for q in range(2):
                        po = slice(q * 512, (q + 1) * 512)
                        co = jp * 1024 + q * 512
                        for kb in range(2):
                            nc.tensor.matmul(
                                pf[0:13, po],
                                wsb["w9T"][:, kb, :],
                                h8[:, kb * H + co : kb * H + co + 512],
                                start=(kb == 0),
                                stop=(kb == 1),
                            )
                    nc.scalar.activation(
                        o_sb[:, jp * 1024 : (jp + 1) * 1024], pf[0:13],
                        AF.Identity, bias=wsb["b9v"],
                    )
                nc.sync.dma_start(out_d[:], o_sb)

    nc.compile()
    return nc


def _mix(a_steps, b_steps, lead=0):
    """Emit `lead` a-steps, then alternate a/b 1:1 until b is exhausted, then
    the remaining a-steps."""
    ia = 0
    for _ in range(min(lead, len(a_steps))):
        a_steps[ia]()
        ia += 1
    for ib in range(len(b_steps)):
        if ia < len(a_steps):
            a_steps[ia]()
            ia += 1
        b_steps[ib]()
    while ia < len(a_steps):
        a_steps[ia]()
        ia += 1


def make_in_maps(inputs):
    """Per-core input dicts from the full problem inputs."""
    wd = _prep_weights(inputs)
    wpack32 = np.zeros((128, _wcols(0)), dtype=np.float32)
    import ml_dtypes

    wpack16 = np.zeros((128, _wcols(1)), dtype=ml_dtypes.bfloat16)
    col = {0: 0, 1: 0}
    for name, shape, isb in _WEIGHT_SPECS:
        w = int(np.prod(shape[1:])) if len(shape) > 1 else 1
        dst = wpack16 if isb else wpack32
        v = wd[name].reshape(shape[0], w)
        dst[0 : shape[0], col[isb] : col[isb] + w] = (
            _bf16(v) if isb else v.astype(np.float32)
        )
        col[isb] += w
    pts = np.asarray(inputs["points"], dtype=np.float32)
    in_maps = []
    for c in range(8):
        b, h = c // 2, c % 2
        m = {"wpack32": wpack32, "wpack16": wpack16}
        m["wboot"] = np.ascontiguousarray(
            np.concatenate([wd["u1T"], wd["ones3"]], axis=1).astype(np.float32))
        m["choff"] = np.ascontiguousarray(
            np.tile(np.repeat(np.arange(8, dtype=np.uint16) * 512, 8), (128, 1)))
        m["pts_full"] = np.ascontiguousarray(pts[b])
        pm = np.ascontiguousarray(pts[b][:, h * H : (h + 1) * H])
        m["pts_mine"] = pm
        m["pts_mine_bf"] = _bf16(pm)
        in_maps.append(m)
    return in_maps


def kernel(**inputs):
    from concourse.bass_utils import run_bass_kernel_spmd

    if "nc" not in _CACHE:
        _CACHE["nc"] = build_kernel()
    nc = _CACHE["nc"]
    in_maps = make_in_maps(inputs)
    res = run_bass_kernel_spmd(nc, in_maps, core_ids=list(range(8)))
    out = np.zeros((B, 13, N), dtype=np.float32)
    for c in range(8):
        b, h = c // 2, c % 2
        out[b][:, h * H : (h + 1) * H] = res.results[c]["out"]
    return out


# revision 55
# speedup vs baseline: 1.8193x; 1.0095x over previous
"""DGCNN part-segmentation forward pass on 8 Trainium2 NeuronCores.

Sharding: data-parallel over the batch (B=4) x 2-way split of the N=4096
points within each batch element.  Core c handles batch element c//2,
point rows [(c%2)*2048, (c%2+1)*2048).  The two cores of a pair exchange
their half of each EdgeConv output with pairwise AllGathers (one per
1024-point group, launched as soon as that group's output is ready) and
a pairwise AllReduce-max for the global pooling feature.

Precision: the feature path (EdgeConv transforms, convs 6-9) runs in
bfloat16 with fp32 PSUM accumulation.  Layer 1's kNN runs on raw fp32
points (3-D points have many near-ties; bf16 inputs measurably flip
neighbors), via a small separate fp32 lhs/rhs pair.  Layers 2-3 rank
neighbors of the bf16 features exactly: the distance matmul contracts
66 bf16 rows = 64 feature channels + a hi/lo bf16 pair carrying the
fp32 -|x_j|^2 row at ~16-bit precision.  The sq rows travel inside the
AllGather so no distance prep remains on the post-exchange critical
path.

The emission order is software-pipelined: engine queues are in-order,
so each phase leads with a few distance tiles, then runs the edge-conv
phase solid (its AllGather fires early), then the bulk of the
distance/top-k scans; the next layer's first distance quarters overlap
the current layer's second edge phase.

Per EdgeConv layer:
  top-20: per 128-row tile: 8x top-8 per 512-column chunk (DVE Max) ->
          64 candidates; 3 peeling rounds (max8 + match_replace) ->
          top-24 values; threshold mask + prefix-sum compaction +
          gpsimd local_scatter -> global indices of the top-20
  gather: batched GPSIMD ap_gather of the bf16 u = U x transform; the
          first call covers one neighbor slot so the edge pipeline
          starts right after the peels, later calls cover 4 slots
  edge  : psum = I*u_gathered + V*x_i (bf16 matmuls), LeakyReLU (Prelu)
          on the scalar engine, conv2 bf16 matmul; the k-max runs on
          the raw conv2 psums (LeakyReLU is monotone) and one Prelu
          finalizes each group.
"""

import sys

sys.path.insert(0, "/opt/trn_rl_repo")

import numpy as np

B = 4
N = 4096
H = 2048  # points per core (half of a batch element)
KNN = 20
EPS = 1e-5
NEG = -3.0e38
G = KNN * 64  # widx columns per point group

_CACHE = {}


# --------------------------------------------------------------------------
# host-side weight preparation
# --------------------------------------------------------------------------

def _bf16(x):
    import ml_dtypes

    return np.ascontiguousarray(x, dtype=np.float32).astype(ml_dtypes.bfloat16)


def _fold_bn(w, b, g, be):
    s = (g / np.sqrt(np.float32(1.0) + np.float32(EPS))).astype(np.float32)
    return (w * s[:, None]).astype(np.float32), (s * b + be).astype(np.float32)


def _prep_weights(inp):
    f = np.float32
    W, C = {}, {}
    for i in range(1, 9):
        W[i], C[i] = _fold_bn(
            inp["w%d" % i], inp["b%d" % i], inp["g%d" % i], inp["be%d" % i]
        )
    d = {}
    # edge conv layers: split into U (neighbor part) and V (center part).
    # Layer 1's x2r tile holds raw points, layers 2-3 hold 2*x, so the V
    # transpose is halved only for layers 2-3.
    for lid, wi, vscale in ((1, 1, 1.0), (2, 3, 0.5), (3, 5, 0.5)):
        w = W[wi]
        cin = w.shape[1] // 2
        U = w[:, :cin]
        V = w[:, cin:] - w[:, :cin]
        d["u%dT" % lid] = np.ascontiguousarray(U.T)
        d["v%dTh" % lid] = np.ascontiguousarray((V * f(vscale)).T)
        d["c%d" % lid] = C[wi].reshape(64, 1)
    d["w2T"] = np.ascontiguousarray(W[2].T)
    d["cc2"] = C[2].reshape(64, 1)
    d["w4T"] = np.ascontiguousarray(W[4].T)
    d["cc4"] = C[4].reshape(64, 1)
    # conv6 (192 -> 1024)
    w6T = np.ascontiguousarray(W[6].T)  # (192, 1024)
    d["w6aT"] = np.ascontiguousarray(w6T[:128])
    d["w6bT"] = np.ascontiguousarray(w6T[128:])
    d["c6v"] = np.ascontiguousarray(C[6].reshape(8, 128).T)  # (128, 8)
    # conv7 (1216 -> 512): xg part (1024) and local part (192)
    w7 = W[7]
    w7gT = np.ascontiguousarray(w7[:, :1024].T)  # (1024, 512)
    d["w7gT"] = np.ascontiguousarray(w7gT.reshape(8, 128, 512).transpose(1, 0, 2))
    w7lT = np.ascontiguousarray(w7[:, 1024:].T)  # (192, 512)
    d["w7laT"] = np.ascontiguousarray(w7lT[:128])
    d["w7lbT"] = np.ascontiguousarray(w7lT[128:])
    d["c7v"] = np.ascontiguousarray(C[7].reshape(4, 128).T)  # (128, 4)
    # conv8 (512 -> 256)
    w8T = np.ascontiguousarray(W[8].T)  # (512, 256)
    d["w8T"] = np.ascontiguousarray(w8T.reshape(4, 128, 256).transpose(1, 0, 2))
    d["c8v"] = np.ascontiguousarray(C[8].reshape(2, 128).T)  # (128, 2)
    # conv9 (256 -> 13), plain linear
    w9T = np.ascontiguousarray(inp["w9"].T.astype(f))  # (256, 13)
    d["w9T"] = np.ascontiguousarray(w9T.reshape(2, 128, 13).transpose(1, 0, 2))
    d["b9v"] = inp["b9"].astype(f).reshape(13, 1)
    # constants
    d["id64"] = np.eye(64, dtype=f)
    d["ones3"] = np.ones((3, 1), dtype=f)
    d["ones64"] = np.ones((64, 1), dtype=f)
    return d


# name -> (shape, is_bf16)
_WEIGHT_SPECS = [
    ("u1T", (3, 64), 0), ("v1Th", (3, 64), 1), ("c1", (64, 1), 0),
    ("w2T", (64, 64), 1), ("cc2", (64, 1), 0),
    ("u2T", (64, 64), 1), ("v2Th", (64, 64), 1), ("c2", (64, 1), 0),
    ("w4T", (64, 64), 1), ("cc4", (64, 1), 0),
    ("u3T", (64, 64), 1), ("v3Th", (64, 64), 1), ("c3", (64, 1), 0),
    ("w6aT", (128, 1024), 1), ("w6bT", (64, 1024), 1), ("c6v", (128, 8), 0),
    ("w7gT", (128, 8, 512), 0), ("w7laT", (128, 512), 1),
    ("w7lbT", (64, 512), 1), ("c7v", (128, 4), 0),
    ("w8T", (128, 4, 256), 1), ("c8v", (128, 2), 0),
    ("w9T", (128, 2, 13), 1), ("b9v", (13, 1), 0),
    ("id64", (64, 64), 0), ("ones3", (3, 1), 0), ("ones64", (64, 1), 0),
]


def _wcols(bf):
    return sum(int(np.prod(s[1:])) if len(s) > 1 else 1
               for _, s, isb in _WEIGHT_SPECS if isb == bf)


# --------------------------------------------------------------------------
# device kernel builder
# --------------------------------------------------------------------------

def build_kernel():
    import concourse.bacc as bacc
    import concourse.mybir as mybir
    import concourse.tile as tile

    f32 = mybir.dt.float32
    bf = mybir.dt.bfloat16
    i16 = mybir.dt.int16
    u16 = mybir.dt.uint16
    AF = mybir.ActivationFunctionType
    ALU = mybir.AluOpType
    PAIRS = [[0, 1], [2, 3], [4, 5], [6, 7]]

    nc = bacc.Bacc("TRN2", target_bir_lowering=False, num_devices=8)

    pts_full = nc.dram_tensor("pts_full", [3, N], f32, kind="ExternalInput")
    pts_mine = nc.dram_tensor("pts_mine", [3, H], f32, kind="ExternalInput")
    pts_mine_bf = nc.dram_tensor("pts_mine_bf", [3, H], bf,
                                 kind="ExternalInput")
    choff_d = nc.dram_tensor("choff", [128, 64], u16, kind="ExternalInput")
    wboot_d = nc.dram_tensor("wboot", [3, 65], f32, kind="ExternalInput")
    wpack32_d = nc.dram_tensor("wpack32", [128, _wcols(0)], f32,
                               kind="ExternalInput")
    wpack16_d = nc.dram_tensor("wpack16", [128, _wcols(1)], bf,
                               kind="ExternalInput")
    out_d = nc.dram_tensor("out", [13, H], f32, kind="ExternalOutput")

    with tile.TileContext(nc) as tc:
        with (
            tc.tile_pool(name="wp", bufs=1) as wp,
            tc.tile_pool(name="per", bufs=1) as per,
            tc.tile_pool(name="psd", bufs=2, space="PSUM") as psd,
            tc.tile_pool(name="pse", bufs=2, space="PSUM") as pse,
            tc.tile_pool(name="dram", bufs=1, space="DRAM") as drp,
        ):
            # ---- persistent tiles ----
            rhsF = per.tile([66, N], bf, tag="rhsF", name="rhsF")
            rhsF1 = per.tile([4, N], f32, tag="rhsF1", name="rhsF1")
            lhsTm = per.tile([66, H], bf, tag="lhsTm", name="lhsTm")
            lhsTm1 = per.tile([4, H], f32, tag="lhsTm1", name="lhsTm1")
            u_t = per.tile([64, N], f32, tag="u", name="u_t")
            x2r = per.tile([64, H], bf, tag="x2r", name="x2r")
            xcat_a = per.tile([128, H], bf, tag="xcata", name="xcat_a")
            xcat_b = per.tile([64, H], bf, tag="xcatb", name="xcat_b")
            widx = per.tile([64, KNN * 128], i16, tag="widx", name="widx")
            acc3 = per.tile([64, H], f32, tag="acc3", name="acc3")
            xg_sb = per.tile([128, 8], f32, tag="xg", name="xg_sb")
            b7_sb = per.tile([128, 4], f32, tag="b7", name="b7_sb")
            choff_sb = per.tile([128, 64], u16, tag="choff", name="choff_sb")
            zero128 = per.tile([128, 64], f32, tag="z128", name="zero128")

            idx_dram = drp.tile([H, KNN], i16, tag="idxd", name="idx_dram")

            x1h = xcat_a[0:64]
            x2h = xcat_a[64:128]
            x3h = xcat_b

            # critical inputs first; the row-3 ones of lhsTm1 come from a
            # 32-aligned memset that the pts DMA then partially overwrites
            wbt = wp.tile([3, 65], f32, tag="wb", name="wboot_sb")
            nc.sync.dma_start(wbt, wboot_d[:])
            nc.vector.memset(lhsTm1[0:4], 1.0)
            nc.sync.dma_start(lhsTm1[0:3], pts_mine[:])
            nc.sync.dma_start(x2r[0:3], pts_mine_bf[:])
            nc.sync.dma_start(choff_sb, choff_d[:])
            nc.vector.memset(zero128, 0.0)
            nc.vector.memset(lhsTm[64:66], 1.0)
            # all weights arrive in two packed DMAs; per-weight tiles are
            # views into the packed tiles
            wpt32 = wp.tile([128, _wcols(0)], f32, tag="wp32", name="wp32")
            nc.scalar.dma_start(wpt32, wpack32_d[:])
            wpt16 = wp.tile([128, _wcols(1)], bf, tag="wp16", name="wp16")
            nc.scalar.dma_start(wpt16, wpack16_d[:])
            wsb = {}
            col = {0: 0, 1: 0}
            for name, shape, isb in _WEIGHT_SPECS:
                w = int(np.prod(shape[1:])) if len(shape) > 1 else 1
                src = wpt16 if isb else wpt32
                v = src[0 : shape[0], col[isb] : col[isb] + w]
                if len(shape) == 3:
                    v = v.rearrange("p (a b) -> p a b", a=shape[1])
                wsb[name] = v
                col[isb] += w
            wsb["u1T"] = wbt[0:3, 0:64]
            wsb["ones3"] = wbt[0:3, 64:65]

            LAYER = {
                1: dict(cin=3, uT="u1T", vTh="v1Th", c="c1", w2="w2T",
                        cc="cc2"),
                2: dict(cin=64, uT="u2T", vTh="v2Th", c="c2", w2="w4T",
                        cc="cc4"),
                3: dict(cin=64, uT="u3T", vTh="v3Th", c="c3", w2=None,
                        cc=None),
            }
            XOUT = {1: x1h, 2: x2h, 3: x3h}

            with (
                tc.tile_pool(name="dsb", bufs=4) as dsbp,
                tc.tile_pool(name="pp", bufs=2) as ppl,
                tc.tile_pool(name="tk", bufs=2) as tkp,
                tc.tile_pool(name="gp", bufs=2) as gp,
                tc.tile_pool(name="yp", bufs=3) as ypl,
                tc.tile_pool(name="za", bufs=1) as zap,
            ):
                cands = {}

                def prep_block1(lo):
                    """layer-1 lead: rhsF1 = [2*pts; -|pts|^2] and
                    u_t = u1T.T @ pts for columns [lo, lo+1024)."""
                    stage = ppl.tile([3, 1024], f32, tag="pts", name="stage")
                    nc.sync.dma_start(stage, pts_full[:, lo : lo + 1024])
                    nc.scalar.mul(rhsF1[0:3, lo : lo + 1024], stage, 2.0)
                    xsq = ppl.tile([64, 1024], f32, tag="xsq", name="xsq1")
                    nc.scalar.activation(xsq[0:3], stage, AF.Square)
                    sqrow = ppl.tile([1, 1024], f32, tag="sqr", name="sqrow1")
                    for j in range(2):
                        jl = slice(j * 512, (j + 1) * 512)
                        pq = psd.tile([128, 1024], f32, tag="d", name="pq")
                        nc.tensor.matmul(pq[0:1, 0:512], wsb["ones3"],
                                         xsq[0:3, jl])
                        nc.scalar.mul(sqrow[:, jl], pq[0:1, 0:512], -1.0)
                        pu = psd.tile([128, 1024], f32, tag="d", name="pu")
                        nc.tensor.matmul(pu[0:64, 0:512], wsb["u1T"],
                                         stage[:, jl])
                        nc.scalar.copy(u_t[:, lo + j * 512 : lo + (j + 1) * 512],
                                       pu[0:64, 0:512])
                    nc.sync.dma_start(rhsF1[3:4, lo : lo + 1024], sqrow)

                def prep_block(lid, lo):
                    """u_t[:, lo:lo+1024] = uT.T @ rhsF[0:cin] (bf16)."""
                    P = LAYER[lid]
                    cin = P["cin"]
                    for j in range(2):
                        sl = slice(lo + j * 512, lo + (j + 1) * 512)
                        pu = psd.tile([128, 1024], f32, tag="d", name="pu")
                        nc.tensor.matmul(pu[0:64, 0:512], wsb[P["uT"]],
                                         rhsF[0:cin, sl])
                        nc.scalar.copy(u_t[:, sl], pu[0:64, 0:512])

                def dist_quarter(lid, t, blk):
                    """distance psum + copy + top8 scan for tile t, column
                    block blk (1024 cols), ranking for layer lid."""
                    if t not in cands:
                        cands[t] = (
                            tkp.tile([128, 64], f32, tag="cand", name="cand",
                                     bufs=12),
                            tkp.tile([128, 64], u16, tag="cidx", name="cidx",
                                     bufs=12),
                        )
                    cand, cidx = cands[t]
                    lo = blk * 1024
                    pd = psd.tile([128, 1024], f32, tag="d", name="pd")
                    for q in range(2):
                        cs = slice(lo + q * 512, lo + (q + 1) * 512)
                        if lid == 1:
                            nc.tensor.matmul(
                                pd[:, q * 512 : (q + 1) * 512],
                                lhsTm1[:, t * 128 : (t + 1) * 128],
                                rhsF1[:, cs],
                            )
                        else:
                            nc.tensor.matmul(
                                pd[:, q * 512 : (q + 1) * 512],
                                lhsTm[:, t * 128 : (t + 1) * 128],
                                rhsF[:, cs],
                            )
                    dsb = dsbp.tile([128, 1024], f32, tag="dsb", name="dsb")
                    nc.scalar.copy(dsb, pd)
                    for q in range(2):
                        cc = blk * 2 + q
                        nc.vector.max(
                            out=cand[:, cc * 8 : (cc + 1) * 8],
                            in_=dsb[:, q * 512 : (q + 1) * 512],
                        )
                        nc.vector.max_index(
                            cidx[:, cc * 8 : (cc + 1) * 8],
                            cand[:, cc * 8 : (cc + 1) * 8],
                            dsb[:, q * 512 : (q + 1) * 512],
                        )

                def peel_compact(t):
                    """top-20 selection for tile t from its 64 candidates."""
                    cand, cidx = cands.pop(t)
                    nc.vector.tensor_add(cidx, cidx, choff_sb)
                    candw = tkp.tile([128, 64], f32, tag="candw", name="candw")
                    nc.vector.tensor_copy(candw, cand)
                    t8 = tkp.tile([128, 24], f32, tag="t8", name="t8")
                    nc.vector.max(out=t8[:, 0:8], in_=candw)
                    nc.vector.match_replace(out=candw, in_to_replace=t8[:, 0:8],
                                            in_values=candw, imm_value=NEG)
                    nc.vector.max(out=t8[:, 8:16], in_=candw)
                    nc.vector.match_replace(out=candw,
                                            in_to_replace=t8[:, 8:16],
                                            in_values=candw, imm_value=NEG)
                    nc.vector.max(out=t8[:, 16:24], in_=candw)
                    mask = tkp.tile([128, 64], f32, tag="mask", name="mask")
                    nc.vector.tensor_scalar(mask, cand, t8[:, 19:20], None,
                                            op0=ALU.is_ge)
                    cums = tkp.tile([128, 64], f32, tag="cums", name="cums")
                    nc.vector.tensor_tensor_scan(cums, mask, zero128, 0.0,
                                                 op0=ALU.add, op1=ALU.add)
                    # slot = cums*mask - 1  (-1 marks non-selected: ignored
                    # by local_scatter)
                    nc.vector.tensor_mul(cums, cums, mask)
                    nc.vector.tensor_scalar_add(cums, cums, -1.0)
                    slot = tkp.tile([128, 64], i16, tag="slot", name="slot")
                    nc.vector.tensor_copy(slot, cums)
                    sel = tkp.tile([128, 24], u16, tag="sel", name="sel")
                    nc.gpsimd.local_scatter(out_ap=sel, data_ap=cidx,
                                            idxs_ap=slot, channels=128,
                                            num_elems=24, num_idxs=64)
                    nc.sync.dma_start(
                        idx_dram[t * 128 : (t + 1) * 128, :],
                        sel[:, 0:KNN].bitcast(i16),
                    )

                # gather calls: (k0, nk, widx column base within the group).
                # The first call covers a single neighbor slot so the edge
                # pipeline starts as soon as possible after the peels.
                CALLS = [(0, 1, 0), (1, 4, 64), (5, 4, 320), (9, 4, 576),
                         (13, 4, 832), (17, 3, 1088)]

                def reformat(grp):
                    """widx[p, grp*G + base + r*nk + kk]
                         = idx_dram[1024*grp + 16*r + p, k0 + kk],
                    replicated over the four 16-partition groups; one DMA per
                    (gather call, replica) so the first call's indices land
                    first and later transfers overlap the gathers."""
                    for k0, nk, base in CALLS:
                        src = idx_dram[grp * 1024 : (grp + 1) * 1024,
                                       k0 : k0 + nk].rearrange(
                            "(r p) k -> p r k", p=16
                        )
                        for rep in range(4):
                            dst = widx[rep * 16 : (rep + 1) * 16,
                                       grp * G + base
                                       : grp * G + base + 64 * nk].rearrange(
                                "p (r k) -> p r k", k=nk
                            )
                            nc.sync.dma_start(dst, src)

                def gather_call(grp, call):
                    k0, nk, base = CALLS[call]
                    g = gp.tile([64, 4096], f32, tag="g", name="g")
                    nc.gpsimd.ap_gather(
                        out_ap=g[:, 0 : 1024 * nk],
                        in_ap=u_t,
                        idxs_ap=widx[:, grp * G + base
                                     : grp * G + base + 64 * nk],
                        channels=64,
                        num_elems=N,
                        d=1,
                        num_idxs=1024 * nk,
                    )
                    # columns are (r, kk)-interleaved: col = nk*16*r + 16*kk + p
                    return g[:, 0 : 1024 * nk].rearrange(
                        "c (r f p) -> c f r p", f=nk, p=16
                    )

                def edge_k(lid, grp, gv, kk, k, zacc):
                    """one neighbor slot: y = Prelu(I u_j + V x_i + c);
                    z psum = W2 y; zacc = max(zacc, z).  Layer 3 needs only
                    the running max of the gathered u."""
                    P = LAYER[lid]
                    cin = P["cin"]
                    gk = gv[:, kk]  # (64, 64, 16): r-major, p-minor
                    if lid == 3:
                        gsl = slice(grp * 1024, (grp + 1) * 1024)
                        a3 = acc3[:, gsl].rearrange("c (r p) -> c r p", p=16)
                        if k == 0:
                            nc.vector.tensor_copy(a3, gk)
                        else:
                            nc.vector.tensor_max(a3, a3, gk)
                        return
                    for q in range(2):
                        rsl = slice(q * 32, (q + 1) * 32)
                        csl = slice(q * 512, (q + 1) * 512)
                        msl = slice(grp * 1024 + q * 512,
                                    grp * 1024 + (q + 1) * 512)
                        yp = pse.tile([64, 512], f32, tag="e", name="yp")
                        nc.tensor.matmul(yp, wsb["id64"], gk[:, rsl],
                                         start=True, stop=False)
                        nc.tensor.matmul(yp, wsb[P["vTh"]], x2r[0:cin, msl],
                                         start=False, stop=True)
                        y = ypl.tile([64, 512], bf, tag="y", name="y")
                        nc.scalar.activation(y, yp, AF.Prelu, bias=wsb[P["c"]],
                                             alpha=0.2)
                        zp = pse.tile([64, 512], f32, tag="c2", name="zp")
                        nc.tensor.matmul(zp, wsb[P["w2"]], y)
                        if k == 0:
                            nc.vector.tensor_copy(zacc[:, csl], zp)
                        else:
                            nc.vector.tensor_max(zacc[:, csl], zacc[:, csl],
                                                 zp)

                def edge_fin(lid, grp, zacc):
                    P = LAYER[lid]
                    gsl = slice(grp * 1024, (grp + 1) * 1024)
                    x_out = XOUT[lid]
                    if lid == 3:
                        for q in range(2):
                            sl = slice(grp * 1024 + q * 512,
                                       grp * 1024 + (q + 1) * 512)
                            pe_ = pse.tile([64, 512], f32, tag="e", name="pe3")
                            nc.tensor.matmul(pe_, wsb["id64"], acc3[:, sl],
                                             start=True, stop=False)
                            nc.tensor.matmul(pe_, wsb[P["vTh"]],
                                             x2r[0 : P["cin"], sl],
                                             start=False, stop=True)
                            nc.scalar.activation(x_out[:, sl], pe_, AF.Prelu,
                                                 bias=wsb[P["c"]], alpha=0.2)
                    else:
                        nc.scalar.activation(x_out[:, gsl], zacc, AF.Prelu,
                                             bias=wsb[P["cc"]], alpha=0.2)

                ccs = {}

                def exchange_start(lid, grp):
                    """export x_out group plus the hi/lo bf16 pair carrying
                    its fp32 -|x|^2 row, and launch the pairwise AllGather."""
                    gsl = slice(grp * 1024, (grp + 1) * 1024)
                    x_half = XOUT[lid]
                    xsq = ppl.tile([64, 1024], f32, tag="xsq", name="xsqe")
                    nc.scalar.activation(xsq, x_half[:, gsl], AF.Square)
                    sqrow = ppl.tile([1, 1024], f32, tag="sqr", name="sqrowe")
                    for j in range(2):
                        jl = slice(j * 512, (j + 1) * 512)
                        pq = psd.tile([128, 1024], f32, tag="d", name="pqe")
                        nc.tensor.matmul(pq[0:1, 0:512], wsb["ones64"],
                                         xsq[:, jl])
                        nc.scalar.mul(sqrow[:, jl], pq[0:1, 0:512], -1.0)
                    hi = ppl.tile([1, 1024], bf, tag="hi", name="hi")
                    nc.scalar.copy(hi, sqrow)
                    lo = ppl.tile([1, 1024], bf, tag="lo", name="lo")
                    nc.vector.tensor_tensor(lo, sqrow, hi, op=ALU.subtract)
                    ccin = drp.tile([66, 1024], bf, tag="cci%d" % grp,
                                    name="ccin%d" % grp)
                    nc.scalar.dma_start(ccin[0:64], x_half[:, gsl])
                    nc.scalar.dma_start(ccin[64:65], hi)
                    nc.scalar.dma_start(ccin[65:66], lo)
                    ccout = drp.tile([132, 1024], bf, tag="cco%d" % grp,
                                     name="ccout%d" % grp)
                    nc.gpsimd.collective_compute(
                        "AllGather", ALU.bypass, replica_groups=PAIRS,
                        ins=[ccin], outs=[ccout],
                    )
                    return ccout

                def exchange_finish(ccout, grp):
                    lo0 = grp * 1024
                    lo1 = 2048 + grp * 1024
                    nc.sync.dma_start(rhsF[0:64, lo0 : lo0 + 1024], ccout[0:64])
                    nc.sync.dma_start(rhsF[64:66, lo0 : lo0 + 1024],
                                      ccout[64:66])
                    nc.sync.dma_start(rhsF[0:64, lo1 : lo1 + 1024],
                                      ccout[66:130])
                    nc.sync.dma_start(rhsF[64:66, lo1 : lo1 + 1024],
                                      ccout[130:132])

                def lhs_prep(lid, grp):
                    """lhsTm/x2r columns for group grp from layer lid's out."""
                    gsl = slice(grp * 1024, (grp + 1) * 1024)
                    x_out = XOUT[lid]
                    nc.scalar.mul(lhsTm[0:64, gsl], x_out[:, gsl], 2.0)
                    nc.scalar.mul(x2r[0:64, gsl], x_out[:, gsl], 2.0)

                def edge_steps(lid, grp):
                    """emission steps for the edge phase of (lid, grp); the
                    final step also launches the AllGather for layers 1-2."""
                    steps = []
                    state = {}
                    zacc = None
                    if lid != 3:
                        zacc = zap.tile([64, 1024], f32, tag="za", name="zacc")

                    def mk_gather(call):
                        def f():
                            state[call] = gather_call(grp, call)
                        return f

                    def mk_k(call, kk, k):
                        def f():
                            edge_k(lid, grp, state[call], kk, k, zacc)
                        return f

                    def fin():
                        edge_fin(lid, grp, zacc)
                        if lid < 3:
                            ccs[grp] = exchange_start(lid, grp)
                            lhs_prep(lid, grp)

                    for call, (k0, nk, base) in enumerate(CALLS):
                        steps.append(mk_gather(call))
                        for kk in range(nk):
                            steps.append(mk_k(call, kk, k0 + kk))
                    steps.append(fin)
                    return steps

                carry = []

                def phase_B(lid):
                    """grp0 edge (leading, so its AllGather fires early)
                    followed by grp1 tiles' dist/topk."""
                    a = []
                    for t in range(8, 16):
                        for blk in range(4):
                            a.append(lambda t=t, blk=blk:
                                     dist_quarter(lid, t, blk))
                        a.append(lambda t=t: peel_compact(t))
                    for f in a[:12]:
                        f()
                    for f in edge_steps(lid, 0):
                        f()
                    for f in a[12:]:
                        f()
                    reformat(1)

                def phase_C(lid):
                    """grp1 edge followed by the next layer's lead work (or
                    conv6's first half for layer 3)."""
                    nxt = lid + 1 if lid < 3 else None
                    if nxt is not None:
                        exchange_finish(ccs[0], 0)
                    es = edge_steps(lid, 1)
                    if nxt is None:
                        _mix([lambda ob=ob: conv6_chunk(ob, 0)
                              for ob in range(8)], es)
                        return
                    # lead with a few next-layer dist quarters: rhsF for the
                    # g0 blocks landed mid-phase-B, so these are ready and
                    # cover the reformat+gather spin-up of the g1 edge
                    for t in range(5):
                        dist_quarter(nxt, t, 0)
                    for f in es:
                        f()
                    prep_block(nxt, 0)
                    prep_block(nxt, 2048)
                    for t in range(5, 8):
                        dist_quarter(nxt, t, 0)
                    for t in range(3):
                        dist_quarter(nxt, t, 2)
                    for t in range(3, 8):
                        carry.append(lambda t=t: dist_quarter(nxt, t, 2))

                def phase_A(lid):
                    """finish the leftover and second-half dist quarters of
                    tiles 0..8 for layer lid, then peel."""
                    exchange_finish(ccs[1], 1)
                    for f in carry:
                        f()
                    carry.clear()
                    prep_block(lid, 1024)
                    prep_block(lid, 3072)
                    for t in range(8):
                        dist_quarter(lid, t, 1)
                        dist_quarter(lid, t, 3)
                    for t in range(8):
                        peel_compact(t)
                    reformat(0)

                def conv6_chunk(ob, jp):
                    obs = slice(ob * 128, (ob + 1) * 128)
                    pf = psd.tile([128, 1024], f32, tag="d", name="pf6")
                    for q in range(2):
                        sl = slice(jp * 1024 + q * 512,
                                   jp * 1024 + (q + 1) * 512)
                        po = slice(q * 512, (q + 1) * 512)
                        nc.tensor.matmul(pf[:, po], wsb["w6aT"][:, obs],
                                         xcat_a[:, sl], start=True, stop=False)
                        nc.tensor.matmul(pf[:, po], wsb["w6bT"][:, obs],
                                         xcat_b[:, sl], start=False, stop=True)
                    h6 = gp.tile([128, 1024], f32, tag="h6", name="h6", bufs=2)
                    nc.scalar.activation(h6, pf, AF.Prelu,
                                         bias=wsb["c6v"][:, ob : ob + 1],
                                         alpha=0.2)
                    xgt = xgts[ob]
                    nc.vector.reduce_max(xgt[:, jp : jp + 1], h6,
                                         axis=mybir.AxisListType.X)

                xgts = [tkp.tile([128, 2], f32, tag="xgt%d" % ob,
                                 name="xgt%d" % ob) for ob in range(8)]

                # ================= layer 1 lead-in =================
                for blk in range(4):
                    prep_block1(blk * 1024)
                    for t in range(8):
                        dist_quarter(1, t, blk)
                for t in range(8):
                    peel_compact(t)
                reformat(0)

                # ================= the three layers =================
                phase_B(1)
                phase_C(1)
                phase_A(2)
                phase_B(2)
                phase_C(2)
                phase_A(3)
                phase_B(3)
                phase_C(3)

                # ============ conv6 second half + global max pool ============
                for ob in range(8):
                    conv6_chunk(ob, 1)
                    nc.vector.reduce_max(xg_sb[:, ob : ob + 1], xgts[ob],
                                         axis=mybir.AxisListType.X)
                ccg_i = drp.tile([128, 8], f32, tag="cgi", name="ccg_in")
                nc.scalar.dma_start(ccg_i, xg_sb)
                ccg_o = drp.tile([256, 8], f32, tag="cgo", name="ccg_out")
                nc.gpsimd.collective_compute(
                    "AllGather", ALU.bypass, replica_groups=PAIRS,
                    ins=[ccg_i], outs=[ccg_o],
                )

            # layer scratch pools released here; final stage below.
            with tc.tile_pool(name="fin", bufs=1) as fin:
                # conv7 local part without bias (overlaps the AllReduce)
                h7 = fin.tile([128, 4 * H], bf, tag="h7", name="h7")
                for ob in range(4):
                    obs = slice(ob * 128, (ob + 1) * 128)
                    for jp in range(2):
                        pf = psd.tile([128, 1024], f32, tag="d", name="pf7")
                        for q in range(2):
                            sl = slice(jp * 1024 + q * 512,
                                       jp * 1024 + (q + 1) * 512)
                            po = slice(q * 512, (q + 1) * 512)
                            nc.tensor.matmul(pf[:, po], wsb["w7laT"][:, obs],
                                             xcat_a[:, sl],
                                             start=True, stop=False)
                            nc.tensor.matmul(pf[:, po], wsb["w7lbT"][:, obs],
                                             xcat_b[:, sl],
                                             start=False, stop=True)
                        nc.scalar.copy(
                            h7[:, ob * H + jp * 1024 : ob * H + (jp + 1) * 1024],
                            pf,
                        )

                # max of the pair's two xg halves (cheaper than AllReduce,
                # which carries a 1.875x cost multiplier)
                nc.sync.dma_start(xg_sb, ccg_o[0:128])
                xgp = fin.tile([128, 8], f32, tag="xgp", name="xgp")
                nc.sync.dma_start(xgp, ccg_o[128:256])
                nc.vector.tensor_max(xg_sb, xg_sb, xgp)

                # conv7 effective bias: c7 + W7g @ xg
                for ob in range(4):
                    pb = psd.tile([128, 1024], f32, tag="d", name="pb7")
                    for kb in range(8):
                        nc.tensor.matmul(
                            pb[:, 0:1],
                            wsb["w7gT"][:, kb, ob * 128 : (ob + 1) * 128],
                            xg_sb[:, kb : kb + 1],
                            start=(kb == 0),
                            stop=(kb == 7),
                        )
                    nc.scalar.activation(b7_sb[:, ob : ob + 1], pb[:, 0:1],
                                         AF.Identity,
                                         bias=wsb["c7v"][:, ob : ob + 1])

                h7a = fin.tile([128, 4 * H], bf, tag="h7a", name="h7a")
                for jp in range(4):
                    for ob in range(4):
                        sl = slice(ob * H + jp * 512, ob * H + (jp + 1) * 512)
                        nc.scalar.activation(
                            h7a[:, sl], h7[:, sl], AF.Prelu,
                            bias=b7_sb[:, ob : ob + 1], alpha=0.2,
                        )
                h8 = fin.tile([128, 2 * H], bf, tag="h8", name="h8")
                o_sb = fin.tile([13, H], f32, tag="osb", name="o_sb")
                for jp in range(2):
                    for ob in range(2):
                        pf = psd.tile([128, 1024], f32, tag="d", name="pf8")
                        for q in range(2):
                            po = slice(q * 512, (q + 1) * 512)
                            co = jp * 1024 + q * 512
                            for kb in range(4):
                                nc.tensor.matmul(
                                    pf[:, po],
                                    wsb["w8T"][:, kb, ob * 128 : (ob + 1) * 128],
                                    h7a[:, kb * H + co : kb * H + co + 512],
                                    start=(kb == 0),
                                    stop=(kb == 3),
                                )
                        nc.scalar.activation(
                            h8[:, ob * H + jp * 1024 : ob * H + (jp + 1) * 1024],
                            pf, AF.Prelu, bias=wsb["c8v"][:, ob : ob + 1],
                            alpha=0.2,
                        )
                    pf = psd.tile([128, 1024], f32, tag="d", name="pf9")
                    for q in range(2):
                        po = slice(q * 512, (q + 1) * 512)
                        co = jp * 1024 + q * 512
                        for kb in range(2):
                            nc.tensor.matmul(
                                pf[0:13, po],
                                wsb["w9T"][:, kb, :],
                                h8[:, kb * H + co : kb * H + co + 512],
                                start=(kb == 0),
                                stop=(kb == 1),
                            )
                    nc.scalar.activation(
                        o_sb[:, jp * 1024 : (jp + 1) * 1024], pf[0:13],
                        AF.Identity, bias=wsb["b9v"],
                    )
                nc.sync.dma_start(out_d[:], o_sb)

    nc.compile()
    return nc


def _mix(a_steps, b_steps, lead=0):
    """Emit `lead` a-steps, then alternate a/b 1:1 until b is exhausted, then
    the remaining a-steps."""
    ia = 0
    for _ in range(min(lead, len(a_steps))):
        a_steps[ia]()
        ia += 1
    for ib in range(len(b_steps)):
        if ia < len(a_steps):
            a_steps[ia]()
            ia += 1
        b_steps[ib]()
    while ia < len(a_steps):
        a_steps[ia]()
        ia += 1


def make_in_maps(inputs):
    """Per-core input dicts from the full problem inputs."""
    wd = _prep_weights(inputs)
    wpack32 = np.zeros((128, _wcols(0)), dtype=np.float32)
    import ml_dtypes

    wpack16 = np.zeros((128, _wcols(1)), dtype=ml_dtypes.bfloat16)
    col = {0: 0, 1: 0}
    for name, shape, isb in _WEIGHT_SPECS:
        w = int(np.prod(shape[1:])) if len(shape) > 1 else 1
        dst = wpack16 if isb else wpack32
        v = wd[name].reshape(shape[0], w)
        dst[0 : shape[0], col[isb] : col[isb] + w] = (
            _bf16(v) if isb else v.astype(np.float32)
        )
        col[isb] += w
    pts = np.asarray(inputs["points"], dtype=np.float32)
    in_maps = []
    for c in range(8):
        b, h = c // 2, c % 2
        m = {"wpack32": wpack32, "wpack16": wpack16}
        m["wboot"] = np.ascontiguousarray(
            np.concatenate([wd["u1T"], wd["ones3"]], axis=1).astype(np.float32))
        m["choff"] = np.ascontiguousarray(
            np.tile(np.repeat(np.arange(8, dtype=np.uint16) * 512, 8), (128, 1)))
        m["pts_full"] = np.ascontiguousarray(pts[b])
        pm = np.ascontiguousarray(pts[b][:, h * H : (h + 1) * H])
        m["pts_mine"] = pm
        m["pts_mine_bf"] = _bf16(pm)
        in_maps.append(m)
    return in_maps


def kernel(**inputs):
    from concourse.bass_utils import run_bass_kernel_spmd

    if "nc" not in _CACHE:
        _CACHE["nc"] = build_kernel()
    nc = _CACHE["nc"]
    in_maps = make_in_maps(inputs)
    res = run_bass_kernel_spmd(nc, in_maps, core_ids=list(range(8)))
    out = np.zeros((B, 13, N), dtype=np.float32)
    for c in range(8):
        b, h = c // 2, c % 2
        out[b][:, h * H : (h + 1) * H] = res.results[c]["out"]
    return out


# revision 56
# speedup vs baseline: 1.8373x; 1.0099x over previous
"""DGCNN part-segmentation forward pass on 8 Trainium2 NeuronCores.

Sharding: data-parallel over the batch (B=4) x 2-way split of the N=4096
points within each batch element.  Core c handles batch element c//2,
point rows [(c%2)*2048, (c%2+1)*2048).  The two cores of a pair exchange
their half of each EdgeConv output with pairwise AllGathers (one per
1024-point group, launched as soon as that group's output is ready) and
a pairwise AllReduce-max for the global pooling feature.

Precision: the feature path (EdgeConv transforms, convs 6-9) runs in
bfloat16 with fp32 PSUM accumulation.  Layer 1's kNN runs on raw fp32
points (3-D points have many near-ties; bf16 inputs measurably flip
neighbors), via a small separate fp32 lhs/rhs pair.  Layers 2-3 rank
neighbors of the bf16 features exactly: the distance matmul contracts
66 bf16 rows = 64 feature channels + a hi/lo bf16 pair carrying the
fp32 -|x_j|^2 row at ~16-bit precision.  The sq rows travel inside the
AllGather so no distance prep remains on the post-exchange critical
path.

The emission order is software-pipelined: engine queues are in-order,
so each phase leads with a few distance tiles, then runs the edge-conv
phase solid (its AllGather fires early), then the bulk of the
distance/top-k scans; the next layer's first distance quarters overlap
the current layer's second edge phase.

Per EdgeConv layer:
  top-20: per 128-row tile: 8x top-8 per 512-column chunk (DVE Max) ->
          64 candidates; 3 peeling rounds (max8 + match_replace) ->
          top-24 values; threshold mask + prefix-sum compaction +
          gpsimd local_scatter -> global indices of the top-20
  gather: batched GPSIMD ap_gather of the bf16 u = U x transform; the
          first call covers one neighbor slot so the edge pipeline
          starts right after the peels, later calls cover 4 slots
  edge  : psum = I*u_gathered + V*x_i (bf16 matmuls), LeakyReLU (Prelu)
          on the scalar engine, conv2 bf16 matmul; the k-max runs on
          the raw conv2 psums (LeakyReLU is monotone) and one Prelu
          finalizes each group.
"""

import sys

sys.path.insert(0, "/opt/trn_rl_repo")

import numpy as np

B = 4
N = 4096
H = 2048  # points per core (half of a batch element)
KNN = 20
EPS = 1e-5
NEG = -3.0e38
G = KNN * 64  # widx columns per point group

_CACHE = {}


# --------------------------------------------------------------------------
# host-side weight preparation
# --------------------------------------------------------------------------

def _bf16(x):
    import ml_dtypes

    return np.ascontiguousarray(x, dtype=np.float32).astype(ml_dtypes.bfloat16)


def _fold_bn(w, b, g, be):
    s = (g / np.sqrt(np.float32(1.0) + np.float32(EPS))).astype(np.float32)
    return (w * s[:, None]).astype(np.float32), (s * b + be).astype(np.float32)


def _prep_weights(inp):
    f = np.float32
    W, C = {}, {}
    for i in range(1, 9):
        W[i], C[i] = _fold_bn(
            inp["w%d" % i], inp["b%d" % i], inp["g%d" % i], inp["be%d" % i]
        )
    d = {}
    # edge conv layers: split into U (neighbor part) and V (center part).
    # Layer 1's x2r tile holds raw points, layers 2-3 hold 2*x, so the V
    # transpose is halved only for layers 2-3.
    for lid, wi, vscale in ((1, 1, 1.0), (2, 3, 0.5), (3, 5, 0.5)):
        w = W[wi]
        cin = w.shape[1] // 2
        U = w[:, :cin]
        V = w[:, cin:] - w[:, :cin]
        d["u%dT" % lid] = np.ascontiguousarray(U.T)
        d["v%dTh" % lid] = np.ascontiguousarray((V * f(vscale)).T)
        d["c%d" % lid] = C[wi].reshape(64, 1)
    d["w2T"] = np.ascontiguousarray(W[2].T)
    d["cc2"] = C[2].reshape(64, 1)
    d["w4T"] = np.ascontiguousarray(W[4].T)
    d["cc4"] = C[4].reshape(64, 1)
    # conv6 (192 -> 1024)
    w6T = np.ascontiguousarray(W[6].T)  # (192, 1024)
    d["w6aT"] = np.ascontiguousarray(w6T[:128])
    d["w6bT"] = np.ascontiguousarray(w6T[128:])
    d["c6v"] = np.ascontiguousarray(C[6].reshape(8, 128).T)  # (128, 8)
    # conv7 (1216 -> 512): xg part (1024) and local part (192)
    w7 = W[7]
    w7gT = np.ascontiguousarray(w7[:, :1024].T)  # (1024, 512)
    d["w7gT"] = np.ascontiguousarray(w7gT.reshape(8, 128, 512).transpose(1, 0, 2))
    w7lT = np.ascontiguousarray(w7[:, 1024:].T)  # (192, 512)
    d["w7laT"] = np.ascontiguousarray(w7lT[:128])
    d["w7lbT"] = np.ascontiguousarray(w7lT[128:])
    d["c7v"] = np.ascontiguousarray(C[7].reshape(4, 128).T)  # (128, 4)
    # conv8 (512 -> 256)
    w8T = np.ascontiguousarray(W[8].T)  # (512, 256)
    d["w8T"] = np.ascontiguousarray(w8T.reshape(4, 128, 256).transpose(1, 0, 2))
    d["c8v"] = np.ascontiguousarray(C[8].reshape(2, 128).T)  # (128, 2)
    # conv9 (256 -> 13), plain linear
    w9T = np.ascontiguousarray(inp["w9"].T.astype(f))  # (256, 13)
    d["w9T"] = np.ascontiguousarray(w9T.reshape(2, 128, 13).transpose(1, 0, 2))
    d["b9v"] = inp["b9"].astype(f).reshape(13, 1)
    # constants
    d["id64"] = np.eye(64, dtype=f)
    d["ones3"] = np.ones((3, 1), dtype=f)
    d["ones64"] = np.ones((64, 1), dtype=f)
    return d


# name -> (shape, is_bf16)
_WEIGHT_SPECS = [
    ("u1T", (3, 64), 0), ("v1Th", (3, 64), 1), ("c1", (64, 1), 0),
    ("w2T", (64, 64), 1), ("cc2", (64, 1), 0),
    ("u2T", (64, 64), 1), ("v2Th", (64, 64), 1), ("c2", (64, 1), 0),
    ("w4T", (64, 64), 1), ("cc4", (64, 1), 0),
    ("u3T", (64, 64), 1), ("v3Th", (64, 64), 1), ("c3", (64, 1), 0),
    ("w6aT", (128, 1024), 1), ("w6bT", (64, 1024), 1), ("c6v", (128, 8), 0),
    ("w7gT", (128, 8, 512), 0), ("w7laT", (128, 512), 1),
    ("w7lbT", (64, 512), 1), ("c7v", (128, 4), 0),
    ("w8T", (128, 4, 256), 1), ("c8v", (128, 2), 0),
    ("w9T", (128, 2, 13), 1), ("b9v", (13, 1), 0),
    ("id64", (64, 64), 0), ("ones3", (3, 1), 0), ("ones64", (64, 1), 0),
]


def _wcols(bf):
    return sum(int(np.prod(s[1:])) if len(s) > 1 else 1
               for _, s, isb in _WEIGHT_SPECS if isb == bf)


# --------------------------------------------------------------------------
# device kernel builder
# --------------------------------------------------------------------------

def build_kernel():
    import concourse.bacc as bacc
    import concourse.mybir as mybir
    import concourse.tile as tile

    f32 = mybir.dt.float32
    bf = mybir.dt.bfloat16
    i16 = mybir.dt.int16
    u16 = mybir.dt.uint16
    AF = mybir.ActivationFunctionType
    ALU = mybir.AluOpType
    PAIRS = [[0, 1], [2, 3], [4, 5], [6, 7]]

    nc = bacc.Bacc("TRN2", target_bir_lowering=False, num_devices=8)

    pts_full = nc.dram_tensor("pts_full", [3, N], f32, kind="ExternalInput")
    pts_mine = nc.dram_tensor("pts_mine", [3, H], f32, kind="ExternalInput")
    pts_mine_bf = nc.dram_tensor("pts_mine_bf", [3, H], bf,
                                 kind="ExternalInput")
    choff_d = nc.dram_tensor("choff", [128, 64], u16, kind="ExternalInput")
    wboot_d = nc.dram_tensor("wboot", [3, 65], f32, kind="ExternalInput")
    wpack32_d = nc.dram_tensor("wpack32", [128, _wcols(0)], f32,
                               kind="ExternalInput")
    wpack16_d = nc.dram_tensor("wpack16", [128, _wcols(1)], bf,
                               kind="ExternalInput")
    out_d = nc.dram_tensor("out", [13, H], f32, kind="ExternalOutput")

    with tile.TileContext(nc) as tc:
        with (
            tc.tile_pool(name="wp", bufs=1) as wp,
            tc.tile_pool(name="per", bufs=1) as per,
            tc.tile_pool(name="psd", bufs=2, space="PSUM") as psd,
            tc.tile_pool(name="pse", bufs=2, space="PSUM") as pse,
            tc.tile_pool(name="dram", bufs=1, space="DRAM") as drp,
        ):
            # ---- persistent tiles ----
            rhsF = per.tile([66, N], bf, tag="rhsF", name="rhsF")
            rhsF1 = per.tile([4, N], f32, tag="rhsF1", name="rhsF1")
            lhsTm = per.tile([66, H], bf, tag="lhsTm", name="lhsTm")
            lhsTm1 = per.tile([4, H], f32, tag="lhsTm1", name="lhsTm1")
            u_t = per.tile([64, N], f32, tag="u", name="u_t")
            x2r = per.tile([64, H], bf, tag="x2r", name="x2r")
            xcat_a = per.tile([128, H], bf, tag="xcata", name="xcat_a")
            xcat_b = per.tile([64, H], bf, tag="xcatb", name="xcat_b")
            widx = per.tile([64, KNN * 128], i16, tag="widx", name="widx")
            acc3 = per.tile([64, H], f32, tag="acc3", name="acc3")
            xg_sb = per.tile([128, 8], f32, tag="xg", name="xg_sb")
            b7_sb = per.tile([128, 4], f32, tag="b7", name="b7_sb")
            choff_sb = per.tile([128, 64], u16, tag="choff", name="choff_sb")
            zero128 = per.tile([128, 64], f32, tag="z128", name="zero128")

            idx_dram = drp.tile([H, KNN], i16, tag="idxd", name="idx_dram")

            x1h = xcat_a[0:64]
            x2h = xcat_a[64:128]
            x3h = xcat_b

            # critical inputs first; the row-3 ones of lhsTm1 come from a
            # 32-aligned memset that the pts DMA then partially overwrites
            wbt = wp.tile([3, 65], f32, tag="wb", name="wboot_sb")
            nc.sync.dma_start(wbt, wboot_d[:])
            nc.vector.memset(lhsTm1[0:4], 1.0)
            nc.sync.dma_start(lhsTm1[0:3], pts_mine[:])
            nc.sync.dma_start(x2r[0:3], pts_mine_bf[:])
            nc.sync.dma_start(choff_sb, choff_d[:])
            nc.vector.memset(zero128, 0.0)
            nc.vector.memset(lhsTm[64:66], 1.0)
            # all weights arrive in two packed DMAs; per-weight tiles are
            # views into the packed tiles
            wpt32 = wp.tile([128, _wcols(0)], f32, tag="wp32", name="wp32")
            nc.scalar.dma_start(wpt32, wpack32_d[:])
            wpt16 = wp.tile([128, _wcols(1)], bf, tag="wp16", name="wp16")
            nc.scalar.dma_start(wpt16, wpack16_d[:])
            wsb = {}
            col = {0: 0, 1: 0}
            for name, shape, isb in _WEIGHT_SPECS:
                w = int(np.prod(shape[1:])) if len(shape) > 1 else 1
                src = wpt16 if isb else wpt32
                v = src[0 : shape[0], col[isb] : col[isb] + w]
                if len(shape) == 3:
                    v = v.rearrange("p (a b) -> p a b", a=shape[1])
                wsb[name] = v
                col[isb] += w
            wsb["u1T"] = wbt[0:3, 0:64]
            wsb["ones3"] = wbt[0:3, 64:65]

            LAYER = {
                1: dict(cin=3, uT="u1T", vTh="v1Th", c="c1", w2="w2T",
                        cc="cc2"),
                2: dict(cin=64, uT="u2T", vTh="v2Th", c="c2", w2="w4T",
                        cc="cc4"),
                3: dict(cin=64, uT="u3T", vTh="v3Th", c="c3", w2=None,
                        cc=None),
            }
            XOUT = {1: x1h, 2: x2h, 3: x3h}

            with (
                tc.tile_pool(name="dsb", bufs=4) as dsbp,
                tc.tile_pool(name="pp", bufs=2) as ppl,
                tc.tile_pool(name="tk", bufs=2) as tkp,
                tc.tile_pool(name="gp", bufs=2) as gp,
                tc.tile_pool(name="yp", bufs=3) as ypl,
                tc.tile_pool(name="za", bufs=1) as zap,
            ):
                cands = {}

                def prep_block1(lo):
                    """layer-1 lead: rhsF1 = [2*pts; -|pts|^2] and
                    u_t = u1T.T @ pts for columns [lo, lo+1024)."""
                    stage = ppl.tile([3, 1024], f32, tag="pts", name="stage")
                    nc.sync.dma_start(stage, pts_full[:, lo : lo + 1024])
                    nc.scalar.mul(rhsF1[0:3, lo : lo + 1024], stage, 2.0)
                    xsq = ppl.tile([64, 1024], f32, tag="xsq", name="xsq1")
                    nc.scalar.activation(xsq[0:3], stage, AF.Square)
                    sqrow = ppl.tile([1, 1024], f32, tag="sqr", name="sqrow1")
                    for j in range(2):
                        jl = slice(j * 512, (j + 1) * 512)
                        pq = psd.tile([128, 1024], f32, tag="d", name="pq")
                        nc.tensor.matmul(pq[0:1, 0:512], wsb["ones3"],
                                         xsq[0:3, jl])
                        nc.scalar.mul(sqrow[:, jl], pq[0:1, 0:512], -1.0)
                        pu = psd.tile([128, 1024], f32, tag="d", name="pu")
                        nc.tensor.matmul(pu[0:64, 0:512], wsb["u1T"],
                                         stage[:, jl])
                        nc.scalar.copy(u_t[:, lo + j * 512 : lo + (j + 1) * 512],
                                       pu[0:64, 0:512])
                    nc.sync.dma_start(rhsF1[3:4, lo : lo + 1024], sqrow)

                def prep_block(lid, lo):
                    """u_t[:, lo:lo+1024] = uT.T @ rhsF[0:cin] (bf16)."""
                    P = LAYER[lid]
                    cin = P["cin"]
                    for j in range(2):
                        sl = slice(lo + j * 512, lo + (j + 1) * 512)
                        pu = psd.tile([128, 1024], f32, tag="d", name="pu")
                        nc.tensor.matmul(pu[0:64, 0:512], wsb[P["uT"]],
                                         rhsF[0:cin, sl])
                        nc.scalar.copy(u_t[:, sl], pu[0:64, 0:512])

                def dist_quarter(lid, t, blk):
                    """distance psum + copy + top8 scan for tile t, column
                    block blk (1024 cols), ranking for layer lid."""
                    if t not in cands:
                        cands[t] = (
                            tkp.tile([128, 64], f32, tag="cand", name="cand",
                                     bufs=12),
                            tkp.tile([128, 64], u16, tag="cidx", name="cidx",
                                     bufs=12),
                        )
                    cand, cidx = cands[t]
                    lo = blk * 1024
                    pd = psd.tile([128, 1024], f32, tag="d", name="pd")
                    for q in range(2):
                        cs = slice(lo + q * 512, lo + (q + 1) * 512)
                        if lid == 1:
                            nc.tensor.matmul(
                                pd[:, q * 512 : (q + 1) * 512],
                                lhsTm1[:, t * 128 : (t + 1) * 128],
                                rhsF1[:, cs],
                            )
                        else:
                            nc.tensor.matmul(
                                pd[:, q * 512 : (q + 1) * 512],
                                lhsTm[:, t * 128 : (t + 1) * 128],
                                rhsF[:, cs],
                            )
                    dsb = dsbp.tile([128, 1024], f32, tag="dsb", name="dsb")
                    nc.scalar.copy(dsb, pd)
                    for q in range(2):
                        cc = blk * 2 + q
                        nc.vector.max(
                            out=cand[:, cc * 8 : (cc + 1) * 8],
                            in_=dsb[:, q * 512 : (q + 1) * 512],
                        )
                        nc.vector.max_index(
                            cidx[:, cc * 8 : (cc + 1) * 8],
                            cand[:, cc * 8 : (cc + 1) * 8],
                            dsb[:, q * 512 : (q + 1) * 512],
                        )

                def peel_compact(t):
                    """top-20 selection for tile t from its 64 candidates."""
                    cand, cidx = cands.pop(t)
                    nc.vector.tensor_add(cidx, cidx, choff_sb)
                    candw = tkp.tile([128, 64], f32, tag="candw", name="candw")
                    nc.vector.tensor_copy(candw, cand)
                    t8 = tkp.tile([128, 24], f32, tag="t8", name="t8")
                    nc.vector.max(out=t8[:, 0:8], in_=candw)
                    nc.vector.match_replace(out=candw, in_to_replace=t8[:, 0:8],
                                            in_values=candw, imm_value=NEG)
                    nc.vector.max(out=t8[:, 8:16], in_=candw)
                    nc.vector.match_replace(out=candw,
                                            in_to_replace=t8[:, 8:16],
                                            in_values=candw, imm_value=NEG)
                    nc.vector.max(out=t8[:, 16:24], in_=candw)
                    mask = tkp.tile([128, 64], f32, tag="mask", name="mask")
                    nc.vector.tensor_scalar(mask, cand, t8[:, 19:20], None,
                                            op0=ALU.is_ge)
                    cums = tkp.tile([128, 64], f32, tag="cums", name="cums")
                    nc.vector.tensor_tensor_scan(cums, mask, zero128, 0.0,
                                                 op0=ALU.add, op1=ALU.add)
                    # slot = cums*mask - 1  (-1 marks non-selected: ignored
                    # by local_scatter)
                    nc.vector.tensor_mul(cums, cums, mask)
                    nc.vector.tensor_scalar_add(cums, cums, -1.0)
                    slot = tkp.tile([128, 64], i16, tag="slot", name="slot")
                    nc.vector.tensor_copy(slot, cums)
                    sel = tkp.tile([128, 24], u16, tag="sel", name="sel")
                    nc.gpsimd.local_scatter(out_ap=sel, data_ap=cidx,
                                            idxs_ap=slot, channels=128,
                                            num_elems=24, num_idxs=64)
                    nc.sync.dma_start(
                        idx_dram[t * 128 : (t + 1) * 128, :],
                        sel[:, 0:KNN].bitcast(i16),
                    )

                # gather calls: (k0, nk, widx column base within the group).
                # The first call covers a single neighbor slot so the edge
                # pipeline starts as soon as possible after the peels.
                CALLS = [(0, 1, 0), (1, 4, 64), (5, 4, 320), (9, 4, 576),
                         (13, 4, 832), (17, 3, 1088)]

                def reformat(grp):
                    """widx[p, grp*G + base + r*nk + kk]
                         = idx_dram[1024*grp + 16*r + p, k0 + kk],
                    replicated over the four 16-partition groups; one DMA per
                    (gather call, replica) so the first call's indices land
                    first and later transfers overlap the gathers."""
                    for k0, nk, base in CALLS:
                        src = idx_dram[grp * 1024 : (grp + 1) * 1024,
                                       k0 : k0 + nk].rearrange(
                            "(r p) k -> p r k", p=16
                        )
                        for rep in range(4):
                            dst = widx[rep * 16 : (rep + 1) * 16,
                                       grp * G + base
                                       : grp * G + base + 64 * nk].rearrange(
                                "p (r k) -> p r k", k=nk
                            )
                            nc.sync.dma_start(dst, src)

                def gather_call(grp, call):
                    k0, nk, base = CALLS[call]
                    g = gp.tile([64, 4096], f32, tag="g", name="g")
                    nc.gpsimd.ap_gather(
                        out_ap=g[:, 0 : 1024 * nk],
                        in_ap=u_t,
                        idxs_ap=widx[:, grp * G + base
                                     : grp * G + base + 64 * nk],
                        channels=64,
                        num_elems=N,
                        d=1,
                        num_idxs=1024 * nk,
                    )
                    # columns are (r, kk)-interleaved: col = nk*16*r + 16*kk + p
                    return g[:, 0 : 1024 * nk].rearrange(
                        "c (r f p) -> c f r p", f=nk, p=16
                    )

                def edge_k(lid, grp, gv, kk, k, zacc):
                    """one neighbor slot: y = Prelu(I u_j + V x_i + c);
                    z psum = W2 y; zacc = max(zacc, z).  Layer 3 needs only
                    the running max of the gathered u."""
                    P = LAYER[lid]
                    cin = P["cin"]
                    gk = gv[:, kk]  # (64, 64, 16): r-major, p-minor
                    if lid == 3:
                        gsl = slice(grp * 1024, (grp + 1) * 1024)
                        a3 = acc3[:, gsl].rearrange("c (r p) -> c r p", p=16)
                        if k == 0:
                            nc.vector.tensor_copy(a3, gk)
                        else:
                            nc.vector.tensor_max(a3, a3, gk)
                        return
                    for q in range(2):
                        rsl = slice(q * 32, (q + 1) * 32)
                        csl = slice(q * 512, (q + 1) * 512)
                        msl = slice(grp * 1024 + q * 512,
                                    grp * 1024 + (q + 1) * 512)
                        yp = pse.tile([64, 512], f32, tag="e", name="yp")
                        nc.tensor.matmul(yp, wsb["id64"], gk[:, rsl],
                                         start=True, stop=False)
                        nc.tensor.matmul(yp, wsb[P["vTh"]], x2r[0:cin, msl],
                                         start=False, stop=True)
                        y = ypl.tile([64, 512], bf, tag="y", name="y")
                        nc.scalar.activation(y, yp, AF.Prelu, bias=wsb[P["c"]],
                                             alpha=0.2)
                        zp = pse.tile([64, 512], f32, tag="c2", name="zp")
                        nc.tensor.matmul(zp, wsb[P["w2"]], y)
                        if k == 0:
                            nc.vector.tensor_copy(zacc[:, csl], zp)
                        else:
                            nc.vector.tensor_max(zacc[:, csl], zacc[:, csl],
                                                 zp)

                def edge_fin(lid, grp, zacc):
                    P = LAYER[lid]
                    gsl = slice(grp * 1024, (grp + 1) * 1024)
                    x_out = XOUT[lid]
                    if lid == 3:
                        for q in range(2):
                            sl = slice(grp * 1024 + q * 512,
                                       grp * 1024 + (q + 1) * 512)
                            pe_ = pse.tile([64, 512], f32, tag="e", name="pe3")
                            nc.tensor.matmul(pe_, wsb["id64"], acc3[:, sl],
                                             start=True, stop=False)
                            nc.tensor.matmul(pe_, wsb[P["vTh"]],
                                             x2r[0 : P["cin"], sl],
                                             start=False, stop=True)
                            nc.scalar.activation(x_out[:, sl], pe_, AF.Prelu,
                                                 bias=wsb[P["c"]], alpha=0.2)
                    else:
                        nc.scalar.activation(x_out[:, gsl], zacc, AF.Prelu,
                                             bias=wsb[P["cc"]], alpha=0.2)

                ccs = {}

                def exchange_start(lid, grp):
                    """export x_out group plus the hi/lo bf16 pair carrying
                    its fp32 -|x|^2 row, and launch the pairwise AllGather."""
                    gsl = slice(grp * 1024, (grp + 1) * 1024)
                    x_half = XOUT[lid]
                    xsq = ppl.tile([64, 1024], f32, tag="xsq", name="xsqe")
                    nc.scalar.activation(xsq, x_half[:, gsl], AF.Square)
                    sqrow = ppl.tile([1, 1024], f32, tag="sqr", name="sqrowe")
                    for j in range(2):
                        jl = slice(j * 512, (j + 1) * 512)
                        pq = psd.tile([128, 1024], f32, tag="d", name="pqe")
                        nc.tensor.matmul(pq[0:1, 0:512], wsb["ones64"],
                                         xsq[:, jl])
                        nc.scalar.mul(sqrow[:, jl], pq[0:1, 0:512], -1.0)
                    hi = ppl.tile([1, 1024], bf, tag="hi", name="hi")
                    nc.scalar.copy(hi, sqrow)
                    lo = ppl.tile([1, 1024], bf, tag="lo", name="lo")
                    nc.vector.tensor_tensor(lo, sqrow, hi, op=ALU.subtract)
                    ccin = drp.tile([66, 1024], bf, tag="cci%d" % grp,
                                    name="ccin%d" % grp)
                    nc.scalar.dma_start(ccin[0:64], x_half[:, gsl])
                    nc.scalar.dma_start(ccin[64:65], hi)
                    nc.scalar.dma_start(ccin[65:66], lo)
                    ccout = drp.tile([132, 1024], bf, tag="cco%d" % grp,
                                     name="ccout%d" % grp)
                    nc.gpsimd.collective_compute(
                        "AllGather", ALU.bypass, replica_groups=PAIRS,
                        ins=[ccin], outs=[ccout],
                    )
                    return ccout

                def exchange_finish(ccout, grp):
                    lo0 = grp * 1024
                    lo1 = 2048 + grp * 1024
                    nc.sync.dma_start(rhsF[0:64, lo0 : lo0 + 1024], ccout[0:64])
                    nc.sync.dma_start(rhsF[64:66, lo0 : lo0 + 1024],
                                      ccout[64:66])
                    nc.sync.dma_start(rhsF[0:64, lo1 : lo1 + 1024],
                                      ccout[66:130])
                    nc.sync.dma_start(rhsF[64:66, lo1 : lo1 + 1024],
                                      ccout[130:132])

                def lhs_prep(lid, grp):
                    """lhsTm/x2r columns for group grp from layer lid's out."""
                    gsl = slice(grp * 1024, (grp + 1) * 1024)
                    x_out = XOUT[lid]
                    nc.scalar.mul(lhsTm[0:64, gsl], x_out[:, gsl], 2.0)
                    nc.scalar.mul(x2r[0:64, gsl], x_out[:, gsl], 2.0)

                def edge_steps(lid, grp):
                    """emission steps for the edge phase of (lid, grp); the
                    final step also launches the AllGather for layers 1-2."""
                    steps = []
                    state = {}
                    zacc = None
                    if lid != 3:
                        zacc = zap.tile([64, 1024], f32, tag="za", name="zacc")

                    def mk_gather(call):
                        def f():
                            state[call] = gather_call(grp, call)
                        return f

                    def mk_k(call, kk, k):
                        def f():
                            edge_k(lid, grp, state[call], kk, k, zacc)
                        return f

                    def fin():
                        edge_fin(lid, grp, zacc)
                        if lid < 3:
                            ccs[grp] = exchange_start(lid, grp)
                            lhs_prep(lid, grp)

                    for call, (k0, nk, base) in enumerate(CALLS):
                        steps.append(mk_gather(call))
                        for kk in range(nk):
                            steps.append(mk_k(call, kk, k0 + kk))
                    steps.append(fin)
                    return steps

                carry = []

                def phase_B(lid):
                    """grp0 edge (leading, so its AllGather fires early)
                    followed by grp1 tiles' dist/topk."""
                    a = []
                    for t in range(8, 16):
                        for blk in range(4):
                            a.append(lambda t=t, blk=blk:
                                     dist_quarter(lid, t, blk))
                        a.append(lambda t=t: peel_compact(t))
                    for f in a[:14]:
                        f()
                    for f in edge_steps(lid, 0):
                        f()
                    for f in a[14:]:
                        f()
                    reformat(1)

                def phase_C(lid):
                    """grp1 edge followed by the next layer's lead work (or
                    conv6's first half for layer 3)."""
                    nxt = lid + 1 if lid < 3 else None
                    if nxt is not None:
                        exchange_finish(ccs[0], 0)
                    es = edge_steps(lid, 1)
                    if nxt is None:
                        _mix([lambda ob=ob: conv6_chunk(ob, 0)
                              for ob in range(8)], es)
                        return
                    # lead with a few next-layer dist quarters: rhsF for the
                    # g0 blocks landed mid-phase-B, so these are ready and
                    # cover the reformat+gather spin-up of the g1 edge
                    for t in range(6):
                        dist_quarter(nxt, t, 0)
                    for f in es:
                        f()
                    prep_block(nxt, 0)
                    prep_block(nxt, 2048)
                    for t in range(6, 8):
                        dist_quarter(nxt, t, 0)
                    for t in range(3):
                        dist_quarter(nxt, t, 2)
                    for t in range(3, 8):
                        carry.append(lambda t=t: dist_quarter(nxt, t, 2))

                def phase_A(lid):
                    """finish the leftover and second-half dist quarters of
                    tiles 0..8 for layer lid, then peel."""
                    exchange_finish(ccs[1], 1)
                    for f in carry:
                        f()
                    carry.clear()
                    prep_block(lid, 1024)
                    prep_block(lid, 3072)
                    for t in range(8):
                        dist_quarter(lid, t, 1)
                        dist_quarter(lid, t, 3)
                    for t in range(8):
                        peel_compact(t)
                    reformat(0)

                def conv6_chunk(ob, jp):
                    obs = slice(ob * 128, (ob + 1) * 128)
                    pf = psd.tile([128, 1024], f32, tag="d", name="pf6")
                    for q in range(2):
                        sl = slice(jp * 1024 + q * 512,
                                   jp * 1024 + (q + 1) * 512)
                        po = slice(q * 512, (q + 1) * 512)
                        nc.tensor.matmul(pf[:, po], wsb["w6aT"][:, obs],
                                         xcat_a[:, sl], start=True, stop=False)
                        nc.tensor.matmul(pf[:, po], wsb["w6bT"][:, obs],
                                         xcat_b[:, sl], start=False, stop=True)
                    h6 = gp.tile([128, 1024], f32, tag="h6", name="h6", bufs=2)
                    nc.scalar.activation(h6, pf, AF.Prelu,
                                         bias=wsb["c6v"][:, ob : ob + 1],
                                         alpha=0.2)
                    xgt = xgts[ob]
                    nc.vector.reduce_max(xgt[:, jp : jp + 1], h6,
                                         axis=mybir.AxisListType.X)

                xgts = [tkp.tile([128, 2], f32, tag="xgt%d" % ob,
                                 name="xgt%d" % ob) for ob in range(8)]

                # ================= layer 1 lead-in =================
                for blk in range(4):
                    prep_block1(blk * 1024)
                    for t in range(8):
                        dist_quarter(1, t, blk)
                for t in range(8):
                    peel_compact(t)
                reformat(0)

                # ================= the three layers =================
                phase_B(1)
                phase_C(1)
                phase_A(2)
                phase_B(2)
                phase_C(2)
                phase_A(3)
                phase_B(3)
                phase_C(3)

                # ============ conv6 second half + global max pool ============
                for ob in range(8):
                    conv6_chunk(ob, 1)
                    nc.vector.reduce_max(xg_sb[:, ob : ob + 1], xgts[ob],
                                         axis=mybir.AxisListType.X)
                ccg_i = drp.tile([128, 8], f32, tag="cgi", name="ccg_in")
                nc.scalar.dma_start(ccg_i, xg_sb)
                ccg_o = drp.tile([256, 8], f32, tag="cgo", name="ccg_out")
                nc.gpsimd.collective_compute(
                    "AllGather", ALU.bypass, replica_groups=PAIRS,
                    ins=[ccg_i], outs=[ccg_o],
                )

            # layer scratch pools released here; final stage below.
            with tc.tile_pool(name="fin", bufs=1) as fin:
                # conv7 local part without bias (overlaps the AllReduce)
                h7 = fin.tile([128, 4 * H], bf, tag="h7", name="h7")
                for ob in range(4):
                    obs = slice(ob * 128, (ob + 1) * 128)
                    for jp in range(2):
                        pf = psd.tile([128, 1024], f32, tag="d", name="pf7")
                        for q in range(2):
                            sl = slice(jp * 1024 + q * 512,
                                       jp * 1024 + (q + 1) * 512)
                            po = slice(q * 512, (q + 1) * 512)
                            nc.tensor.matmul(pf[:, po], wsb["w7laT"][:, obs],
                                             xcat_a[:, sl],
                                             start=True, stop=False)
                            nc.tensor.matmul(pf[:, po], wsb["w7lbT"][:, obs],
                                             xcat_b[:, sl],
                                             start=False, stop=True)
                        nc.scalar.copy(
                            h7[:, ob * H + jp * 1024 : ob * H + (jp + 1) * 1024],
                            pf,
                        )

                # max of the pair's two xg halves (cheaper than AllReduce,
                # which carries a 1.875x cost multiplier)
                nc.sync.dma_start(xg_sb, ccg_o[0:128])
                xgp = fin.tile([128, 8], f32, tag="xgp", name="xgp")
                nc.sync.dma_start(xgp, ccg_o[128:256])
                nc.vector.tensor_max(xg_sb, xg_sb, xgp)

                # conv7 effective bias: c7 + W7g @ xg
                for ob in range(4):
                    pb = psd.tile([128, 1024], f32, tag="d", name="pb7")
                    for kb in range(8):
                        nc.tensor.matmul(
                            pb[:, 0:1],
                            wsb["w7gT"][:, kb, ob * 128 : (ob + 1) * 128],
                            xg_sb[:, kb : kb + 1],
                            start=(kb == 0),
                            stop=(kb == 7),
                        )
                    nc.scalar.activation(b7_sb[:, ob : ob + 1], pb[:, 0:1],
                                         AF.Identity,
                                         bias=wsb["c7v"][:, ob : ob + 1])

                h7a = fin.tile([128, 4 * H], bf, tag="h7a", name="h7a")
                for jp in range(4):
                    for ob in range(4):
                        sl = slice(ob * H + jp * 512, ob * H + (jp + 1) * 512)
                        nc.scalar.activation(
                            h7a[:, sl], h7[:, sl], AF.Prelu,
                            bias=b7_sb[:, ob : ob + 1], alpha=0.2,
                        )
                h8 = fin.tile([128, 2 * H], bf, tag="h8", name="h8")
                o_sb = fin.tile([13, H], f32, tag="osb", name="o_sb")
                for jp in range(2):
                    for ob in range(2):
                        pf = psd.tile([128, 1024], f32, tag="d", name="pf8")
                        for q in range(2):
                            po = slice(q * 512, (q + 1) * 512)
                            co = jp * 1024 + q * 512
                            for kb in range(4):
                                nc.tensor.matmul(
                                    pf[:, po],
                                    wsb["w8T"][:, kb, ob * 128 : (ob + 1) * 128],
                                    h7a[:, kb * H + co : kb * H + co + 512],
                                    start=(kb == 0),
                                    stop=(kb == 3),
                                )
                        nc.scalar.activation(
                            h8[:, ob * H + jp * 1024 : ob * H + (jp + 1) * 1024],
                            pf, AF.Prelu, bias=wsb["c8v"][:, ob : ob + 1],
                            alpha=0.2,
                        )
                    pf = psd.tile([128, 1024], f32, tag="d", name="pf9")
                    for q in range(2):
                        po = slice(q * 512, (q + 1) * 512)
                        co = jp * 1024 + q * 512
                        for kb in range(2):
                            nc.tensor.matmul(
                                pf[0:13, po],
                                wsb["w9T"][:, kb, :],
                                h8[:, kb * H + co : kb * H + co + 512],
                                start=(kb == 0),
                                stop=(kb == 1),
                            )
                    nc.scalar.activation(
                        o_sb[:, jp * 1024 : (jp + 1) * 1024], pf[0:13],
                        AF.Identity, bias=wsb["b9v"],
                    )
                nc.sync.dma_start(out_d[:], o_sb)

    nc.compile()
    return nc


def _mix(a_steps, b_steps, lead=0):
    """Emit `lead` a-steps, then alternate a/b 1:1 until b is exhausted, then
    the remaining a-steps."""
    ia = 0
    for _ in range(min(lead, len(a_steps))):
        a_steps[ia]()
        ia += 1
    for ib in range(len(b_steps)):
        if ia < len(a_steps):
            a_steps[ia]()
            ia += 1
        b_steps[ib]()
    while ia < len(a_steps):
        a_steps[ia]()
        ia += 1


def make_in_maps(inputs):
    """Per-core input dicts from the full problem inputs."""
    wd = _prep_weights(inputs)
    wpack32 = np.zeros((128, _wcols(0)), dtype=np.float32)
    import ml_dtypes

    wpack16 = np.zeros((128, _wcols(1)), dtype=ml_dtypes.bfloat16)
    col = {0: 0, 1: 0}
    for name, shape, isb in _WEIGHT_SPECS:
        w = int(np.prod(shape[1:])) if len(shape) > 1 else 1
        dst = wpack16 if isb else wpack32
        v = wd[name].reshape(shape[0], w)
        dst[0 : shape[0], col[isb] : col[isb] + w] = (
            _bf16(v) if isb else v.astype(np.float32)
        )
        col[isb] += w
    pts = np.asarray(inputs["points"], dtype=np.float32)
    in_maps = []
    for c in range(8):
        b, h = c // 2, c % 2
        m = {"wpack32": wpack32, "wpack16": wpack16}
        m["wboot"] = np.ascontiguousarray(
            np.concatenate([wd["u1T"], wd["ones3"]], axis=1).astype(np.float32))
        m["choff"] = np.ascontiguousarray(
            np.tile(np.repeat(np.arange(8, dtype=np.uint16) * 512, 8), (128, 1)))
        m["pts_full"] = np.ascontiguousarray(pts[b])
        pm = np.ascontiguousarray(pts[b][:, h * H : (h + 1) * H])
        m["pts_mine"] = pm
        m["pts_mine_bf"] = _bf16(pm)
        in_maps.append(m)
    return in_maps


def kernel(**inputs):
    from concourse.bass_utils import run_bass_kernel_spmd

    if "nc" not in _CACHE:
        _CACHE["nc"] = build_kernel()
    nc = _CACHE["nc"]
    in_maps = make_in_maps(inputs)
    res = run_bass_kernel_spmd(nc, in_maps, core_ids=list(range(8)))
    out = np.zeros((B, 13, N), dtype=np.float32)
    for c in range(8):
        b, h = c // 2, c % 2
        out[b][:, h * H : (h + 1) * H] = res.results[c]["out"]
    return out
